# revision 1
# baseline (speedup 1.0000x reference)
"""GIN 2-layer message-passing network on 8 Trainium2 NeuronCores.

Strategy (matches the dst-partitioned sharding hint):
  - Nodes are split into 8 contiguous chunks of N/8; core c owns chunk c and
    all edges whose destination lands in it. The `+ 1*h_i` self term is NOT
    materialized as self-edges: each destination tile adds its own dense
    feature rows via one identity matmul (saves ~6% of gather slots).
  - segment_sum becomes, per core: dma_gather of source-node feature rows
    (fp16) in edge order (4 SWDGE queues — the gather is descriptor-count
    bound at ~5.3 ns/row, so slot count is the kernel's critical resource),
    then a segment-sum on the tensor engine using one-hot matrices built on
    the vector engine, accumulating in PSUM per 128-destination-node tile:
        aggT[feat, dst] += G[edges, feat].T-contract-. onehot[edges, dst]
  - Edge groups (tile, quarter) are padded to 32 slots, not 128. Groups
    sharing a 128-slot column are disambiguated by a *level*: one-hot
    columns are shifted by 128*level (fp16 iota 0..511), so a full-column
    matmul for one group sees exact zeros on the other groups' rows. This
    avoids PE partition-offset matmuls, which hang real HW.
  - The one-hot depends only on the edge structure, so layer 1 stores it to
    DRAM and layer 2 reloads it with dense DMA instead of re-running the
    (slow, ~76 G elem/s) DVE broadcast compare.
  - The MLP runs in "transposed land" ([feat, nodes] layout) so activations
    never need transposing between matmuls; per tile:
        h = relu(w.T @ aggT + b)  via PE matmul + ACT relu-with-bias.
  - Between the two GIN layers the per-core h chunks are exchanged with 4
    AllGathers (one per quarter of each core's rows) so layer-2 gathers can
    index any node with int16 indices (< 32768 rows per gather source).
  - log_softmax of the final [40, nodes] tile is done after a PE transpose
    back to [nodes, 40]: row-max, subtract, exp-with-accumulated-sum (ACT),
    ln, subtract.

All per-core variability lives in the *data* (index / one-hot-column arrays,
padded to a per-group max across cores) so a single SPMD NEFF serves all 8
cores.
"""

import os
import sys

sys.path.insert(0, "/opt/trn_rl_repo")
sys.path.insert(0, "/opt/trn_rl_repo/concourse")
os.environ.setdefault("TRN_TYPE", "TRN2")

import numpy as np
import ml_dtypes

BF16 = ml_dtypes.bfloat16
F16 = np.float16

NCORES = 8


class Cfg:
    def __init__(self, n, feat, hid, cls, tiles_per_batch=5):
        assert n % (NCORES * 4) == 0
        self.N = n
        self.F = feat          # input feature dim (must be 128 here)
        self.H = hid           # hidden dim (128)
        self.CLS = cls         # classes
        self.NPC = n // NCORES          # nodes per core
        self.QROWS = self.NPC // 4      # rows per quarter per core
        self.SRCROWS = self.QROWS * NCORES  # rows per gather source tensor
        self.NT = -(-self.NPC // 128)   # dst tiles per core
        self.last_rows = self.NPC - (self.NT - 1) * 128
        self.B = tiles_per_batch


FULL = Cfg(100000, 128, 128, 40,
           tiles_per_batch=int(os.environ.get("GIN_B", "5")))


def _prep_graph(edge_index, cfg):
    """Host-side sharding: returns (schedule, per-core index arrays).

    schedule: dict with
      slots[t*4+q]   padded slot count per (tile, quarter) group (max/cores)
      batches        list of lists of tile ids
      call_slots[b][q], call_off[b][q], slot_off maps for emission
    per-core: gidx_wr [128, TOT//16] int16, dstloc [128, TOT//128] bf16
    """
    N, NPC, QROWS, NT = cfg.N, cfg.NPC, cfg.QROWS, cfg.NT
    noself = bool(int(os.environ.get("GIN_NOSELF", "1")))
    pad = int(os.environ.get("GIN_PAD", "32"))
    src = np.asarray(edge_index[0], dtype=np.int64)
    dst = np.asarray(edge_index[1], dtype=np.int64)
    if not noself:
        # self-edges give the +h_i term of the GIN aggregate
        allid = np.arange(N, dtype=np.int64)
        src = np.concatenate([src, allid])
        dst = np.concatenate([dst, allid])

    core = dst // NPC
    per_core = []
    counts = np.zeros((NCORES, NT * 4), np.int64)
    for c in range(NCORES):
        m = core == c
        s = src[m]
        dloc = (dst[m] - c * NPC).astype(np.int64)
        t = dloc >> 7
        q = (s % NPC) // QROWS
        gidxv = (s // NPC) * QROWS + (s % QROWS)
        dstin = dloc & 127
        gid = (t * 4 + q).astype(np.int64)
        counts[c] = np.bincount(gid, minlength=NT * 4)
        per_core.append((gid, gidxv.astype(np.int32), dstin.astype(np.int32)))

    cmax = counts.max(axis=0)                       # [NT*4]
    slots = -(-cmax // pad) * pad                   # slots per (t,q)
    # batches of tiles
    B = cfg.B
    batches = [list(range(b, min(b + B, NT))) for b in range(0, NT, B)]
    # slot offsets in (b, q, t) order; call boundaries stay 128-aligned
    off = 0
    slot_off = np.zeros(NT * 4, np.int64)
    call_slots = []           # [b][q] -> num slots in that gather call
    call_off = []             # [b][q] -> slot offset of call start
    # level disambiguates groups sharing a 128-slot column: one-hot columns
    # are shifted by 128*level, so a full-column matmul for one group sees
    # zeros on the other groups' rows.
    level = np.zeros(NT * 4, np.int64)
    for tiles in batches:
        cs, co = [], []
        for q in range(4):
            co.append(off)
            s0 = off
            for i, t in enumerate(tiles):
                level[t * 4 + q] = i % 4
                slot_off[t * 4 + q] = off
                off += slots[t * 4 + q]
            off = -(-off // 128) * 128      # align next call to 128
            cs.append(off - s0)
        call_slots.append(cs)
        call_off.append(co)
    tot = off
    assert tot % 128 == 0

    gidx_all, dstloc_all = [], []
    for c in range(NCORES):
        gid, gidxv, dstin = per_core[c]
        order = np.argsort(gid, kind="stable")
        gs = gid[order]
        cnt = counts[c]
        starts = np.zeros(NT * 4, np.int64)
        np.cumsum(cnt[:-1], out=starts[1:])
        rank = np.arange(len(gs)) - starts[gs]
        slot = slot_off[gs] + rank
        gflat = np.zeros(tot, np.int16)
        dflat = np.full(tot, 2000.0, np.float32)
        gflat[slot] = gidxv[order].astype(np.int16)
        # one-hot column = dst-in-tile + 128*level of the slot's group
        dflat[slot] = dstin[order] + 128.0 * level[gs]
        # wrap for dma_gather: [p, col] = gflat[col*16 + p%16], replicated x8
        gwr = np.tile(gflat.reshape(tot // 16, 16).T, (8, 1)).copy()
        dloc = dflat.reshape(tot // 128, 128).T.astype(F16).copy()
        gidx_all.append((gwr, gflat.astype(np.int32).reshape(128, tot // 128)))
        dstloc_all.append(dloc)

    sched = dict(slots=slots, batches=batches, call_slots=call_slots,
                 call_off=call_off, slot_off=slot_off, tot=tot,
                 noself=noself, level=level)
    return sched, gidx_all, dstloc_all


def _perm_rows(x, cfg):
    """x [N, F] -> 4 arrays [SRCROWS, F]; source s holds global row
    g = r*NPC + s*QROWS + u at position r*QROWS + u."""
    N, NPC, QROWS = cfg.N, cfg.NPC, cfg.QROWS
    g = np.arange(N)
    s = (g % NPC) // QROWS
    pos = (g // NPC) * QROWS + (g % QROWS)
    out = []
    for si in range(4):
        m = s == si
        a = np.empty((cfg.SRCROWS, x.shape[1]), x.dtype)
        a[pos[m]] = x[m]
        out.append(a)
    return out


def _build_nc(cfg, sched, eps1, eps2):
    from concourse import mybir
    import concourse.bacc as bacc
    import concourse.tile as tile

    F, H, CLS, NT, NPC = cfg.F, cfg.H, cfg.CLS, cfg.NT, cfg.NPC
    slots_arr = sched["slots"]
    slot_off = sched["slot_off"]
    noself = sched["noself"]
    batches = sched["batches"]
    call_slots = sched["call_slots"]
    tot = sched["tot"]
    f32 = mybir.dt.float32
    bf16 = mybir.dt.float16      # data-path dtype (fp16: exact ints to 2048)
    AT = mybir.ActivationFunctionType
    OP = mybir.AluOpType
    level = sched["level"]
    call_off = sched["call_off"]

    # per (b, q): group list [(tile, c0, c1, lvl, ohpos)] with call-relative
    # column ranges; boundary columns appear in two groups.
    groups_bq = {}
    maxOHC = 0
    for b, tiles in enumerate(batches):
        for q in range(4):
            lst, ohpos = [], 0
            for t in tiles:
                ns = int(slots_arr[t * 4 + q])
                if ns == 0:
                    continue
                rel = int(slot_off[t * 4 + q] - call_off[b][q])
                c0, c1 = rel // 128, -(-(rel + ns) // 128)
                lst.append((t, c0, c1, int(level[t * 4 + q]), ohpos))
                ohpos += c1 - c0
            groups_bq[(b, q)] = lst
            maxOHC = max(maxOHC, ohpos)

    assert eps1 == 0.0 and eps2 == 0.0, "nonzero eps not implemented"

    nqueues = int(os.environ.get("GIN_QUEUES", "4"))
    single_packet = bool(int(os.environ.get("GIN_SP", "0")))
    scratch = int(os.environ.get("GIN_SCRATCH", "32768"))
    nc = bacc.Bacc("TRN2", target_bir_lowering=False, debug=False,
                   num_devices=NCORES, num_swdge_queues=nqueues,
                   dynamic_dma_scratch_size=scratch)

    elem = int(os.environ.get("GIN_ELEM", str(F)))  # bench knob: 256/512
    xq = [nc.dram_tensor(f"xq{q}", [cfg.SRCROWS * F // elem, elem], bf16,
                         kind="ExternalInput")
          for q in range(4)]
    w1_t = nc.dram_tensor("w1", [F, H], f32, kind="ExternalInput")
    w2_t = nc.dram_tensor("w2", [H, H], f32, kind="ExternalInput")
    w3_t = nc.dram_tensor("w3", [H, H], f32, kind="ExternalInput")
    w4_t = nc.dram_tensor("w4", [H, CLS], f32, kind="ExternalInput")
    b1_t = nc.dram_tensor("b1", [H, 1], f32, kind="ExternalInput")
    b2_t = nc.dram_tensor("b2", [H, 1], f32, kind="ExternalInput")
    b3_t = nc.dram_tensor("b3", [H, 1], f32, kind="ExternalInput")
    b4_t = nc.dram_tensor("b4", [CLS, 1], f32, kind="ExternalInput")
    xown_t = nc.dram_tensor("xown", [NPC, F], bf16, kind="ExternalInput")
    iota_t = nc.dram_tensor("iota", [128, 512], bf16, kind="ExternalInput")
    ident_t = nc.dram_tensor("ident", [128, 128], f32, kind="ExternalInput")
    gidx_t = nc.dram_tensor("gidx", [128, tot // 16], mybir.dt.int16,
                            kind="ExternalInput")
    indirect = bool(int(os.environ.get("GIN_INDIRECT", "0")))
    gidx32_t = nc.dram_tensor("gidx32", [128, tot // 128], mybir.dt.int32,
                              kind="ExternalInput")
    dstloc_t = nc.dram_tensor("dstloc", [128, tot // 128], bf16,
                              kind="ExternalInput")
    out_t = nc.dram_tensor("out", [NPC, CLS], f32, kind="ExternalOutput")

    maxS = max(max(cs) for cs in call_slots)

    with tile.TileContext(nc) as tc:
        with tc.tile_pool(name="const", bufs=1) as cp, \
             tc.tile_pool(name="gp", bufs=6) as gp, \
             tc.tile_pool(name="ohp", bufs=6) as ohp, \
             tc.tile_pool(name="work", bufs=4) as wp, \
             tc.tile_pool(name="small", bufs=6) as sp, \
             tc.tile_pool(name="aggps", bufs=4, space="PSUM") as aggps, \
             tc.tile_pool(name="mmps", bufs=4, space="PSUM") as mmps, \
             tc.tile_pool(name="dram", bufs=1, space="DRAM") as dp:

            w1 = cp.tile([F, H], f32); nc.sync.dma_start(w1[:], w1_t.ap())
            w2 = cp.tile([H, H], f32); nc.sync.dma_start(w2[:], w2_t.ap())
            w3 = cp.tile([H, H], f32); nc.sync.dma_start(w3[:], w3_t.ap())
            w4 = cp.tile([H, CLS], f32); nc.sync.dma_start(w4[:], w4_t.ap())
            b1 = cp.tile([H, 1], f32); nc.sync.dma_start(b1[:], b1_t.ap())
            b2 = cp.tile([H, 1], f32); nc.sync.dma_start(b2[:], b2_t.ap())
            b3 = cp.tile([H, 1], f32); nc.sync.dma_start(b3[:], b3_t.ap())
            b4 = cp.tile([CLS, 1], f32); nc.sync.dma_start(b4[:], b4_t.ap())
            iota = cp.tile([128, 512], bf16); nc.sync.dma_start(iota[:], iota_t.ap())
            ident = cp.tile([128, 128], f32); nc.sync.dma_start(ident[:], ident_t.ap())
            gidx_sb = cp.tile([128, tot // 16], mybir.dt.int16)
            nc.sync.dma_start(gidx_sb[:], gidx_t.ap())
            gidx32_sb = None
            if indirect:
                gidx32_sb = cp.tile([128, tot // 128], mybir.dt.int32)
                nc.sync.dma_start(gidx32_sb[:], gidx32_t.ap())
            dstloc_sb = cp.tile([128, tot // 128], bf16)
            nc.sync.dma_start(dstloc_sb[:], dstloc_t.ap())

            h_own = dp.tile([NPC, H], bf16)
            oh_reuse = bool(int(os.environ.get("GIN_OH_REUSE", "1")))
            oh_dram = None
            ncalls = len(batches) * 4
            if oh_reuse:
                oh_dram = dp.tile([128, ncalls * maxOHC, 128], bf16)
            # dp.tile (unlike tc.tile) forwards addr_space; Shared is the
            # collectives fast path for HBM-HBM AllGather outputs.
            ag_space = ("Shared" if int(os.environ.get("GIN_AG_SHARED", "0"))
                        else "Local")
            no_ag = bool(os.environ.get("GIN_NO_AG"))
            h_ag = [dp.tile([cfg.SRCROWS, H], bf16, addr_space=ag_space,
                            name=f"h_ag{s}")
                    for s in range(4)]

            skips = set(os.environ.get("GIN_SKIP", "").split(","))
            identb = None
            if noself:
                identb = cp.tile([128, 128], bf16)
                nc.vector.tensor_copy(out=identb[:], in_=ident[:])

            def layer(sources, is_first, batch_limit=None, skip_mlp=False,
                      own_src=None):
                """Emit one GIN layer. sources: list of 4 gather-source APs.
                own_src(t, rows) -> DRAM AP of this tile's own feature rows
                (realizes the +1*h_i self term via an identity matmul)."""
                ag_emitted = [False] * 4
                for b, tiles in enumerate(batches):
                    if batch_limit is not None and b >= batch_limit:
                        break
                    G, OH = [], []
                    for q in range(4):
                        S = call_slots[b][q]
                        if S == 0:
                            G.append(None); OH.append(None)
                            continue
                        o = sched["call_off"][b][q]
                        gi = gidx_sb[:, o // 16:(o + S) // 16]
                        dl = dstloc_sb[:, o // 128:(o + S) // 128]
                        g = gp.tile([128, maxS // 128, elem], bf16, tag="g",
                                    bufs=max(2, 6 * F // elem))
                        if "gather" not in skips:
                            if int(os.environ.get("GIN_TMODE", "0")):
                                gt = gp.tile([128, maxS], bf16, tag="gt",
                                             bufs=2)
                                nc.gpsimd.dma_gather(
                                    gt[:, :S], sources[q], gi, S, S, elem,
                                    transpose=True,
                                    single_packet=single_packet,
                                    queue_num=q % nqueues)
                            elif indirect:
                                from concourse.bass import IndirectOffsetOnAxis
                                nc.gpsimd.indirect_dma_start(
                                    out=g[:, : S // 128, :],
                                    out_offset=None,
                                    in_=sources[q],
                                    in_offset=IndirectOffsetOnAxis(
                                        ap=gidx32_sb[
                                            :, o // 128:(o + S) // 128],
                                        axis=0),
                                )
                            else:
                                nc.gpsimd.dma_gather(
                                    g[:, : S // 128, :], sources[q], gi,
                                    S, S, elem, single_packet=single_packet,
                                    queue_num=q % nqueues)
                        oh = None
                        if "oh" not in skips:
                            glist = groups_bq[(b, q)]
                            nohc = (glist[-1][4] + glist[-1][2] - glist[-1][1]
                                    if glist else 0)
                            oh = ohp.tile([128, maxOHC, 128], bf16, tag="oh")
                            od = b * 4 + q
                            if oh_reuse and not is_first:
                                if nohc:
                                    # ACT-queue DMA keeps the sync queue free
                                    # for stores while the gather stream runs
                                    nc.scalar.dma_start(
                                        oh[:, :nohc, :],
                                        oh_dram[:][:, od * maxOHC:
                                                   od * maxOHC + nohc, :])
                            else:
                                swap = int(os.environ.get("GIN_OH_SWAP", "0"))
                                for (t, c0, c1, lvl, op_) in glist:
                                    n = c1 - c0
                                    ins = [
                                        iota[:, lvl * 128:
                                             (lvl + 1) * 128].unsqueeze(
                                            1).broadcast_to([128, n, 128]),
                                        dl[:, c0:c1].unsqueeze(
                                            2).broadcast_to([128, n, 128]),
                                    ]
                                    nc.vector.tensor_tensor(
                                        out=oh[:, op_:op_ + n, :],
                                        in0=ins[swap], in1=ins[1 - swap],
                                        op=OP.is_equal)
                                if oh_reuse and is_first and nohc:
                                    nc.sync.dma_start(
                                        oh_dram[:][:, od * maxOHC:
                                                   od * maxOHC + nohc, :],
                                        oh[:, :nohc, :])
                        G.append(g); OH.append(oh)

                    if "mm" in skips:
                        continue
                    for t in tiles:
                        # full-column pieces; level-shifted one-hots zero out
                        # the other groups' rows in shared columns
                        pieces = []          # (q, col, ohcol)
                        for q in range(4):
                            for (tt, c0, c1, lvl, op_) in groups_bq[(b, q)]:
                                if tt != t:
                                    continue
                                for j in range(c1 - c0):
                                    pieces.append((q, c0 + j, op_ + j))
                        rows = 128 if t < NT - 1 else cfg.last_rows
                        nmm = len(pieces) + (1 if noself else 0)
                        if nmm == 0:
                            continue
                        agg = aggps.tile([128, 128], f32, tag="agg")
                        k = 0
                        if noself:
                            own = sp.tile([128, 128], bf16, tag="own", bufs=4)
                            nc.scalar.dma_start(own[:rows, :],
                                                own_src(t, rows))
                            nc.tensor.matmul(
                                out=agg[:], lhsT=own[:rows, :],
                                rhs=identb[:rows, :],
                                start=True, stop=(nmm == 1))
                            k += 1
                        for (q, col, ohcol) in pieces:
                            nc.tensor.matmul(
                                out=agg[:],
                                lhsT=G[q][:, col, :],
                                rhs=OH[q][:, ohcol, :],
                                start=(k == 0),
                                stop=(k == nmm - 1))
                            k += 1

                        aggT = wp.tile([128, 128], f32, tag="aggT")
                        nc.scalar.activation(out=aggT[:], in_=agg[:], func=AT.Copy)

                        if skip_mlp:
                            continue
                        if is_first:
                            ps1 = mmps.tile([128, 128], f32, tag="mm")
                            nc.tensor.matmul(out=ps1[:], lhsT=w1[:], rhs=aggT[:],
                                             start=True, stop=True)
                            h1 = wp.tile([128, 128], f32, tag="h1")
                            nc.scalar.activation(out=h1[:], in_=ps1[:],
                                                 func=AT.Relu, bias=b1[:])
                            ps2 = mmps.tile([128, 128], f32, tag="mm")
                            nc.tensor.matmul(out=ps2[:], lhsT=w2[:], rhs=h1[:],
                                             start=True, stop=True)
                            h2 = wp.tile([128, 128], f32, tag="h2")
                            nc.scalar.activation(out=h2[:], in_=ps2[:],
                                                 func=AT.Relu, bias=b2[:])
                            # transpose back to [nodes, feat], cast bf16, store
                            pst = mmps.tile([128, 128], f32, tag="mm")
                            nc.tensor.transpose(out=pst[:], in_=h2[:],
                                                identity=ident[:])
                            hbf = sp.tile([128, 128], bf16, tag="hbf")
                            nc.vector.tensor_copy(out=hbf[:], in_=pst[:])
                            nc.sync.dma_start(
                                h_own[:][t * 128: t * 128 + rows, :],
                                hbf[:rows, :])
                            # fire AllGathers as soon as their rows are done
                            for s in range(4):
                                if not ag_emitted[s] and \
                                        (t + 1) * 128 >= (s + 1) * cfg.QROWS:
                                    ag_emitted[s] = True
                                    if no_ag:
                                        continue
                                    nc.gpsimd.collective_compute(
                                        "AllGather", OP.bypass,
                                        replica_groups=[list(range(NCORES))],
                                        ins=[h_own[:][s * cfg.QROWS:
                                                      (s + 1) * cfg.QROWS, :]],
                                        outs=[h_ag[s][:]])
                        else:
                            ps1 = mmps.tile([128, 128], f32, tag="mm")
                            nc.tensor.matmul(out=ps1[:], lhsT=w3[:], rhs=aggT[:],
                                             start=True, stop=True)
                            h3 = wp.tile([128, 128], f32, tag="h1")
                            nc.scalar.activation(out=h3[:], in_=ps1[:],
                                                 func=AT.Relu, bias=b3[:])
                            ps2 = mmps.tile([128, 128], f32, tag="mm")
                            nc.tensor.matmul(out=ps2[:CLS, :128],
                                             lhsT=w4[:], rhs=h3[:],
                                             start=True, stop=True)
                            c4 = sp.tile([CLS, 128], f32, tag="c4")
                            nc.vector.tensor_tensor(
                                out=c4[:], in0=ps2[:CLS, :128],
                                in1=b4[:].broadcast_to([CLS, 128]),
                                op=OP.add)
                            psf = mmps.tile([128, 128], f32, tag="mm")
                            nc.tensor.transpose(out=psf[:128, :CLS], in_=c4[:],
                                                identity=ident[:CLS, :CLS])
                            mx = sp.tile([128, 1], f32, tag="mx")
                            nc.vector.tensor_reduce(
                                out=mx[:], in_=psf[:128, :CLS],
                                axis=mybir.AxisListType.X, op=OP.max)
                            tsh = sp.tile([128, CLS], f32, tag="tsh")
                            nc.vector.tensor_tensor(
                                out=tsh[:], in0=psf[:128, :CLS],
                                in1=mx[:].broadcast_to([128, CLS]),
                                op=OP.subtract)
                            esum = sp.tile([128, 1], f32, tag="esum")
                            edum = sp.tile([128, CLS], f32, tag="edum")
                            nc.scalar.activation(out=edum[:], in_=tsh[:],
                                                 func=AT.Exp, accum_out=esum[:])
                            lse = sp.tile([128, 1], f32, tag="lse")
                            nc.scalar.activation(out=lse[:], in_=esum[:],
                                                 func=AT.Ln)
                            osb = sp.tile([128, CLS], f32, tag="osb")
                            nc.vector.tensor_tensor(
                                out=osb[:], in0=tsh[:],
                                in1=lse[:].broadcast_to([128, CLS]),
                                op=OP.subtract)
                            nc.sync.dma_start(
                                out_t.ap()[t * 128: t * 128 + rows, :],
                                osb[:rows, :])

            l1b = int(os.environ.get("GIN_L1_BATCHES", "0"))
            repeat = int(os.environ.get("GIN_REPEAT", "1"))

            def own1(t, rows):
                return xown_t.ap()[t * 128: t * 128 + rows, :]

            def own2(t, rows):
                return h_own[:][t * 128: t * 128 + rows, :]

            for _rep in range(repeat):
                if l1b:
                    layer([x.ap() for x in xq], is_first=True,
                          batch_limit=l1b, skip_mlp=True, own_src=own1)
                else:
                    layer([x.ap() for x in xq], is_first=True, own_src=own1)
                    if no_ag:
                        layer([x.ap() for x in xq], is_first=False,
                              own_src=own1)
                    else:
                        layer([h[:] for h in h_ag], is_first=False,
                              own_src=own2)

    nc.compile()
    return nc


LAST_BENCH_NS = None


def _exec_spmd_timed(nc, in_maps, iters):
    """jit-once SPMD exec (mirrors bass2jax.run_bass_via_pjrt) + steady-state
    timing of repeated NEFF executions. Returns per-core result dicts."""
    import time

    import jax
    import numpy as _np
    from jax.experimental.shard_map import shard_map
    from jax.sharding import Mesh, NamedSharding, PartitionSpec

    from concourse import bass2jax, mybir
    from concourse.bass2jax import _bass_exec_p, install_neuronx_cc_hook, \
        partition_id_tensor

    install_neuronx_cc_hook()
    n_cores = len(in_maps)
    partition_name = (nc.partition_id_tensor.name
                      if nc.partition_id_tensor else None)
    in_names, out_names, out_avals, zero_outs = [], [], [], []
    for alloc in nc.m.functions[0].allocations:
        if not isinstance(alloc, mybir.MemoryLocationSet):
            continue
        name = alloc.memorylocations[0].name
        if alloc.kind == "ExternalInput":
            if name != partition_name:
                in_names.append(name)
        elif alloc.kind == "ExternalOutput":
            out_names.append(name)
            shape = tuple(alloc.tensor_shape)
            dtype = mybir.dt.np(alloc.dtype)
            out_avals.append(jax.core.ShapedArray(shape, dtype))
            zero_outs.append(_np.zeros(shape, dtype))
    n_params = len(in_names)
    all_in_names = in_names + out_names
    if partition_name is not None:
        all_in_names = all_in_names + [partition_name]

    chain = int(os.environ.get("GIN_CHAIN", "1"))

    def _body(*args):
        ins = list(args[:n_params])
        outs = list(args[n_params:])
        for _ in range(chain):
            operands = ins + outs
            if partition_name is not None:
                operands.append(partition_id_tensor())
            outs = list(_bass_exec_p.bind(
                *operands,
                out_avals=tuple(out_avals),
                in_names=tuple(all_in_names),
                out_names=tuple(out_names),
                lowering_input_output_aliases=(),
                sim_require_finite=True,
                sim_require_nnan=True,
                nc=nc,
            ))
        return tuple(outs)

    devices = jax.devices()[:n_cores]
    mesh = Mesh(_np.asarray(devices), ("core",))
    spec = PartitionSpec("core")
    n_outs = len(out_names)
    fn = jax.jit(
        shard_map(_body, mesh=mesh, in_specs=(spec,) * (n_params + n_outs),
                  out_specs=(spec,) * n_outs, check_rep=False),
        keep_unused=True,
    )
    sh = NamedSharding(mesh, spec)
    concat_in = [
        jax.device_put(
            _np.concatenate([_np.asarray(in_maps[c][nm]) for c in
                             range(n_cores)], axis=0), sh)
        for nm in in_names
    ]
    concat_zeros = [
        jax.device_put(_np.zeros((n_cores * z.shape[0], *z.shape[1:]),
                                 z.dtype), sh)
        for z in zero_outs
    ]
    out = jax.block_until_ready(fn(*concat_in, *concat_zeros))
    times = []
    for _ in range(max(iters, 1)):
        t0 = time.perf_counter()
        r = jax.block_until_ready(fn(*concat_in, *concat_zeros))
        times.append((time.perf_counter() - t0) * 1e9)
    global LAST_BENCH_NS
    LAST_BENCH_NS = times
    res = []
    for c in range(n_cores):
        res.append({
            nm: _np.asarray(out[i]).reshape(n_cores, *out_avals[i].shape)[c]
            for i, nm in enumerate(out_names)
        })
    return res


def _run(inputs, cfg):
    from concourse.bass_utils import run_bass_kernel_spmd

    x = np.asarray(inputs["x"], np.float32)
    edge_index = np.asarray(inputs["edge_index"])
    eps1 = float(np.asarray(inputs["eps1"]))
    eps2 = float(np.asarray(inputs["eps2"]))

    sched, gidx_all, dstloc_all = _prep_graph(edge_index, cfg)
    xqs = _perm_rows(x.astype(F16), cfg)

    nc = _build_nc(cfg, sched, eps1, eps2)

    iota_np = np.tile(np.arange(512, dtype=np.float32),
                      (128, 1)).astype(F16)
    ident_np = np.eye(128, dtype=np.float32)
    base = {
        "w1": np.asarray(inputs["w1"], np.float32),
        "w2": np.asarray(inputs["w2"], np.float32),
        "w3": np.asarray(inputs["w3"], np.float32),
        "w4": np.asarray(inputs["w4"], np.float32),
        "b1": np.asarray(inputs["b1"], np.float32).reshape(-1, 1),
        "b2": np.asarray(inputs["b2"], np.float32).reshape(-1, 1),
        "b3": np.asarray(inputs["b3"], np.float32).reshape(-1, 1),
        "b4": np.asarray(inputs["b4"], np.float32).reshape(-1, 1),
        "iota": iota_np,
        "ident": ident_np,
    }
    elem = int(os.environ.get("GIN_ELEM", str(cfg.F)))
    shift = (elem // cfg.F).bit_length() - 1
    for q in range(4):
        base[f"xq{q}"] = np.ascontiguousarray(
            xqs[q].reshape(-1, elem))

    xbf = x.astype(F16)
    in_maps = []
    for c in range(NCORES):
        m = dict(base)
        gwr, g32 = gidx_all[c]
        m["gidx"] = gwr >> shift if shift else gwr
        m["gidx32"] = g32 >> shift if shift else g32
        m["dstloc"] = dstloc_all[c]
        m["xown"] = xbf[c * cfg.NPC:(c + 1) * cfg.NPC]
        in_maps.append(m)

    bench = int(os.environ.get("GIN_BENCH", "0"))
    if bench:
        results = _exec_spmd_timed(nc, in_maps, bench)
    else:
        res = run_bass_kernel_spmd(nc, in_maps, core_ids=list(range(NCORES)))
        results = res.results
    out = np.concatenate([r["out"] for r in results], axis=0)
    return out.astype(np.float32)


def kernel(**inputs):
    return _run(inputs, FULL)



# revision 2
# speedup vs baseline: 1.1678x; 1.1678x over previous
"""GIN 2-layer message-passing network on 8 Trainium2 NeuronCores.

Strategy (matches the dst-partitioned sharding hint):
  - Nodes are split into 8 contiguous chunks of N/8; core c owns chunk c and
    all edges whose destination lands in it. The `+ 1*h_i` self term is NOT
    materialized as self-edges: each destination tile adds its own dense
    feature rows via one identity matmul (saves ~6% of gather slots).
  - segment_sum becomes, per core: dma_gather of source-node feature rows
    (fp16) in edge order (4 SWDGE queues — the gather is descriptor-count
    bound at ~5.3 ns/row, so slot count is the kernel's critical resource),
    then a segment-sum on the tensor engine using one-hot matrices built on
    the vector engine, accumulating in PSUM per 128-destination-node tile:
        aggT[feat, dst] += G[edges, feat].T-contract-. onehot[edges, dst]
  - Edge groups (tile, quarter) are padded to 32 slots, not 128. Groups
    sharing a 128-slot column are disambiguated by a *level*: one-hot
    columns are shifted by 128*level (fp16 iota 0..511), so a full-column
    matmul for one group sees exact zeros on the other groups' rows. This
    avoids PE partition-offset matmuls, which hang real HW.
  - The one-hot depends only on the edge structure, so layer 1 stores it to
    DRAM and layer 2 reloads it with dense DMA instead of re-running the
    (slow, ~76 G elem/s) DVE broadcast compare.
  - The MLP runs in "transposed land" ([feat, nodes] layout) so activations
    never need transposing between matmuls; per tile:
        h = relu(w.T @ aggT + b)  via PE matmul + ACT relu-with-bias.
  - Between the two GIN layers the per-core h chunks are exchanged with 4
    AllGathers (one per quarter of each core's rows) so layer-2 gathers can
    index any node with int16 indices (< 32768 rows per gather source).
  - log_softmax of the final [40, nodes] tile is done after a PE transpose
    back to [nodes, 40]: row-max, subtract, exp-with-accumulated-sum (ACT),
    ln, subtract.

All per-core variability lives in the *data* (index / one-hot-column arrays,
padded to a per-group max across cores) so a single SPMD NEFF serves all 8
cores.
"""

import os
import sys

sys.path.insert(0, "/opt/trn_rl_repo")
sys.path.insert(0, "/opt/trn_rl_repo/concourse")
os.environ.setdefault("TRN_TYPE", "TRN2")

import numpy as np
import ml_dtypes

BF16 = ml_dtypes.bfloat16
F16 = np.float16

NCORES = 8


class Cfg:
    def __init__(self, n, feat, hid, cls, tiles_per_batch=5):
        assert n % (NCORES * 4) == 0
        self.N = n
        self.F = feat          # input feature dim (must be 128 here)
        self.H = hid           # hidden dim (128)
        self.CLS = cls         # classes
        self.NPC = n // NCORES          # nodes per core
        self.QROWS = self.NPC // 4      # rows per quarter per core
        self.SRCROWS = self.QROWS * NCORES  # rows per gather source tensor
        self.NT = -(-self.NPC // 128)   # dst tiles per core
        self.last_rows = self.NPC - (self.NT - 1) * 128
        self.B = tiles_per_batch


FULL = Cfg(100000, 128, 128, 40,
           tiles_per_batch=int(os.environ.get("GIN_B", "5")))


def _prep_graph(edge_index, cfg):
    """Host-side sharding: returns (schedule, per-core index arrays).

    schedule: dict with
      slots[t*4+q]   padded slot count per (tile, quarter) group (max/cores)
      batches        list of lists of tile ids
      call_slots[b][q], call_off[b][q], slot_off maps for emission
    per-core: gidx_wr [128, TOT//16] int16, dstloc [128, TOT//128] bf16
    """
    N, NPC, QROWS, NT = cfg.N, cfg.NPC, cfg.QROWS, cfg.NT
    noself = bool(int(os.environ.get("GIN_NOSELF", "1")))
    pad = int(os.environ.get("GIN_PAD", "32"))
    src = np.asarray(edge_index[0], dtype=np.int64)
    dst = np.asarray(edge_index[1], dtype=np.int64)
    if not noself:
        # self-edges give the +h_i term of the GIN aggregate
        allid = np.arange(N, dtype=np.int64)
        src = np.concatenate([src, allid])
        dst = np.concatenate([dst, allid])

    core = dst // NPC
    per_core = []
    counts = np.zeros((NCORES, NT * 4), np.int64)
    for c in range(NCORES):
        m = core == c
        s = src[m]
        dloc = (dst[m] - c * NPC).astype(np.int64)
        t = dloc >> 7
        q = (s % NPC) // QROWS
        gidxv = (s // NPC) * QROWS + (s % QROWS)
        dstin = dloc & 127
        gid = (t * 4 + q).astype(np.int64)
        counts[c] = np.bincount(gid, minlength=NT * 4)
        per_core.append((gid, gidxv.astype(np.int32), dstin.astype(np.int32)))

    cmax = counts.max(axis=0)                       # [NT*4]
    slots = -(-cmax // pad) * pad                   # slots per (t,q)
    # batches of tiles
    B = cfg.B
    batches = [list(range(b, min(b + B, NT))) for b in range(0, NT, B)]
    # slot offsets in (b, q, t) order; call boundaries stay 128-aligned
    off = 0
    slot_off = np.zeros(NT * 4, np.int64)
    call_slots = []           # [b][q] -> num slots in that gather call
    call_off = []             # [b][q] -> slot offset of call start
    # level disambiguates groups sharing a 128-slot column: one-hot columns
    # are shifted by 128*level, so a full-column matmul for one group sees
    # zeros on the other groups' rows.
    level = np.zeros(NT * 4, np.int64)
    for tiles in batches:
        cs, co = [], []
        for q in range(4):
            co.append(off)
            s0 = off
            for i, t in enumerate(tiles):
                level[t * 4 + q] = i % 4
                slot_off[t * 4 + q] = off
                off += slots[t * 4 + q]
            off = -(-off // 128) * 128      # align next call to 128
            cs.append(off - s0)
        call_slots.append(cs)
        call_off.append(co)
    tot = off
    assert tot % 128 == 0

    gidx_all, dstloc_all = [], []
    for c in range(NCORES):
        gid, gidxv, dstin = per_core[c]
        order = np.argsort(gid, kind="stable")
        gs = gid[order]
        cnt = counts[c]
        starts = np.zeros(NT * 4, np.int64)
        np.cumsum(cnt[:-1], out=starts[1:])
        rank = np.arange(len(gs)) - starts[gs]
        slot = slot_off[gs] + rank
        gflat = np.zeros(tot, np.int16)
        dflat = np.full(tot, 2000.0, np.float32)
        gflat[slot] = gidxv[order].astype(np.int16)
        # one-hot column = dst-in-tile + 128*level of the slot's group
        dflat[slot] = dstin[order] + 128.0 * level[gs]
        # wrap for dma_gather: [p, col] = gflat[col*16 + p%16], replicated x8
        gwr = np.tile(gflat.reshape(tot // 16, 16).T, (8, 1)).copy()
        dloc = dflat.reshape(tot // 128, 128).T.astype(F16).copy()
        gidx_all.append((gwr, gflat.astype(np.int32).reshape(128, tot // 128)))
        dstloc_all.append(dloc)

    sched = dict(slots=slots, batches=batches, call_slots=call_slots,
                 call_off=call_off, slot_off=slot_off, tot=tot,
                 noself=noself, level=level)
    return sched, gidx_all, dstloc_all


def _perm_rows(x, cfg):
    """x [N, F] -> 4 arrays [SRCROWS, F]; source s holds global row
    g = r*NPC + s*QROWS + u at position r*QROWS + u."""
    N, NPC, QROWS = cfg.N, cfg.NPC, cfg.QROWS
    g = np.arange(N)
    s = (g % NPC) // QROWS
    pos = (g // NPC) * QROWS + (g % QROWS)
    out = []
    for si in range(4):
        m = s == si
        a = np.empty((cfg.SRCROWS, x.shape[1]), x.dtype)
        a[pos[m]] = x[m]
        out.append(a)
    return out


def _build_nc(cfg, sched, eps1, eps2):
    from concourse import mybir
    import concourse.bacc as bacc
    import concourse.tile as tile

    F, H, CLS, NT, NPC = cfg.F, cfg.H, cfg.CLS, cfg.NT, cfg.NPC
    slots_arr = sched["slots"]
    slot_off = sched["slot_off"]
    noself = sched["noself"]
    batches = sched["batches"]
    call_slots = sched["call_slots"]
    tot = sched["tot"]
    f32 = mybir.dt.float32
    bf16 = mybir.dt.float16      # data-path dtype (fp16: exact ints to 2048)
    AT = mybir.ActivationFunctionType
    OP = mybir.AluOpType
    level = sched["level"]
    call_off = sched["call_off"]

    # per (b, q): group list [(tile, c0, c1, lvl, ohpos)] with call-relative
    # column ranges; boundary columns appear in two groups.
    groups_bq = {}
    maxOHC = 0
    for b, tiles in enumerate(batches):
        for q in range(4):
            lst, ohpos = [], 0
            for t in tiles:
                ns = int(slots_arr[t * 4 + q])
                if ns == 0:
                    continue
                rel = int(slot_off[t * 4 + q] - call_off[b][q])
                c0, c1 = rel // 128, -(-(rel + ns) // 128)
                lst.append((t, c0, c1, int(level[t * 4 + q]), ohpos))
                ohpos += c1 - c0
            groups_bq[(b, q)] = lst
            maxOHC = max(maxOHC, ohpos)

    assert eps1 == 0.0 and eps2 == 0.0, "nonzero eps not implemented"

    nqueues = int(os.environ.get("GIN_QUEUES", "4"))
    single_packet = bool(int(os.environ.get("GIN_SP", "0")))
    scratch = int(os.environ.get("GIN_SCRATCH", "32768"))
    nc = bacc.Bacc("TRN2", target_bir_lowering=False, debug=False,
                   num_devices=NCORES, num_swdge_queues=nqueues,
                   dynamic_dma_scratch_size=scratch)

    elem = int(os.environ.get("GIN_ELEM", str(F)))  # bench knob: 256/512
    xq = [nc.dram_tensor(f"xq{q}", [cfg.SRCROWS * F // elem, elem], bf16,
                         kind="ExternalInput")
          for q in range(4)]
    w1_t = nc.dram_tensor("w1", [F, H], f32, kind="ExternalInput")
    w2_t = nc.dram_tensor("w2", [H, H], f32, kind="ExternalInput")
    w3_t = nc.dram_tensor("w3", [H, H], f32, kind="ExternalInput")
    w4_t = nc.dram_tensor("w4", [H, CLS], f32, kind="ExternalInput")
    b1_t = nc.dram_tensor("b1", [H, 1], f32, kind="ExternalInput")
    b2_t = nc.dram_tensor("b2", [H, 1], f32, kind="ExternalInput")
    b3_t = nc.dram_tensor("b3", [H, 1], f32, kind="ExternalInput")
    b4_t = nc.dram_tensor("b4", [CLS, 1], f32, kind="ExternalInput")
    xown_t = nc.dram_tensor("xown", [NPC, F], bf16, kind="ExternalInput")
    iota_t = nc.dram_tensor("iota", [128, 512], bf16, kind="ExternalInput")
    ident_t = nc.dram_tensor("ident", [128, 128], f32, kind="ExternalInput")
    gidx_t = nc.dram_tensor("gidx", [128, tot // 16], mybir.dt.int16,
                            kind="ExternalInput")
    indirect = bool(int(os.environ.get("GIN_INDIRECT", "0")))
    gidx32_t = nc.dram_tensor("gidx32", [128, tot // 128], mybir.dt.int32,
                              kind="ExternalInput")
    dstloc_t = nc.dram_tensor("dstloc", [128, tot // 128], bf16,
                              kind="ExternalInput")
    out_t = nc.dram_tensor("out", [NPC, CLS], f32, kind="ExternalOutput")

    maxS = max(max(cs) for cs in call_slots)

    with tile.TileContext(nc) as tc:
        with tc.tile_pool(name="const", bufs=1) as cp, \
             tc.tile_pool(name="gp", bufs=6) as gp, \
             tc.tile_pool(name="ohp", bufs=6) as ohp, \
             tc.tile_pool(name="work", bufs=4) as wp, \
             tc.tile_pool(name="small", bufs=6) as sp, \
             tc.tile_pool(name="aggps", bufs=4, space="PSUM") as aggps, \
             tc.tile_pool(name="mmps", bufs=4, space="PSUM") as mmps, \
             tc.tile_pool(name="dram", bufs=1, space="DRAM") as dp:

            w1 = cp.tile([F, H], f32); nc.sync.dma_start(w1[:], w1_t.ap())
            w2 = cp.tile([H, H], f32); nc.sync.dma_start(w2[:], w2_t.ap())
            w3 = cp.tile([H, H], f32); nc.sync.dma_start(w3[:], w3_t.ap())
            w4 = cp.tile([H, CLS], f32); nc.sync.dma_start(w4[:], w4_t.ap())
            b1 = cp.tile([H, 1], f32); nc.sync.dma_start(b1[:], b1_t.ap())
            b2 = cp.tile([H, 1], f32); nc.sync.dma_start(b2[:], b2_t.ap())
            b3 = cp.tile([H, 1], f32); nc.sync.dma_start(b3[:], b3_t.ap())
            b4 = cp.tile([CLS, 1], f32); nc.sync.dma_start(b4[:], b4_t.ap())
            iota = cp.tile([128, 512], bf16); nc.sync.dma_start(iota[:], iota_t.ap())
            ident = cp.tile([128, 128], f32); nc.sync.dma_start(ident[:], ident_t.ap())
            gidx_sb = cp.tile([128, tot // 16], mybir.dt.int16)
            nc.sync.dma_start(gidx_sb[:], gidx_t.ap())
            gidx32_sb = None
            if indirect:
                gidx32_sb = cp.tile([128, tot // 128], mybir.dt.int32)
                nc.sync.dma_start(gidx32_sb[:], gidx32_t.ap())
            dstloc_sb = cp.tile([128, tot // 128], bf16)
            nc.sync.dma_start(dstloc_sb[:], dstloc_t.ap())

            h_own = dp.tile([NPC, H], bf16)
            oh_reuse = bool(int(os.environ.get("GIN_OH_REUSE", "1")))
            oh_dram = None
            ncalls = len(batches) * 4
            if oh_reuse:
                oh_dram = dp.tile([128, ncalls * maxOHC, 128], bf16)
            # dp.tile (unlike tc.tile) forwards addr_space; Shared is the
            # collectives fast path for HBM-HBM AllGather outputs.
            ag_space = ("Shared" if int(os.environ.get("GIN_AG_SHARED", "0"))
                        else "Local")
            no_ag = bool(os.environ.get("GIN_NO_AG"))
            h_ag = [dp.tile([cfg.SRCROWS, H], bf16, addr_space=ag_space,
                            name=f"h_ag{s}")
                    for s in range(4)]

            skips = set(os.environ.get("GIN_SKIP", "").split(","))
            identb = None
            if noself:
                identb = cp.tile([128, 128], bf16)
                nc.vector.tensor_copy(out=identb[:], in_=ident[:])

            def layer(sources, is_first, batch_limit=None, skip_mlp=False,
                      own_src=None):
                """Emit one GIN layer. sources: list of 4 gather-source APs.
                own_src(t, rows) -> DRAM AP of this tile's own feature rows
                (realizes the +1*h_i self term via an identity matmul)."""
                ag_emitted = [False] * 4
                for b, tiles in enumerate(batches):
                    if batch_limit is not None and b >= batch_limit:
                        break
                    G, OH = [], []
                    for q in range(4):
                        S = call_slots[b][q]
                        if S == 0:
                            G.append(None); OH.append(None)
                            continue
                        o = sched["call_off"][b][q]
                        gi = gidx_sb[:, o // 16:(o + S) // 16]
                        dl = dstloc_sb[:, o // 128:(o + S) // 128]
                        g = gp.tile([128, maxS // 128, elem], bf16, tag="g",
                                    bufs=max(2, 6 * F // elem))
                        if "gather" not in skips:
                            if int(os.environ.get("GIN_TMODE", "0")):
                                gt = gp.tile([128, maxS], bf16, tag="gt",
                                             bufs=2)
                                nc.gpsimd.dma_gather(
                                    gt[:, :S], sources[q], gi, S, S, elem,
                                    transpose=True,
                                    single_packet=single_packet,
                                    queue_num=q % nqueues)
                            elif indirect:
                                from concourse.bass import IndirectOffsetOnAxis
                                nc.gpsimd.indirect_dma_start(
                                    out=g[:, : S // 128, :],
                                    out_offset=None,
                                    in_=sources[q],
                                    in_offset=IndirectOffsetOnAxis(
                                        ap=gidx32_sb[
                                            :, o // 128:(o + S) // 128],
                                        axis=0),
                                )
                            else:
                                nc.gpsimd.dma_gather(
                                    g[:, : S // 128, :], sources[q], gi,
                                    S, S, elem, single_packet=single_packet,
                                    queue_num=q % nqueues)
                        oh = None
                        if "oh" not in skips:
                            glist = groups_bq[(b, q)]
                            nohc = (glist[-1][4] + glist[-1][2] - glist[-1][1]
                                    if glist else 0)
                            oh = ohp.tile([128, maxOHC, 128], bf16, tag="oh")
                            od = b * 4 + q
                            if oh_reuse and not is_first:
                                if nohc:
                                    # ACT-queue DMA keeps the sync queue free
                                    # for stores while the gather stream runs
                                    nc.scalar.dma_start(
                                        oh[:, :nohc, :],
                                        oh_dram[:][:, od * maxOHC:
                                                   od * maxOHC + nohc, :])
                            else:
                                swap = int(os.environ.get("GIN_OH_SWAP", "0"))
                                for (t, c0, c1, lvl, op_) in glist:
                                    n = c1 - c0
                                    ins = [
                                        iota[:, lvl * 128:
                                             (lvl + 1) * 128].unsqueeze(
                                            1).broadcast_to([128, n, 128]),
                                        dl[:, c0:c1].unsqueeze(
                                            2).broadcast_to([128, n, 128]),
                                    ]
                                    nc.vector.tensor_tensor(
                                        out=oh[:, op_:op_ + n, :],
                                        in0=ins[swap], in1=ins[1 - swap],
                                        op=OP.is_equal)
                                if oh_reuse and is_first and nohc:
                                    nc.sync.dma_start(
                                        oh_dram[:][:, od * maxOHC:
                                                   od * maxOHC + nohc, :],
                                        oh[:, :nohc, :])
                        G.append(g); OH.append(oh)

                    if "mm" in skips:
                        continue
                    for t in tiles:
                        # full-column pieces; level-shifted one-hots zero out
                        # the other groups' rows in shared columns
                        pieces = []          # (q, col, ohcol)
                        for q in range(4):
                            for (tt, c0, c1, lvl, op_) in groups_bq[(b, q)]:
                                if tt != t:
                                    continue
                                for j in range(c1 - c0):
                                    pieces.append((q, c0 + j, op_ + j))
                        rows = 128 if t < NT - 1 else cfg.last_rows
                        nmm = len(pieces) + (1 if noself else 0)
                        if nmm == 0:
                            continue
                        agg = aggps.tile([128, 128], f32, tag="agg")
                        k = 0
                        if noself:
                            own = sp.tile([128, 128], bf16, tag="own", bufs=4)
                            nc.scalar.dma_start(own[:rows, :],
                                                own_src(t, rows))
                            nc.tensor.matmul(
                                out=agg[:], lhsT=own[:rows, :],
                                rhs=identb[:rows, :],
                                start=True, stop=(nmm == 1))
                            k += 1
                        for (q, col, ohcol) in pieces:
                            nc.tensor.matmul(
                                out=agg[:],
                                lhsT=G[q][:, col, :],
                                rhs=OH[q][:, ohcol, :],
                                start=(k == 0),
                                stop=(k == nmm - 1))
                            k += 1

                        aggT = wp.tile([128, 128], f32, tag="aggT")
                        nc.scalar.activation(out=aggT[:], in_=agg[:], func=AT.Copy)

                        if skip_mlp:
                            continue
                        if is_first:
                            ps1 = mmps.tile([128, 128], f32, tag="mm")
                            nc.tensor.matmul(out=ps1[:], lhsT=w1[:], rhs=aggT[:],
                                             start=True, stop=True)
                            h1 = wp.tile([128, 128], f32, tag="h1")
                            nc.scalar.activation(out=h1[:], in_=ps1[:],
                                                 func=AT.Relu, bias=b1[:])
                            ps2 = mmps.tile([128, 128], f32, tag="mm")
                            nc.tensor.matmul(out=ps2[:], lhsT=w2[:], rhs=h1[:],
                                             start=True, stop=True)
                            h2 = wp.tile([128, 128], f32, tag="h2")
                            nc.scalar.activation(out=h2[:], in_=ps2[:],
                                                 func=AT.Relu, bias=b2[:])
                            # transpose back to [nodes, feat], cast bf16, store
                            pst = mmps.tile([128, 128], f32, tag="mm")
                            nc.tensor.transpose(out=pst[:], in_=h2[:],
                                                identity=ident[:])
                            hbf = sp.tile([128, 128], bf16, tag="hbf")
                            nc.vector.tensor_copy(out=hbf[:], in_=pst[:])
                            nc.sync.dma_start(
                                h_own[:][t * 128: t * 128 + rows, :],
                                hbf[:rows, :])
                            # fire AllGathers as soon as their rows are done
                            for s in range(4):
                                if not ag_emitted[s] and \
                                        (t + 1) * 128 >= (s + 1) * cfg.QROWS:
                                    ag_emitted[s] = True
                                    if no_ag:
                                        continue
                                    nc.gpsimd.collective_compute(
                                        "AllGather", OP.bypass,
                                        replica_groups=[list(range(NCORES))],
                                        ins=[h_own[:][s * cfg.QROWS:
                                                      (s + 1) * cfg.QROWS, :]],
                                        outs=[h_ag[s][:]])
                        else:
                            ps1 = mmps.tile([128, 128], f32, tag="mm")
                            nc.tensor.matmul(out=ps1[:], lhsT=w3[:], rhs=aggT[:],
                                             start=True, stop=True)
                            h3 = wp.tile([128, 128], f32, tag="h1")
                            nc.scalar.activation(out=h3[:], in_=ps1[:],
                                                 func=AT.Relu, bias=b3[:])
                            ps2 = mmps.tile([128, 128], f32, tag="mm")
                            nc.tensor.matmul(out=ps2[:CLS, :128],
                                             lhsT=w4[:], rhs=h3[:],
                                             start=True, stop=True)
                            c4 = sp.tile([CLS, 128], f32, tag="c4")
                            nc.vector.tensor_tensor(
                                out=c4[:], in0=ps2[:CLS, :128],
                                in1=b4[:].broadcast_to([CLS, 128]),
                                op=OP.add)
                            psf = mmps.tile([128, 128], f32, tag="mm")
                            nc.tensor.transpose(out=psf[:128, :CLS], in_=c4[:],
                                                identity=ident[:CLS, :CLS])
                            mx = sp.tile([128, 1], f32, tag="mx")
                            nc.vector.tensor_reduce(
                                out=mx[:], in_=psf[:128, :CLS],
                                axis=mybir.AxisListType.X, op=OP.max)
                            tsh = sp.tile([128, CLS], f32, tag="tsh")
                            nc.vector.tensor_tensor(
                                out=tsh[:], in0=psf[:128, :CLS],
                                in1=mx[:].broadcast_to([128, CLS]),
                                op=OP.subtract)
                            esum = sp.tile([128, 1], f32, tag="esum")
                            edum = sp.tile([128, CLS], f32, tag="edum")
                            nc.scalar.activation(out=edum[:], in_=tsh[:],
                                                 func=AT.Exp, accum_out=esum[:])
                            lse = sp.tile([128, 1], f32, tag="lse")
                            nc.scalar.activation(out=lse[:], in_=esum[:],
                                                 func=AT.Ln)
                            osb = sp.tile([128, CLS], f32, tag="osb")
                            nc.vector.tensor_tensor(
                                out=osb[:], in0=tsh[:],
                                in1=lse[:].broadcast_to([128, CLS]),
                                op=OP.subtract)
                            nc.sync.dma_start(
                                out_t.ap()[t * 128: t * 128 + rows, :],
                                osb[:rows, :])

            l1b = int(os.environ.get("GIN_L1_BATCHES", "0"))
            repeat = int(os.environ.get("GIN_REPEAT", "1"))

            def own1(t, rows):
                return xown_t.ap()[t * 128: t * 128 + rows, :]

            def own2(t, rows):
                return h_own[:][t * 128: t * 128 + rows, :]

            for _rep in range(repeat):
                if l1b:
                    layer([x.ap() for x in xq], is_first=True,
                          batch_limit=l1b, skip_mlp=True, own_src=own1)
                else:
                    layer([x.ap() for x in xq], is_first=True, own_src=own1)
                    if no_ag:
                        layer([x.ap() for x in xq], is_first=False,
                              own_src=own1)
                    else:
                        layer([h[:] for h in h_ag], is_first=False,
                              own_src=own2)

    nc.compile()
    return nc


LAST_BENCH_NS = None


def _exec_spmd_timed(nc, in_maps, iters):
    """jit-once SPMD exec (mirrors bass2jax.run_bass_via_pjrt) + steady-state
    timing of repeated NEFF executions. Returns per-core result dicts."""
    import time

    import jax
    import numpy as _np
    from jax.experimental.shard_map import shard_map
    from jax.sharding import Mesh, NamedSharding, PartitionSpec

    from concourse import bass2jax, mybir
    from concourse.bass2jax import _bass_exec_p, install_neuronx_cc_hook, \
        partition_id_tensor

    install_neuronx_cc_hook()
    n_cores = len(in_maps)
    partition_name = (nc.partition_id_tensor.name
                      if nc.partition_id_tensor else None)
    in_names, out_names, out_avals, zero_outs = [], [], [], []
    for alloc in nc.m.functions[0].allocations:
        if not isinstance(alloc, mybir.MemoryLocationSet):
            continue
        name = alloc.memorylocations[0].name
        if alloc.kind == "ExternalInput":
            if name != partition_name:
                in_names.append(name)
        elif alloc.kind == "ExternalOutput":
            out_names.append(name)
            shape = tuple(alloc.tensor_shape)
            dtype = mybir.dt.np(alloc.dtype)
            out_avals.append(jax.core.ShapedArray(shape, dtype))
            zero_outs.append(_np.zeros(shape, dtype))
    n_params = len(in_names)
    all_in_names = in_names + out_names
    if partition_name is not None:
        all_in_names = all_in_names + [partition_name]

    chain = int(os.environ.get("GIN_CHAIN", "1"))

    def _body(*args):
        ins = list(args[:n_params])
        outs = list(args[n_params:])
        for _ in range(chain):
            operands = ins + outs
            if partition_name is not None:
                operands.append(partition_id_tensor())
            outs = list(_bass_exec_p.bind(
                *operands,
                out_avals=tuple(out_avals),
                in_names=tuple(all_in_names),
                out_names=tuple(out_names),
                lowering_input_output_aliases=(),
                sim_require_finite=True,
                sim_require_nnan=True,
                nc=nc,
            ))
        return tuple(outs)

    devices = jax.devices()[:n_cores]
    mesh = Mesh(_np.asarray(devices), ("core",))
    spec = PartitionSpec("core")
    n_outs = len(out_names)
    fn = jax.jit(
        shard_map(_body, mesh=mesh, in_specs=(spec,) * (n_params + n_outs),
                  out_specs=(spec,) * n_outs, check_rep=False),
        keep_unused=True,
    )
    sh = NamedSharding(mesh, spec)
    concat_in = [
        jax.device_put(
            _np.concatenate([_np.asarray(in_maps[c][nm]) for c in
                             range(n_cores)], axis=0), sh)
        for nm in in_names
    ]
    concat_zeros = [
        jax.device_put(_np.zeros((n_cores * z.shape[0], *z.shape[1:]),
                                 z.dtype), sh)
        for z in zero_outs
    ]
    out = jax.block_until_ready(fn(*concat_in, *concat_zeros))
    times = []
    for _ in range(max(iters, 1)):
        t0 = time.perf_counter()
        r = jax.block_until_ready(fn(*concat_in, *concat_zeros))
        times.append((time.perf_counter() - t0) * 1e9)
    global LAST_BENCH_NS
    LAST_BENCH_NS = times
    res = []
    for c in range(n_cores):
        res.append({
            nm: _np.asarray(out[i]).reshape(n_cores, *out_avals[i].shape)[c]
            for i, nm in enumerate(out_names)
        })
    return res


def _run(inputs, cfg):
    from concourse.bass_utils import run_bass_kernel_spmd

    x = np.asarray(inputs["x"], np.float32)
    edge_index = np.asarray(inputs["edge_index"])
    eps1 = float(np.asarray(inputs["eps1"]))
    eps2 = float(np.asarray(inputs["eps2"]))

    sched, gidx_all, dstloc_all = _prep_graph(edge_index, cfg)
    xqs = _perm_rows(x.astype(F16), cfg)

    nc = _build_nc(cfg, sched, eps1, eps2)

    iota_np = np.tile(np.arange(512, dtype=np.float32),
                      (128, 1)).astype(F16)
    ident_np = np.eye(128, dtype=np.float32)
    base = {
        "w1": np.asarray(inputs["w1"], np.float32),
        "w2": np.asarray(inputs["w2"], np.float32),
        "w3": np.asarray(inputs["w3"], np.float32),
        "w4": np.asarray(inputs["w4"], np.float32),
        "b1": np.asarray(inputs["b1"], np.float32).reshape(-1, 1),
        "b2": np.asarray(inputs["b2"], np.float32).reshape(-1, 1),
        "b3": np.asarray(inputs["b3"], np.float32).reshape(-1, 1),
        "b4": np.asarray(inputs["b4"], np.float32).reshape(-1, 1),
        "iota": iota_np,
        "ident": ident_np,
    }
    elem = int(os.environ.get("GIN_ELEM", str(cfg.F)))
    shift = (elem // cfg.F).bit_length() - 1
    for q in range(4):
        base[f"xq{q}"] = np.ascontiguousarray(
            xqs[q].reshape(-1, elem))

    xbf = x.astype(F16)
    in_maps = []
    for c in range(NCORES):
        m = dict(base)
        gwr, g32 = gidx_all[c]
        m["gidx"] = gwr >> shift if shift else gwr
        m["gidx32"] = g32 >> shift if shift else g32
        m["dstloc"] = dstloc_all[c]
        m["xown"] = xbf[c * cfg.NPC:(c + 1) * cfg.NPC]
        in_maps.append(m)

    bench = int(os.environ.get("GIN_BENCH", "0"))
    trace = int(os.environ.get("GIN_TRACE", "0"))
    if bench:
        results = _exec_spmd_timed(nc, in_maps, bench)
    elif trace:
        tdir = os.environ.get("GIN_TRACE_DIR") or None
        tcores = [int(c) for c in
                  os.environ.get("GIN_TRACE_CORES", "0").split(",")]
        res = run_bass_kernel_spmd(nc, in_maps, core_ids=list(range(NCORES)),
                                   trace=True, tmpdir=tdir,
                                   trace_cores=tcores)
        print("TRACE exec_time_ns:", res.exec_time_ns,
              "mean:", res.mean_exec_time_ns)
        results = res.results
    else:
        res = run_bass_kernel_spmd(nc, in_maps, core_ids=list(range(NCORES)))
        results = res.results
    out = np.concatenate([r["out"] for r in results], axis=0)
    return out.astype(np.float32)


def kernel(**inputs):
    return _run(inputs, FULL)



# revision 25
# speedup vs baseline: 1.4234x; 1.2189x over previous
"""GIN 2-layer message-passing network on 8 Trainium2 NeuronCores.

Strategy (matches the dst-partitioned sharding hint):
  - Nodes are split into 8 contiguous chunks of N/8; core c owns chunk c and
    all edges whose destination lands in it. The `+ 1*h_i` self term is NOT
    materialized as self-edges: each destination tile adds its own dense
    feature rows via one identity matmul (saves ~6% of gather slots).
  - segment_sum becomes, per core: dma_gather of source-node feature rows
    (fp16) in edge order (4 SWDGE queues — the gather is descriptor-count
    bound at ~5.3 ns/row, so slot count is the kernel's critical resource),
    then a segment-sum on the tensor engine using one-hot matrices built on
    the vector engine, accumulating in PSUM per 128-destination-node tile:
        aggT[feat, dst] += G[edges, feat].T-contract-. onehot[edges, dst]
  - Edge groups (tile, quarter) are padded to 32 slots, not 128. Groups
    sharing a 128-slot column are disambiguated by a *level*: one-hot
    columns are shifted by 128*level (fp16 iota 0..511), so a full-column
    matmul for one group sees exact zeros on the other groups' rows. This
    avoids PE partition-offset matmuls, which hang real HW.
  - The one-hot depends only on the edge structure, so layer 1 stores it to
    DRAM and layer 2 reloads it with dense DMA instead of re-running the
    (slow, ~76 G elem/s) DVE broadcast compare.
  - The MLP runs in "transposed land" ([feat, nodes] layout) so activations
    never need transposing between matmuls; per tile:
        h = relu(w.T @ aggT + b)  via PE matmul + ACT relu-with-bias.
  - Between the two GIN layers the per-core h chunks are exchanged with 4
    AllGathers (one per quarter of each core's rows) so layer-2 gathers can
    index any node with int16 indices (< 32768 rows per gather source).
  - log_softmax of the final [40, nodes] tile is done after a PE transpose
    back to [nodes, 40]: row-max, subtract, exp-with-accumulated-sum (ACT),
    ln, subtract.

All per-core variability lives in the *data* (index / one-hot-column arrays,
padded to a per-group max across cores) so a single SPMD NEFF serves all 8
cores.
"""

import os
import sys

sys.path.insert(0, "/opt/trn_rl_repo")
sys.path.insert(0, "/opt/trn_rl_repo/concourse")
os.environ.setdefault("TRN_TYPE", "TRN2")

import numpy as np
import ml_dtypes

BF16 = ml_dtypes.bfloat16
F16 = np.float16

NCORES = 8


class Cfg:
    def __init__(self, n, feat, hid, cls, tiles_per_batch=5):
        assert n % (NCORES * 4) == 0
        self.N = n
        self.F = feat          # input feature dim (must be 128 here)
        self.H = hid           # hidden dim (128)
        self.CLS = cls         # classes
        self.NPC = n // NCORES          # nodes per core
        self.QROWS = self.NPC // 4      # rows per quarter per core
        self.SRCROWS = self.QROWS * NCORES  # rows per gather source tensor
        self.NT = -(-self.NPC // 128)   # dst tiles per core
        self.last_rows = self.NPC - (self.NT - 1) * 128
        self.B = tiles_per_batch


FULL = Cfg(100000, 128, 128, 40,
           tiles_per_batch=int(os.environ.get("GIN_B", "5")))


def _prep_graph(edge_index, cfg):
    """Host-side sharding: returns (schedule, per-core index arrays).

    schedule: dict with
      slots[t*4+q]   padded slot count per (tile, quarter) group (max/cores)
      batches        list of lists of tile ids
      call_slots[b][q], call_off[b][q], slot_off maps for emission
    per-core: gidx_wr [128, TOT//16] int16, dstloc [128, TOT//128] bf16
    """
    N, NPC, QROWS, NT = cfg.N, cfg.NPC, cfg.QROWS, cfg.NT
    noself = bool(int(os.environ.get("GIN_NOSELF", "1")))
    pad = int(os.environ.get("GIN_PAD", "32"))
    src = np.asarray(edge_index[0], dtype=np.int64)
    dst = np.asarray(edge_index[1], dtype=np.int64)
    if not noself:
        # self-edges give the +h_i term of the GIN aggregate
        allid = np.arange(N, dtype=np.int64)
        src = np.concatenate([src, allid])
        dst = np.concatenate([dst, allid])

    core = dst // NPC
    per_core = []
    counts = np.zeros((NCORES, NT * 4), np.int64)
    for c in range(NCORES):
        m = core == c
        s = src[m]
        dloc = (dst[m] - c * NPC).astype(np.int64)
        t = dloc >> 7
        q = (s % NPC) // QROWS
        gidxv = (s // NPC) * QROWS + (s % QROWS)
        dstin = dloc & 127
        gid = (t * 4 + q).astype(np.int64)
        counts[c] = np.bincount(gid, minlength=NT * 4)
        per_core.append((gid, gidxv.astype(np.int32), dstin.astype(np.int32)))

    cmax = counts.max(axis=0)                       # [NT*4]
    slots = -(-cmax // pad) * pad                   # slots per (t,q)
    # batches of tiles
    B = cfg.B
    batches = [list(range(b, min(b + B, NT))) for b in range(0, NT, B)]
    # slot offsets in (b, q, t) order; call boundaries stay 128-aligned
    off = 0
    slot_off = np.zeros(NT * 4, np.int64)
    call_slots = []           # [b][q] -> num slots in that gather call
    call_off = []             # [b][q] -> slot offset of call start
    # level disambiguates groups sharing a 128-slot column: one-hot columns
    # are shifted by 128*level, so a full-column matmul for one group sees
    # zeros on the other groups' rows.
    level = np.zeros(NT * 4, np.int64)
    for tiles in batches:
        cs, co = [], []
        for q in range(4):
            co.append(off)
            s0 = off
            for i, t in enumerate(tiles):
                level[t * 4 + q] = i % 4
                slot_off[t * 4 + q] = off
                off += slots[t * 4 + q]
            off = -(-off // 128) * 128      # align next call to 128
            cs.append(off - s0)
        call_slots.append(cs)
        call_off.append(co)
    tot = off
    assert tot % 128 == 0

    gidx_all, dstloc_all = [], []
    for c in range(NCORES):
        gid, gidxv, dstin = per_core[c]
        order = np.argsort(gid, kind="stable")
        gs = gid[order]
        cnt = counts[c]
        starts = np.zeros(NT * 4, np.int64)
        np.cumsum(cnt[:-1], out=starts[1:])
        rank = np.arange(len(gs)) - starts[gs]
        slot = slot_off[gs] + rank
        gflat = np.zeros(tot, np.int16)
        dflat = np.full(tot, 2000.0, np.float32)
        gflat[slot] = gidxv[order].astype(np.int16)
        # one-hot column = dst-in-tile + 128*level of the slot's group
        dflat[slot] = dstin[order] + 128.0 * level[gs]
        # wrap for dma_gather: [p, col] = gflat[col*16 + p%16], replicated x8
        gwr = np.tile(gflat.reshape(tot // 16, 16).T, (8, 1)).copy()
        dloc = dflat.reshape(tot // 128, 128).T.astype(F16).copy()
        gidx_all.append((gwr, gflat.astype(np.int32).reshape(128, tot // 128)))
        dstloc_all.append(dloc)

    sched = dict(slots=slots, batches=batches, call_slots=call_slots,
                 call_off=call_off, slot_off=slot_off, tot=tot,
                 noself=noself, level=level)
    return sched, gidx_all, dstloc_all


def _perm_rows(x, cfg):
    """x [N, F] -> 4 arrays [SRCROWS, F]; source s holds global row
    g = r*NPC + s*QROWS + u at position r*QROWS + u."""
    N, NPC, QROWS = cfg.N, cfg.NPC, cfg.QROWS
    g = np.arange(N)
    s = (g % NPC) // QROWS
    pos = (g // NPC) * QROWS + (g % QROWS)
    out = []
    for si in range(4):
        m = s == si
        a = np.empty((cfg.SRCROWS, x.shape[1]), x.dtype)
        a[pos[m]] = x[m]
        out.append(a)
    return out


def _groups(sched):
    """Per (b, q): group list [(tile, c0, c1, lvl, ohpos)] with call-relative
    column ranges; boundary columns appear in two groups."""
    slots_arr = sched["slots"]
    slot_off = sched["slot_off"]
    batches = sched["batches"]
    call_off = sched["call_off"]
    level = sched["level"]
    groups_bq = {}
    maxOHC = 0
    for b, tiles in enumerate(batches):
        for q in range(4):
            lst, ohpos = [], 0
            for t in tiles:
                ns = int(slots_arr[t * 4 + q])
                if ns == 0:
                    continue
                rel = int(slot_off[t * 4 + q] - call_off[b][q])
                c0, c1 = rel // 128, -(-(rel + ns) // 128)
                lst.append((t, c0, c1, int(level[t * 4 + q]), ohpos))
                ohpos += c1 - c0
            groups_bq[(b, q)] = lst
            maxOHC = max(maxOHC, ohpos)
    return groups_bq, maxOHC


def _host_onehots(sched, dstloc, np_dt):
    """Host-precomputed one-hot planes, laid out exactly like the device
    oh_dram reuse buffer: [128, ncalls*maxOHC, 128]."""
    groups_bq, maxOHC = _groups(sched)
    batches = sched["batches"]
    call_off = sched["call_off"]
    ncalls = len(batches) * 4
    out = np.zeros((128, ncalls * maxOHC, 128), np_dt)
    cols = np.arange(128, dtype=np.float32)
    dl32 = dstloc.astype(np.float32)
    for b in range(len(batches)):
        for q in range(4):
            od = b * 4 + q
            base = call_off[b][q] // 128
            for (t, c0, c1, lvl, op_) in groups_bq[(b, q)]:
                for j in range(c1 - c0):
                    v = dl32[:, base + c0 + j]
                    out[:, od * maxOHC + op_ + j, :] = (
                        v[:, None] == (128.0 * lvl + cols)).astype(np_dt)
    return out


def _build_nc(cfg, sched, eps1, eps2):
    from concourse import mybir
    import concourse.bacc as bacc
    import concourse.tile as tile

    F, H, CLS, NT, NPC = cfg.F, cfg.H, cfg.CLS, cfg.NT, cfg.NPC
    slots_arr = sched["slots"]
    slot_off = sched["slot_off"]
    noself = sched["noself"]
    batches = sched["batches"]
    call_slots = sched["call_slots"]
    tot = sched["tot"]
    f32 = mybir.dt.float32
    bf16 = mybir.dt.float16      # data-path dtype (fp16: exact ints to 2048)
    AT = mybir.ActivationFunctionType
    OP = mybir.AluOpType
    level = sched["level"]
    call_off = sched["call_off"]

    groups_bq, maxOHC = _groups(sched)

    assert eps1 == 0.0 and eps2 == 0.0, "nonzero eps not implemented"

    nqueues = int(os.environ.get("GIN_QUEUES", "4"))
    single_packet = bool(int(os.environ.get("GIN_SP", "0")))
    scratch = int(os.environ.get("GIN_SCRATCH", "32768"))
    nc = bacc.Bacc("TRN2", target_bir_lowering=False, debug=False,
                   num_devices=NCORES, num_swdge_queues=nqueues,
                   dynamic_dma_scratch_size=scratch)

    oh_host = bool(int(os.environ.get("GIN_OH_HOST", "0")))
    # fp8 one-hots are exact in sim but cost ~2% rel err on HW (the PE
    # appears to quantize the fp16 operand in mixed-dtype mode) -> fp16.
    oh_dt = {"fp8": mybir.dt.float8e4, "fp16": mybir.dt.float16}[
        os.environ.get("GIN_OH_DT", "fp16")]
    qspread = bool(int(os.environ.get("GIN_QSPREAD", "0")))

    elem = int(os.environ.get("GIN_ELEM", str(F)))  # bench knob: 256/512
    xq = [nc.dram_tensor(f"xq{q}", [cfg.SRCROWS * F // elem, elem], bf16,
                         kind="ExternalInput")
          for q in range(4)]
    w1_t = nc.dram_tensor("w1", [F, H], f32, kind="ExternalInput")
    w2_t = nc.dram_tensor("w2", [H, H], f32, kind="ExternalInput")
    w3_t = nc.dram_tensor("w3", [H, H], f32, kind="ExternalInput")
    w4_t = nc.dram_tensor("w4", [H, CLS], f32, kind="ExternalInput")
    b1_t = nc.dram_tensor("b1", [H, 1], f32, kind="ExternalInput")
    b2_t = nc.dram_tensor("b2", [H, 1], f32, kind="ExternalInput")
    b3_t = nc.dram_tensor("b3", [H, 1], f32, kind="ExternalInput")
    b4_t = nc.dram_tensor("b4", [CLS, 1], f32, kind="ExternalInput")
    xown_t = nc.dram_tensor("xown", [NPC, F], bf16, kind="ExternalInput")
    iota_t = nc.dram_tensor("iota", [128, 512], bf16, kind="ExternalInput")
    ident_t = nc.dram_tensor("ident", [128, 128], f32, kind="ExternalInput")
    gidx_t = nc.dram_tensor("gidx", [128, tot // 16], mybir.dt.int16,
                            kind="ExternalInput")
    indirect = bool(int(os.environ.get("GIN_INDIRECT", "0")))
    gidx32_t = nc.dram_tensor("gidx32", [128, tot // 128], mybir.dt.int32,
                              kind="ExternalInput")
    dstloc_t = nc.dram_tensor("dstloc", [128, tot // 128], bf16,
                              kind="ExternalInput")
    ohin_t = None
    if oh_host:
        ncalls_all = len(batches) * 4
        ohin_t = nc.dram_tensor("ohin", [128, ncalls_all * maxOHC, 128],
                                oh_dt, kind="ExternalInput")
    out_t = nc.dram_tensor("out", [NPC, CLS], f32, kind="ExternalOutput")

    maxS = max(max(cs) for cs in call_slots)

    with tile.TileContext(nc) as tc:
        with tc.tile_pool(name="const", bufs=1) as cp, \
             tc.tile_pool(name="gp", bufs=6) as gp, \
             tc.tile_pool(name="ohp", bufs=6) as ohp, \
             tc.tile_pool(name="work", bufs=4) as wp, \
             tc.tile_pool(name="small", bufs=4) as sp, \
             tc.tile_pool(name="aggps", bufs=4, space="PSUM") as aggps, \
             tc.tile_pool(name="mmps", bufs=4, space="PSUM") as mmps, \
             tc.tile_pool(name="dram", bufs=1, space="DRAM") as dp:

            w1 = cp.tile([F, H], f32); nc.sync.dma_start(w1[:], w1_t.ap())
            w2 = cp.tile([H, H], f32); nc.sync.dma_start(w2[:], w2_t.ap())
            w3 = cp.tile([H, H], f32); nc.sync.dma_start(w3[:], w3_t.ap())
            w4 = cp.tile([H, CLS], f32); nc.sync.dma_start(w4[:], w4_t.ap())
            b1 = cp.tile([H, 1], f32); nc.sync.dma_start(b1[:], b1_t.ap())
            b2 = cp.tile([H, 1], f32); nc.sync.dma_start(b2[:], b2_t.ap())
            b3 = cp.tile([H, 1], f32); nc.sync.dma_start(b3[:], b3_t.ap())
            b4 = cp.tile([CLS, 1], f32); nc.sync.dma_start(b4[:], b4_t.ap())
            iota = cp.tile([128, 512], bf16); nc.sync.dma_start(iota[:], iota_t.ap())
            ident = cp.tile([128, 128], f32); nc.sync.dma_start(ident[:], ident_t.ap())
            gidx_sb = cp.tile([128, tot // 16], mybir.dt.int16)
            nc.sync.dma_start(gidx_sb[:], gidx_t.ap())
            gidx32_sb = None
            if indirect:
                gidx32_sb = cp.tile([128, tot // 128], mybir.dt.int32)
                nc.sync.dma_start(gidx32_sb[:], gidx32_t.ap())
            dstloc_sb = cp.tile([128, tot // 128], bf16)
            nc.sync.dma_start(dstloc_sb[:], dstloc_t.ap())

            h_own = dp.tile([NPC, H], bf16)
            # rebuilding the one-hot on DVE each layer overlaps with the
            # gather DMA stream; the DRAM store+reload contends with it
            # (measured: 1.78ms rebuild vs 2.00ms reuse).
            oh_reuse = bool(int(os.environ.get("GIN_OH_REUSE", "0")))
            oh_dram = None
            ncalls = len(batches) * 4
            if oh_reuse and not oh_host:
                oh_dram = dp.tile([128, ncalls * maxOHC, 128], bf16)
            # dp.tile (unlike tc.tile) forwards addr_space; Shared is the
            # collectives fast path for HBM-HBM AllGather outputs.
            ag_space = ("Shared" if int(os.environ.get("GIN_AG_SHARED", "0"))
                        else "Local")
            no_ag = bool(os.environ.get("GIN_NO_AG"))
            h_ag = [dp.tile([cfg.SRCROWS, H], bf16, addr_space=ag_space,
                            name=f"h_ag{s}")
                    for s in range(4)]

            skips = set(os.environ.get("GIN_SKIP", "").split(","))
            identb = None
            if noself:
                identb = cp.tile([128, 128], bf16)
                nc.vector.tensor_copy(out=identb[:], in_=ident[:])

            def layer(sources, is_first, batch_limit=None, skip_mlp=False,
                      own_src=None):
                """Emit one GIN layer. sources: list of 4 gather-source APs.
                own_src(t, rows) -> DRAM AP of this tile's own feature rows
                (realizes the +1*h_i self term via an identity matmul)."""
                ag_emitted = [False] * 4
                for b, tiles in enumerate(batches):
                    if batch_limit is not None and b >= batch_limit:
                        break
                    G, OH = [], []
                    for q in range(4):
                        S = call_slots[b][q]
                        if S == 0:
                            G.append(None); OH.append(None)
                            continue
                        o = sched["call_off"][b][q]
                        gi = gidx_sb[:, o // 16:(o + S) // 16]
                        dl = dstloc_sb[:, o // 128:(o + S) // 128]
                        g = gp.tile([128, maxS // 128, elem], bf16, tag="g",
                                    bufs=max(2, 6 * F // elem))
                        if "gather" not in skips:
                            if int(os.environ.get("GIN_TMODE", "0")):
                                gt = gp.tile([128, maxS], bf16, tag="gt",
                                             bufs=2)
                                nc.gpsimd.dma_gather(
                                    gt[:, :S], sources[q], gi, S, S, elem,
                                    transpose=True,
                                    single_packet=single_packet,
                                    queue_num=((b * 4 + q) if qspread
                                               else q) % nqueues)
                            elif indirect:
                                from concourse.bass import IndirectOffsetOnAxis
                                nc.gpsimd.indirect_dma_start(
                                    out=g[:, : S // 128, :],
                                    out_offset=None,
                                    in_=sources[q],
                                    in_offset=IndirectOffsetOnAxis(
                                        ap=gidx32_sb[
                                            :, o // 128:(o + S) // 128],
                                        axis=0),
                                )
                            else:
                                nc.gpsimd.dma_gather(
                                    g[:, : S // 128, :], sources[q], gi,
                                    S, S, elem, single_packet=single_packet,
                                    queue_num=((b * 4 + q) if qspread
                                               else q) % nqueues)
                        oh = None
                        if "oh" not in skips:
                            glist = groups_bq[(b, q)]
                            nohc = (glist[-1][4] + glist[-1][2] - glist[-1][1]
                                    if glist else 0)
                            oh = ohp.tile([128, maxOHC, 128],
                                          oh_dt if oh_host else bf16, tag="oh")
                            od = b * 4 + q
                            if oh_host:
                                if nohc:
                                    # host-precomputed one-hot planes; ACT
                                    # (HWDGE) queue keeps sync free for stores
                                    nc.scalar.dma_start(
                                        oh[:, :nohc, :],
                                        ohin_t.ap()[:, od * maxOHC:
                                                    od * maxOHC + nohc, :])
                            elif oh_reuse and not is_first:
                                if nohc:
                                    # ACT-queue DMA keeps the sync queue free
                                    # for stores while the gather stream runs
                                    nc.scalar.dma_start(
                                        oh[:, :nohc, :],
                                        oh_dram[:][:, od * maxOHC:
                                                   od * maxOHC + nohc, :])
                            else:
                                swap = int(os.environ.get("GIN_OH_SWAP", "0"))
                                for (t, c0, c1, lvl, op_) in glist:
                                    n = c1 - c0
                                    ins = [
                                        iota[:, lvl * 128:
                                             (lvl + 1) * 128].unsqueeze(
                                            1).broadcast_to([128, n, 128]),
                                        dl[:, c0:c1].unsqueeze(
                                            2).broadcast_to([128, n, 128]),
                                    ]
                                    nc.vector.tensor_tensor(
                                        out=oh[:, op_:op_ + n, :],
                                        in0=ins[swap], in1=ins[1 - swap],
                                        op=OP.is_equal)
                                if oh_reuse and is_first and nohc:
                                    nc.sync.dma_start(
                                        oh_dram[:][:, od * maxOHC:
                                                   od * maxOHC + nohc, :],
                                        oh[:, :nohc, :])
                        G.append(g); OH.append(oh)

                    if "mm" in skips:
                        continue
                    for t in tiles:
                        # full-column pieces; level-shifted one-hots zero out
                        # the other groups' rows in shared columns
                        pieces = []          # (q, col, ohcol)
                        for q in range(4):
                            for (tt, c0, c1, lvl, op_) in groups_bq[(b, q)]:
                                if tt != t:
                                    continue
                                for j in range(c1 - c0):
                                    pieces.append((q, c0 + j, op_ + j))
                        rows = 128 if t < NT - 1 else cfg.last_rows
                        nmm = len(pieces) + (1 if noself else 0)
                        if nmm == 0:
                            continue
                        agg = aggps.tile([128, 128], f32, tag="agg")
                        k = 0
                        if noself:
                            own = sp.tile([128, 128], bf16, tag="own", bufs=4)
                            nc.scalar.dma_start(own[:rows, :],
                                                own_src(t, rows))
                            nc.tensor.matmul(
                                out=agg[:], lhsT=own[:rows, :],
                                rhs=identb[:rows, :],
                                start=True, stop=(nmm == 1))
                            k += 1
                        for (q, col, ohcol) in pieces:
                            nc.tensor.matmul(
                                out=agg[:],
                                lhsT=G[q][:, col, :],
                                rhs=OH[q][:, ohcol, :],
                                start=(k == 0),
                                stop=(k == nmm - 1))
                            k += 1

                        aggT = wp.tile([128, 128], f32, tag="aggT")
                        nc.scalar.activation(out=aggT[:], in_=agg[:], func=AT.Copy)

                        if skip_mlp:
                            continue
                        if is_first:
                            ps1 = mmps.tile([128, 128], f32, tag="mm")
                            nc.tensor.matmul(out=ps1[:], lhsT=w1[:], rhs=aggT[:],
                                             start=True, stop=True)
                            h1 = wp.tile([128, 128], f32, tag="h1")
                            nc.scalar.activation(out=h1[:], in_=ps1[:],
                                                 func=AT.Relu, bias=b1[:])
                            ps2 = mmps.tile([128, 128], f32, tag="mm")
                            nc.tensor.matmul(out=ps2[:], lhsT=w2[:], rhs=h1[:],
                                             start=True, stop=True)
                            h2 = wp.tile([128, 128], f32, tag="h2")
                            nc.scalar.activation(out=h2[:], in_=ps2[:],
                                                 func=AT.Relu, bias=b2[:])
                            # transpose back to [nodes, feat], cast bf16, store
                            pst = mmps.tile([128, 128], f32, tag="mm")
                            nc.tensor.transpose(out=pst[:], in_=h2[:],
                                                identity=ident[:])
                            hbf = sp.tile([128, 128], bf16, tag="hbf")
                            nc.vector.tensor_copy(out=hbf[:], in_=pst[:])
                            nc.sync.dma_start(
                                h_own[:][t * 128: t * 128 + rows, :],
                                hbf[:rows, :])
                            # fire AllGathers as soon as their rows are done
                            for s in range(4):
                                if not ag_emitted[s] and \
                                        (t + 1) * 128 >= (s + 1) * cfg.QROWS:
                                    ag_emitted[s] = True
                                    if no_ag:
                                        continue
                                    nc.gpsimd.collective_compute(
                                        "AllGather", OP.bypass,
                                        replica_groups=[list(range(NCORES))],
                                        ins=[h_own[:][s * cfg.QROWS:
                                                      (s + 1) * cfg.QROWS, :]],
                                        outs=[h_ag[s][:]])
                        else:
                            ps1 = mmps.tile([128, 128], f32, tag="mm")
                            nc.tensor.matmul(out=ps1[:], lhsT=w3[:], rhs=aggT[:],
                                             start=True, stop=True)
                            h3 = wp.tile([128, 128], f32, tag="h1")
                            nc.scalar.activation(out=h3[:], in_=ps1[:],
                                                 func=AT.Relu, bias=b3[:])
                            ps2 = mmps.tile([128, 128], f32, tag="mm")
                            nc.tensor.matmul(out=ps2[:CLS, :128],
                                             lhsT=w4[:], rhs=h3[:],
                                             start=True, stop=True)
                            c4 = sp.tile([CLS, 128], f32, tag="c4")
                            nc.vector.tensor_tensor(
                                out=c4[:], in0=ps2[:CLS, :128],
                                in1=b4[:].broadcast_to([CLS, 128]),
                                op=OP.add)
                            psf = mmps.tile([128, 128], f32, tag="mm")
                            nc.tensor.transpose(out=psf[:128, :CLS], in_=c4[:],
                                                identity=ident[:CLS, :CLS])
                            mx = sp.tile([128, 1], f32, tag="mx")
                            nc.vector.tensor_reduce(
                                out=mx[:], in_=psf[:128, :CLS],
                                axis=mybir.AxisListType.X, op=OP.max)
                            tsh = sp.tile([128, CLS], f32, tag="tsh")
                            nc.vector.tensor_tensor(
                                out=tsh[:], in0=psf[:128, :CLS],
                                in1=mx[:].broadcast_to([128, CLS]),
                                op=OP.subtract)
                            esum = sp.tile([128, 1], f32, tag="esum")
                            edum = sp.tile([128, CLS], f32, tag="edum")
                            nc.scalar.activation(out=edum[:], in_=tsh[:],
                                                 func=AT.Exp, accum_out=esum[:])
                            lse = sp.tile([128, 1], f32, tag="lse")
                            nc.scalar.activation(out=lse[:], in_=esum[:],
                                                 func=AT.Ln)
                            osb = sp.tile([128, CLS], f32, tag="osb")
                            nc.vector.tensor_tensor(
                                out=osb[:], in0=tsh[:],
                                in1=lse[:].broadcast_to([128, CLS]),
                                op=OP.subtract)
                            nc.sync.dma_start(
                                out_t.ap()[t * 128: t * 128 + rows, :],
                                osb[:rows, :])

            l1b = int(os.environ.get("GIN_L1_BATCHES", "0"))
            repeat = int(os.environ.get("GIN_REPEAT", "1"))

            def own1(t, rows):
                return xown_t.ap()[t * 128: t * 128 + rows, :]

            def own2(t, rows):
                return h_own[:][t * 128: t * 128 + rows, :]

            for _rep in range(repeat):
                if l1b:
                    layer([x.ap() for x in xq], is_first=True,
                          batch_limit=l1b, skip_mlp=True, own_src=own1)
                else:
                    layer([x.ap() for x in xq], is_first=True, own_src=own1)
                    if no_ag:
                        layer([x.ap() for x in xq], is_first=False,
                              own_src=own1)
                    else:
                        layer([h[:] for h in h_ag], is_first=False,
                              own_src=own2)

    nc.compile()
    return nc


#  v2: SBUF-resident source tables + transpose-mode SBUF gathers.
#
#  The gather microbench shows SBUF-source dma_gather at ~1.5 ns/desc vs
#  ~3.7 ns/desc for HBM-source (random 256B reads pay an HBM penalty that
#  SRAM does not). v2 therefore keeps the gather sources in SBUF:
#    - Nodes are padded per-core to 12544 (=98*128) "virtual" rows; the
#      node table is split into 4 quarters of 25088 rows. One quarter
#      (49 KiB/partition fp16, wrapped layout: row i -> partition i%128,
#      rank i//128) is resident per pass; each layer runs 4 passes.
#    - SBUF-source gather is transpose-only: output is [feat, slots]. Each
#      128-slot piece is PE-transposed back to [slots, feat] (fp16, PSUM),
#      copied to SBUF (DVE/ACT alternating), then the usual one-hot matmul
#      accumulates agg[feat, dst] in PSUM; pass results accumulate into an
#      SBUF agg buffer (fp16) since PSUM cannot hold all 98 tiles.
#    - The self term rides along as host-injected self-edges (SPMD-uniform;
#      +6% slots but no per-core table indexing).
#    - One-hot planes are host-precomputed (fp8) and streamed from DRAM.
#    - Layer 1 tables load from host-prepared partition-major inputs (one
#      64KB-descriptor DMA per partition); layer 2 tables come from 4
#      range-AllGathers of the partition-major h buffer.

def _npcv(cfg):
    return cfg.NT * 128          # virtual rows per core (full: 12544)


def _sched_from_counts(counts, NT, pad, B):
    """Shared scheduling: per-(tile,quarter) slot padding, batches, call
    offsets, level assignment. counts: [NCORES, NT*4]."""
    cmax = counts.max(axis=0)
    slots = -(-cmax // pad) * pad
    batches = [list(range(b, min(b + B, NT))) for b in range(0, NT, B)]
    off = 0
    slot_off = np.zeros(NT * 4, np.int64)
    call_slots, call_off = [], []
    level = np.zeros(NT * 4, np.int64)
    for tiles in batches:
        cs, co = [], []
        for q in range(4):
            co.append(off)
            s0 = off
            for i, t in enumerate(tiles):
                level[t * 4 + q] = i % 4
                slot_off[t * 4 + q] = off
                off += slots[t * 4 + q]
            off = -(-off // 128) * 128
            cs.append(off - s0)
        call_slots.append(cs)
        call_off.append(co)
    tot = off
    assert tot % 128 == 0
    return dict(slots=slots, batches=batches, call_slots=call_slots,
                call_off=call_off, slot_off=slot_off, tot=tot, level=level,
                noself=True)


def _prep_v2(edge_index, cfg):
    """v2 host prep: self-edges injected, virtual-row quartering, per-core
    gather indices + dstloc (for host one-hots)."""
    N, NPC, NT = cfg.N, cfg.NPC, cfg.NT
    NPCV = _npcv(cfg)
    QRV = 2 * NPCV
    pad = int(os.environ.get("GIN_PAD", "32"))
    B = cfg.B
    src = np.asarray(edge_index[0], dtype=np.int64)
    dst = np.asarray(edge_index[1], dtype=np.int64)
    # self term handled by an identity matmul on the own-rows slice
    v = (src // NPC) * NPCV + (src % NPC)      # virtual source row
    vq = v // QRV
    iq = (v % QRV).astype(np.int32)

    core = dst // NPC
    per_core = []
    counts = np.zeros((NCORES, NT * 4), np.int64)
    for c in range(NCORES):
        m = core == c
        dl = dst[m] - c * NPC
        t = dl >> 7
        gid = (t * 4 + vq[m]).astype(np.int64)
        counts[c] = np.bincount(gid, minlength=NT * 4)
        per_core.append((gid, iq[m], (dl & 127).astype(np.int32)))

    sched = _sched_from_counts(counts, NT, pad, B)
    slot_off, level, tot = sched["slot_off"], sched["level"], sched["tot"]

    gidx_all, dstloc_all = [], []
    for c in range(NCORES):
        gid, gidxv, dstin = per_core[c]
        order = np.argsort(gid, kind="stable")
        gs = gid[order]
        cnt = counts[c]
        starts = np.zeros(NT * 4, np.int64)
        np.cumsum(cnt[:-1], out=starts[1:])
        rank = np.arange(len(gs)) - starts[gs]
        slot = slot_off[gs] + rank
        gflat = np.zeros(tot, np.int16)
        dflat = np.full(tot, 2000.0, np.float32)
        gflat[slot] = gidxv[order].astype(np.int16)
        dflat[slot] = dstin[order] + 128.0 * level[gs]
        gwr = np.tile(gflat.reshape(tot // 16, 16).T, (8, 1)).copy()
        dloc = dflat.reshape(tot // 128, 128).T.astype(F16).copy()
        gidx_all.append(gwr)
        dstloc_all.append(dloc)
    return sched, gidx_all, dstloc_all


def _own_pm(x, cfg):
    """Per-core own rows, wrapped partition-major [128, NT, F] fp16."""
    NPCV = _npcv(cfg)
    out = []
    for c in range(NCORES):
        xv = np.zeros((NPCV, x.shape[1]), F16)
        xv[:cfg.NPC] = x[c * cfg.NPC:(c + 1) * cfg.NPC]
        out.append(np.ascontiguousarray(
            xv.reshape(cfg.NT, 128, -1).transpose(1, 0, 2)))
    return out


def _perm_v2(x, cfg):
    """x [N,128] -> 4 partition-major quarter tables [128, ranks, 128]."""
    NPCV = _npcv(cfg)
    QRV = 2 * NPCV
    xv = np.zeros((NCORES, NPCV, x.shape[1]), F16)
    xv[:, :x.shape[0] // NCORES] = x.reshape(NCORES, -1, x.shape[1])
    xall = xv.reshape(NCORES * NPCV, x.shape[1])
    out = []
    for q in range(4):
        quarter = xall[q * QRV:(q + 1) * QRV]
        out.append(np.ascontiguousarray(
            quarter.reshape(QRV // 128, 128, -1).transpose(1, 0, 2)))
    return out


def _build_nc_v2(cfg, sched, eps1, eps2):
    from concourse import mybir
    import concourse.bacc as bacc
    import concourse.tile as tile

    F, H, CLS, NT, NPC = cfg.F, cfg.H, cfg.CLS, cfg.NT, cfg.NPC
    batches = sched["batches"]
    call_slots = sched["call_slots"]
    call_off = sched["call_off"]
    tot = sched["tot"]
    f32 = mybir.dt.float32
    f16 = mybir.dt.float16
    f8 = mybir.dt.float8e4
    AT = mybir.ActivationFunctionType
    OP = mybir.AluOpType
    assert eps1 == 0.0 and eps2 == 0.0

    groups_bq, maxOHC = _groups(sched)
    nqueues = int(os.environ.get("GIN_QUEUES", "4"))
    single_packet = bool(int(os.environ.get("GIN_SP", "0")))
    scratch = int(os.environ.get("GIN_SCRATCH", "32768"))
    nc = bacc.Bacc("TRN2", target_bir_lowering=False, debug=False,
                   num_devices=NCORES, num_swdge_queues=nqueues,
                   dynamic_dma_scratch_size=scratch)

    RANKS = 2 * NT
    xpm = [nc.dram_tensor(f"xpm{q}", [128, RANKS, F], f16,
                          kind="ExternalInput") for q in range(4)]
    w1_t = nc.dram_tensor("w1", [F, H], f16, kind="ExternalInput")
    w2_t = nc.dram_tensor("w2", [H, H], f16, kind="ExternalInput")
    w3_t = nc.dram_tensor("w3", [H, H], f16, kind="ExternalInput")
    w4_t = nc.dram_tensor("w4", [H, CLS], f16, kind="ExternalInput")
    b1_t = nc.dram_tensor("b1", [H, 1], f32, kind="ExternalInput")
    b2_t = nc.dram_tensor("b2", [H, 1], f32, kind="ExternalInput")
    b3_t = nc.dram_tensor("b3", [H, 1], f32, kind="ExternalInput")
    b4_t = nc.dram_tensor("b4", [CLS, 1], f32, kind="ExternalInput")
    id16_t = nc.dram_tensor("id16", [128, 128], f16, kind="ExternalInput")
    id32_t = nc.dram_tensor("id32", [128, 128], f32, kind="ExternalInput")
    gidx_t = nc.dram_tensor("gidx", [128, tot // 16], mybir.dt.int16,
                            kind="ExternalInput")
    xown_t = nc.dram_tensor("xown", [128, NT, F], f16, kind="ExternalInput")
    ncalls_all = len(batches) * 4
    oh_dt = {"fp8": f8, "fp16": f16}[os.environ.get("GIN_OH_DT", "fp16")]
    ohin_t = nc.dram_tensor("ohin", [128, ncalls_all * maxOHC, 128], oh_dt,
                            kind="ExternalInput")
    out_t = nc.dram_tensor("out", [NPC, CLS], f32, kind="ExternalOutput")

    maxS = max(max(cs) for cs in call_slots)
    NRL = NT                 # ranks per core chunk
    step = -(-NRL // 4)
    AG_RANGES = [(lo, min(lo + step, NRL)) for lo in range(0, NRL, step)]
    last_rows = cfg.last_rows

    with tile.TileContext(nc) as tc:
        with tc.tile_pool(name="const", bufs=1) as cp, \
             tc.tile_pool(name="tbl", bufs=2) as tp, \
             tc.tile_pool(name="gp", bufs=2) as gp, \
             tc.tile_pool(name="ohp", bufs=3) as ohp, \
             tc.tile_pool(name="gsb", bufs=8) as gsp, \
             tc.tile_pool(name="work", bufs=3) as wp, \
             tc.tile_pool(name="small", bufs=4) as sp, \
             tc.tile_pool(name="tps", bufs=4, space="PSUM") as tps, \
             tc.tile_pool(name="aggps", bufs=2, space="PSUM") as aggps, \
             tc.tile_pool(name="mmps", bufs=2, space="PSUM") as mmps, \
             tc.tile_pool(name="dram", bufs=1, space="DRAM") as dp:

            w1 = cp.tile([F, H], f16); nc.sync.dma_start(w1[:], w1_t.ap())
            w2 = cp.tile([H, H], f16); nc.sync.dma_start(w2[:], w2_t.ap())
            w3 = cp.tile([H, H], f16); nc.sync.dma_start(w3[:], w3_t.ap())
            w4 = cp.tile([H, CLS], f16); nc.sync.dma_start(w4[:], w4_t.ap())
            b1 = cp.tile([H, 1], f32); nc.sync.dma_start(b1[:], b1_t.ap())
            b2 = cp.tile([H, 1], f32); nc.sync.dma_start(b2[:], b2_t.ap())
            b3 = cp.tile([H, 1], f32); nc.sync.dma_start(b3[:], b3_t.ap())
            b4 = cp.tile([CLS, 1], f32); nc.sync.dma_start(b4[:], b4_t.ap())
            id16 = cp.tile([128, 128], f16)
            nc.sync.dma_start(id16[:], id16_t.ap())
            id32 = cp.tile([128, 128], f32)
            nc.sync.dma_start(id32[:], id32_t.ap())
            gidx_sb = cp.tile([128, tot // 16], mybir.dt.int16)
            nc.sync.dma_start(gidx_sb[:], gidx_t.ap())
            zt = cp.tile([128, 128], f16)
            nc.vector.memset(zt[:], 0.0)

            agg_sb = cp.tile([128, NT, 128], f16)
            h_own = dp.tile([128, NRL * 128], f16)
            ag_space = ("Shared" if int(os.environ.get("GIN_AG_SHARED", "0"))
                        else "Local")
            h_ag = [dp.tile([NCORES * 128, (hi - lo) * 128], f16,
                            addr_space=ag_space, name=f"hag{j}")
                    for j, (lo, hi) in enumerate(AG_RANGES)]

            def load1(qq, table):
                nc.sync.dma_start(table[:], xpm[qq].ap())

            def load2(qq, table):
                for j, (lo, hi) in enumerate(AG_RANGES):
                    for k in range(2):
                        core = 2 * qq + k
                        nc.sync.dma_start(
                            table[:, k * NRL + lo: k * NRL + hi, :],
                            h_ag[j][:][core * 128:(core + 1) * 128, :])

            def layer(loader, own_of, is_first):
                for qq in range(4):
                    table = tp.tile([128, RANKS, F], f16, tag="tbl")
                    loader(qq, table)
                    for b, tiles in enumerate(batches):
                        S = call_slots[b][qq]
                        o = call_off[b][qq]
                        gt = None
                        oh = None
                        if S:
                            gt = gp.tile([128, 1, maxS], f16, tag="gt")
                            nc.gpsimd.dma_gather(
                                gt[:, :, :S], table[:],
                                gidx_sb[:, o // 16:(o + S) // 16], S, S, F,
                                transpose=True, single_packet=single_packet,
                                queue_num=b % nqueues,
                                sbuf_tokens_per_rank=128,
                                sbuf_free_dim_per_rank=256)
                            glist = groups_bq[(b, qq)]
                            nohc = (glist[-1][4] + glist[-1][2] - glist[-1][1]
                                    if glist else 0)
                            if nohc:
                                oh = ohp.tile([128, maxOHC, 128], oh_dt,
                                              tag="oh")
                                od = b * 4 + qq
                                nc.scalar.dma_start(
                                    oh[:, :nohc, :],
                                    ohin_t.ap()[:, od * maxOHC:
                                                od * maxOHC + nohc, :])
                        for t in tiles:
                            pieces = []
                            for (tt, c0, c1, lvl, op_) in groups_bq[(b, qq)]:
                                if tt != t:
                                    continue
                                for j in range(c1 - c0):
                                    pieces.append((c0 + j, op_ + j))
                            if qq == 0:
                                # self term: (1+eps)*h_t via identity matmul
                                agg = aggps.tile([128, 128], f32, tag="agg")
                                own = sp.tile([128, 128], f16, tag="own")
                                nc.scalar.dma_start(own[:], own_of(t))
                                nc.tensor.matmul(
                                    out=agg[:], lhsT=own[:], rhs=id16[:],
                                    start=True, stop=(len(pieces) == 0))
                                kbase = 1
                            else:
                                if not pieces:
                                    continue
                                agg = aggps.tile([128, 128], f32, tag="agg")
                                kbase = 0
                            for k, (col, ohcol) in enumerate(pieces):
                                pst = tps.tile([128, 128], f16, tag="pst")
                                nc.tensor.transpose(
                                    out=pst[:],
                                    in_=gt[:, 0, col * 128:(col + 1) * 128],
                                    identity=id16[:])
                                g2 = gsp.tile([128, 128], f16, tag="g2")
                                if k % 2 == 0:
                                    nc.vector.tensor_copy(out=g2[:],
                                                          in_=pst[:])
                                else:
                                    nc.scalar.activation(out=g2[:],
                                                         in_=pst[:],
                                                         func=AT.Copy)
                                nc.tensor.matmul(
                                    out=agg[:], lhsT=g2[:],
                                    rhs=oh[:, ohcol, :],
                                    start=(kbase == 0 and k == 0),
                                    stop=(k == len(pieces) - 1))
                            if qq == 0:
                                nc.vector.tensor_copy(out=agg_sb[:, t, :],
                                                      in_=agg[:])
                            else:
                                nc.vector.tensor_tensor(
                                    out=agg_sb[:, t, :], in0=agg[:],
                                    in1=agg_sb[:, t, :], op=OP.add)

                # MLP over all tiles from the SBUF agg buffer
                for t in range(NT):
                    rows = 128 if t < NT - 1 else last_rows
                    if is_first:
                        ps1 = mmps.tile([128, 128], f32, tag="mm")
                        nc.tensor.matmul(out=ps1[:], lhsT=w1[:],
                                         rhs=agg_sb[:, t, :],
                                         start=True, stop=True)
                        h1 = wp.tile([128, 128], f16, tag="h1")
                        nc.scalar.activation(out=h1[:], in_=ps1[:],
                                             func=AT.Relu, bias=b1[:])
                        ps2 = mmps.tile([128, 128], f32, tag="mm")
                        nc.tensor.matmul(out=ps2[:], lhsT=w2[:], rhs=h1[:],
                                         start=True, stop=True)
                        h2 = wp.tile([128, 128], f16, tag="h2")
                        nc.scalar.activation(out=h2[:], in_=ps2[:],
                                             func=AT.Relu, bias=b2[:])
                        pst = tps.tile([128, 128], f16, tag="pst")
                        nc.tensor.transpose(out=pst[:], in_=h2[:],
                                            identity=id16[:])
                        hst = sp.tile([128, 128], f16, tag="hst")
                        nc.vector.tensor_copy(out=hst[:], in_=pst[:])
                        nc.sync.dma_start(
                            h_own[:][:, t * 128:(t + 1) * 128], hst[:])
                        if t == NT - 1 and last_rows < 128:
                            # zero the pad rows (virtual >= NPC)
                            nc.sync.dma_start(
                                h_own[:][last_rows:128,
                                         t * 128:(t + 1) * 128],
                                zt[:128 - last_rows, :])
                        for j, (lo, hi) in enumerate(AG_RANGES):
                            if t == hi - 1:
                                nc.gpsimd.collective_compute(
                                    "AllGather", OP.bypass,
                                    replica_groups=[list(range(NCORES))],
                                    ins=[h_own[:][:, lo * 128:hi * 128]],
                                    outs=[h_ag[j][:]])
                    else:
                        ps1 = mmps.tile([128, 128], f32, tag="mm")
                        nc.tensor.matmul(out=ps1[:], lhsT=w3[:],
                                         rhs=agg_sb[:, t, :],
                                         start=True, stop=True)
                        h3 = wp.tile([128, 128], f16, tag="h1")
                        nc.scalar.activation(out=h3[:], in_=ps1[:],
                                             func=AT.Relu, bias=b3[:])
                        ps2 = mmps.tile([128, 128], f32, tag="mm")
                        nc.tensor.matmul(out=ps2[:CLS, :128],
                                         lhsT=w4[:], rhs=h3[:],
                                         start=True, stop=True)
                        c4 = sp.tile([CLS, 128], f32, tag="c4")
                        nc.vector.tensor_tensor(
                            out=c4[:], in0=ps2[:CLS, :128],
                            in1=b4[:].broadcast_to([CLS, 128]), op=OP.add)
                        psf = mmps.tile([128, 128], f32, tag="mm")
                        nc.tensor.transpose(out=psf[:128, :CLS], in_=c4[:],
                                            identity=id32[:CLS, :CLS])
                        mx = sp.tile([128, 1], f32, tag="mx")
                        nc.vector.tensor_reduce(
                            out=mx[:], in_=psf[:128, :CLS],
                            axis=mybir.AxisListType.X, op=OP.max)
                        tsh = sp.tile([128, CLS], f32, tag="tsh")
                        nc.vector.tensor_tensor(
                            out=tsh[:], in0=psf[:128, :CLS],
                            in1=mx[:].broadcast_to([128, CLS]),
                            op=OP.subtract)
                        esum = sp.tile([128, 1], f32, tag="esum")
                        edum = sp.tile([128, CLS], f32, tag="edum")
                        nc.scalar.activation(out=edum[:], in_=tsh[:],
                                             func=AT.Exp, accum_out=esum[:])
                        lse = sp.tile([128, 1], f32, tag="lse")
                        nc.scalar.activation(out=lse[:], in_=esum[:],
                                             func=AT.Ln)
                        osb = sp.tile([128, CLS], f32, tag="osb")
                        nc.vector.tensor_tensor(
                            out=osb[:], in0=tsh[:],
                            in1=lse[:].broadcast_to([128, CLS]),
                            op=OP.subtract)
                        nc.sync.dma_start(
                            out_t.ap()[t * 128: t * 128 + rows, :],
                            osb[:rows, :])

            repeat = int(os.environ.get("GIN_REPEAT", "1"))
            for _rep in range(repeat):
                layer(load1, lambda t: xown_t.ap()[:, t, :], is_first=True)
                layer(load2,
                      lambda t: h_own[:][:, t * 128:(t + 1) * 128],
                      is_first=False)

    nc.compile()
    return nc


def _run_v2(inputs, cfg):
    from concourse.bass_utils import run_bass_kernel_spmd

    x = np.asarray(inputs["x"], np.float32)
    edge_index = np.asarray(inputs["edge_index"])
    eps1 = float(np.asarray(inputs["eps1"]))
    eps2 = float(np.asarray(inputs["eps2"]))

    sched, gidx_all, dstloc_all = _prep_v2(edge_index, cfg)
    nc = _build_nc_v2(cfg, sched, eps1, eps2)

    xq = _perm_v2(x.astype(F16), cfg)
    np_ohdt = {"fp8": ml_dtypes.float8_e4m3,
               "fp16": np.float16}[os.environ.get("GIN_OH_DT", "fp16")]
    base = {
        "w1": np.asarray(inputs["w1"], np.float16),
        "w2": np.asarray(inputs["w2"], np.float16),
        "w3": np.asarray(inputs["w3"], np.float16),
        "w4": np.asarray(inputs["w4"], np.float16),
        "b1": np.asarray(inputs["b1"], np.float32).reshape(-1, 1),
        "b2": np.asarray(inputs["b2"], np.float32).reshape(-1, 1),
        "b3": np.asarray(inputs["b3"], np.float32).reshape(-1, 1),
        "b4": np.asarray(inputs["b4"], np.float32).reshape(-1, 1),
        "id16": np.eye(128, dtype=np.float16),
        "id32": np.eye(128, dtype=np.float32),
    }
    for q in range(4):
        base[f"xpm{q}"] = xq[q]
    xown = _own_pm(x.astype(F16), cfg)

    in_maps = []
    for c in range(NCORES):
        m = dict(base)
        m["gidx"] = gidx_all[c]
        m["xown"] = xown[c]
        m["ohin"] = _host_onehots(sched, dstloc_all[c], np_ohdt)
        in_maps.append(m)

    bench = int(os.environ.get("GIN_BENCH", "0"))
    trace = int(os.environ.get("GIN_TRACE", "0"))
    if bench:
        results = _exec_spmd_timed(nc, in_maps, bench)
    elif trace:
        res = run_bass_kernel_spmd(nc, in_maps, core_ids=list(range(NCORES)),
                                   trace=True,
                                   tmpdir=os.environ.get("GIN_TRACE_DIR"))
        print("TRACE exec_time_ns:", res.exec_time_ns)
        results = res.results
    else:
        res = run_bass_kernel_spmd(nc, in_maps, core_ids=list(range(NCORES)))
        results = res.results
    out = np.concatenate([r["out"] for r in results], axis=0)
    return out.astype(np.float32)


LAST_BENCH_NS = None
DEFERRED = []


def _exec_spmd_timed(nc, in_maps, iters):
    """jit-once SPMD exec (mirrors bass2jax.run_bass_via_pjrt) + steady-state
    timing of repeated NEFF executions. Returns per-core result dicts."""
    import time

    import jax
    import numpy as _np
    from jax.experimental.shard_map import shard_map
    from jax.sharding import Mesh, NamedSharding, PartitionSpec

    from concourse import bass2jax, mybir
    from concourse.bass2jax import _bass_exec_p, install_neuronx_cc_hook, \
        partition_id_tensor

    install_neuronx_cc_hook()
    n_cores = len(in_maps)
    partition_name = (nc.partition_id_tensor.name
                      if nc.partition_id_tensor else None)
    in_names, out_names, out_avals, zero_outs = [], [], [], []
    for alloc in nc.m.functions[0].allocations:
        if not isinstance(alloc, mybir.MemoryLocationSet):
            continue
        name = alloc.memorylocations[0].name
        if alloc.kind == "ExternalInput":
            if name != partition_name:
                in_names.append(name)
        elif alloc.kind == "ExternalOutput":
            out_names.append(name)
            shape = tuple(alloc.tensor_shape)
            dtype = mybir.dt.np(alloc.dtype)
            out_avals.append(jax.core.ShapedArray(shape, dtype))
            zero_outs.append(_np.zeros(shape, dtype))
    n_params = len(in_names)
    all_in_names = in_names + out_names
    if partition_name is not None:
        all_in_names = all_in_names + [partition_name]

    chain = int(os.environ.get("GIN_CHAIN", "1"))

    def _body(*args):
        ins = list(args[:n_params])
        outs = list(args[n_params:])
        for _ in range(chain):
            operands = ins + outs
            if partition_name is not None:
                operands.append(partition_id_tensor())
            outs = list(_bass_exec_p.bind(
                *operands,
                out_avals=tuple(out_avals),
                in_names=tuple(all_in_names),
                out_names=tuple(out_names),
                lowering_input_output_aliases=(),
                sim_require_finite=True,
                sim_require_nnan=True,
                nc=nc,
            ))
        return tuple(outs)

    devices = jax.devices()[:n_cores]
    mesh = Mesh(_np.asarray(devices), ("core",))
    spec = PartitionSpec("core")
    n_outs = len(out_names)
    fn = jax.jit(
        shard_map(_body, mesh=mesh, in_specs=(spec,) * (n_params + n_outs),
                  out_specs=(spec,) * n_outs, check_rep=False),
        keep_unused=True,
    )
    sh = NamedSharding(mesh, spec)
    concat_in = [
        jax.device_put(
            _np.concatenate([_np.asarray(in_maps[c][nm]) for c in
                             range(n_cores)], axis=0), sh)
        for nm in in_names
    ]
    concat_zeros = [
        jax.device_put(_np.zeros((n_cores * z.shape[0], *z.shape[1:]),
                                 z.dtype), sh)
        for z in zero_outs
    ]
    out = jax.block_until_ready(fn(*concat_in, *concat_zeros))
    if os.environ.get("GIN_DEFER"):
        DEFERRED.append((fn, concat_in, concat_zeros))
        times = [0.0]
    else:
        times = []
        for _ in range(max(iters, 1)):
            t0 = time.perf_counter()
            r = jax.block_until_ready(fn(*concat_in, *concat_zeros))
            times.append((time.perf_counter() - t0) * 1e9)
    global LAST_BENCH_NS
    LAST_BENCH_NS = times
    res = []
    for c in range(n_cores):
        res.append({
            nm: _np.asarray(out[i]).reshape(n_cores, *out_avals[i].shape)[c]
            for i, nm in enumerate(out_names)
        })
    return res


def _run(inputs, cfg):
    from concourse.bass_utils import run_bass_kernel_spmd

    x = np.asarray(inputs["x"], np.float32)
    edge_index = np.asarray(inputs["edge_index"])
    eps1 = float(np.asarray(inputs["eps1"]))
    eps2 = float(np.asarray(inputs["eps2"]))

    sched, gidx_all, dstloc_all = _prep_graph(edge_index, cfg)
    xqs = _perm_rows(x.astype(F16), cfg)

    nc = _build_nc(cfg, sched, eps1, eps2)

    iota_np = np.tile(np.arange(512, dtype=np.float32),
                      (128, 1)).astype(F16)
    ident_np = np.eye(128, dtype=np.float32)
    base = {
        "w1": np.asarray(inputs["w1"], np.float32),
        "w2": np.asarray(inputs["w2"], np.float32),
        "w3": np.asarray(inputs["w3"], np.float32),
        "w4": np.asarray(inputs["w4"], np.float32),
        "b1": np.asarray(inputs["b1"], np.float32).reshape(-1, 1),
        "b2": np.asarray(inputs["b2"], np.float32).reshape(-1, 1),
        "b3": np.asarray(inputs["b3"], np.float32).reshape(-1, 1),
        "b4": np.asarray(inputs["b4"], np.float32).reshape(-1, 1),
        "iota": iota_np,
        "ident": ident_np,
    }
    elem = int(os.environ.get("GIN_ELEM", str(cfg.F)))
    shift = (elem // cfg.F).bit_length() - 1
    for q in range(4):
        base[f"xq{q}"] = np.ascontiguousarray(
            xqs[q].reshape(-1, elem))

    oh_host = bool(int(os.environ.get("GIN_OH_HOST", "0")))
    np_ohdt = {"fp8": ml_dtypes.float8_e4m3,
               "fp16": np.float16}[os.environ.get("GIN_OH_DT", "fp16")]

    xbf = x.astype(F16)
    in_maps = []
    for c in range(NCORES):
        m = dict(base)
        gwr, g32 = gidx_all[c]
        m["gidx"] = gwr >> shift if shift else gwr
        m["gidx32"] = g32 >> shift if shift else g32
        m["dstloc"] = dstloc_all[c]
        m["xown"] = xbf[c * cfg.NPC:(c + 1) * cfg.NPC]
        if oh_host:
            m["ohin"] = _host_onehots(sched, dstloc_all[c], np_ohdt)
        in_maps.append(m)

    bench = int(os.environ.get("GIN_BENCH", "0"))
    trace = int(os.environ.get("GIN_TRACE", "0"))
    if bench:
        results = _exec_spmd_timed(nc, in_maps, bench)
    elif trace:
        tdir = os.environ.get("GIN_TRACE_DIR") or None
        tcores = [int(c) for c in
                  os.environ.get("GIN_TRACE_CORES", "0").split(",")]
        res = run_bass_kernel_spmd(nc, in_maps, core_ids=list(range(NCORES)),
                                   trace=True, tmpdir=tdir,
                                   trace_cores=tcores)
        print("TRACE exec_time_ns:", res.exec_time_ns,
              "mean:", res.mean_exec_time_ns)
        results = res.results
    else:
        res = run_bass_kernel_spmd(nc, in_maps, core_ids=list(range(NCORES)))
        results = res.results
    out = np.concatenate([r["out"] for r in results], axis=0)
    return out.astype(np.float32)


def kernel(**inputs):
    if os.environ.get("GIN_V2", "0") == "1":
        return _run_v2(inputs, FULL)
    return _run(inputs, FULL)



# revision 30
# speedup vs baseline: 2.0538x; 1.4428x over previous
"""GIN 2-layer message-passing network on 8 Trainium2 NeuronCores.

Strategy (matches the dst-partitioned sharding hint):
  - Nodes are split into 8 contiguous chunks of N/8; core c owns chunk c and
    all edges whose destination lands in it. The `+ 1*h_i` self term is NOT
    materialized as self-edges: each destination tile adds its own dense
    feature rows via one identity matmul (saves ~6% of gather slots).
  - segment_sum becomes, per core: dma_gather of source-node feature rows
    (fp16) in edge order (4 SWDGE queues — the gather is descriptor-count
    bound at ~5.3 ns/row, so slot count is the kernel's critical resource),
    then a segment-sum on the tensor engine using one-hot matrices built on
    the vector engine, accumulating in PSUM per 128-destination-node tile:
        aggT[feat, dst] += G[edges, feat].T-contract-. onehot[edges, dst]
  - Edge groups (tile, quarter) are padded to 32 slots, not 128. Groups
    sharing a 128-slot column are disambiguated by a *level*: one-hot
    columns are shifted by 128*level (fp16 iota 0..511), so a full-column
    matmul for one group sees exact zeros on the other groups' rows. This
    avoids PE partition-offset matmuls, which hang real HW.
  - The one-hot depends only on the edge structure, so layer 1 stores it to
    DRAM and layer 2 reloads it with dense DMA instead of re-running the
    (slow, ~76 G elem/s) DVE broadcast compare.
  - The MLP runs in "transposed land" ([feat, nodes] layout) so activations
    never need transposing between matmuls; per tile:
        h = relu(w.T @ aggT + b)  via PE matmul + ACT relu-with-bias.
  - Between the two GIN layers the per-core h chunks are exchanged with 4
    AllGathers (one per quarter of each core's rows) so layer-2 gathers can
    index any node with int16 indices (< 32768 rows per gather source).
  - log_softmax of the final [40, nodes] tile is done after a PE transpose
    back to [nodes, 40]: row-max, subtract, exp-with-accumulated-sum (ACT),
    ln, subtract.

All per-core variability lives in the *data* (index / one-hot-column arrays,
padded to a per-group max across cores) so a single SPMD NEFF serves all 8
cores.
"""

import os
import sys

sys.path.insert(0, "/opt/trn_rl_repo")
sys.path.insert(0, "/opt/trn_rl_repo/concourse")
os.environ.setdefault("TRN_TYPE", "TRN2")

import numpy as np
import ml_dtypes

BF16 = ml_dtypes.bfloat16
F16 = np.float16

NCORES = 8


class Cfg:
    def __init__(self, n, feat, hid, cls, tiles_per_batch=5):
        assert n % (NCORES * 4) == 0
        self.N = n
        self.F = feat          # input feature dim (must be 128 here)
        self.H = hid           # hidden dim (128)
        self.CLS = cls         # classes
        self.NPC = n // NCORES          # nodes per core
        self.QROWS = self.NPC // 4      # rows per quarter per core
        self.SRCROWS = self.QROWS * NCORES  # rows per gather source tensor
        self.NT = -(-self.NPC // 128)   # dst tiles per core
        self.last_rows = self.NPC - (self.NT - 1) * 128
        self.B = tiles_per_batch


FULL = Cfg(100000, 128, 128, 40,
           tiles_per_batch=int(os.environ.get("GIN_B", "5")))


def _prep_graph(edge_index, cfg):
    """Host-side sharding: returns (schedule, per-core index arrays).

    schedule: dict with
      slots[t*4+q]   padded slot count per (tile, quarter) group (max/cores)
      batches        list of lists of tile ids
      call_slots[b][q], call_off[b][q], slot_off maps for emission
    per-core: gidx_wr [128, TOT//16] int16, dstloc [128, TOT//128] bf16
    """
    N, NPC, QROWS, NT = cfg.N, cfg.NPC, cfg.QROWS, cfg.NT
    noself = bool(int(os.environ.get("GIN_NOSELF", "1")))
    pad = int(os.environ.get("GIN_PAD", "32"))
    src = np.asarray(edge_index[0], dtype=np.int64)
    dst = np.asarray(edge_index[1], dtype=np.int64)
    if not noself:
        # self-edges give the +h_i term of the GIN aggregate
        allid = np.arange(N, dtype=np.int64)
        src = np.concatenate([src, allid])
        dst = np.concatenate([dst, allid])

    core = dst // NPC
    per_core = []
    counts = np.zeros((NCORES, NT * 4), np.int64)
    for c in range(NCORES):
        m = core == c
        s = src[m]
        dloc = (dst[m] - c * NPC).astype(np.int64)
        t = dloc >> 7
        q = (s % NPC) // QROWS
        gidxv = (s // NPC) * QROWS + (s % QROWS)
        dstin = dloc & 127
        gid = (t * 4 + q).astype(np.int64)
        counts[c] = np.bincount(gid, minlength=NT * 4)
        per_core.append((gid, gidxv.astype(np.int32), dstin.astype(np.int32)))

    cmax = counts.max(axis=0)                       # [NT*4]
    slots = -(-cmax // pad) * pad                   # slots per (t,q)
    # batches of tiles
    B = cfg.B
    batches = [list(range(b, min(b + B, NT))) for b in range(0, NT, B)]
    # slot offsets in (b, q, t) order; call boundaries stay 128-aligned
    off = 0
    slot_off = np.zeros(NT * 4, np.int64)
    call_slots = []           # [b][q] -> num slots in that gather call
    call_off = []             # [b][q] -> slot offset of call start
    # level disambiguates groups sharing a 128-slot column: one-hot columns
    # are shifted by 128*level, so a full-column matmul for one group sees
    # zeros on the other groups' rows.
    level = np.zeros(NT * 4, np.int64)
    for tiles in batches:
        cs, co = [], []
        for q in range(4):
            co.append(off)
            s0 = off
            for i, t in enumerate(tiles):
                level[t * 4 + q] = i % 4
                slot_off[t * 4 + q] = off
                off += slots[t * 4 + q]
            off = -(-off // 128) * 128      # align next call to 128
            cs.append(off - s0)
        call_slots.append(cs)
        call_off.append(co)
    tot = off
    assert tot % 128 == 0

    gidx_all, dstloc_all = [], []
    for c in range(NCORES):
        gid, gidxv, dstin = per_core[c]
        order = np.argsort(gid, kind="stable")
        gs = gid[order]
        cnt = counts[c]
        starts = np.zeros(NT * 4, np.int64)
        np.cumsum(cnt[:-1], out=starts[1:])
        rank = np.arange(len(gs)) - starts[gs]
        slot = slot_off[gs] + rank
        gflat = np.zeros(tot, np.int16)
        dflat = np.full(tot, 2000.0, np.float32)
        gflat[slot] = gidxv[order].astype(np.int16)
        # one-hot column = dst-in-tile + 128*level of the slot's group
        dflat[slot] = dstin[order] + 128.0 * level[gs]
        # wrap for dma_gather: [p, col] = gflat[col*16 + p%16], replicated x8
        gwr = np.tile(gflat.reshape(tot // 16, 16).T, (8, 1)).copy()
        dloc = dflat.reshape(tot // 128, 128).T.astype(F16).copy()
        gidx_all.append((gwr, gflat.astype(np.int32).reshape(128, tot // 128)))
        dstloc_all.append(dloc)

    sched = dict(slots=slots, batches=batches, call_slots=call_slots,
                 call_off=call_off, slot_off=slot_off, tot=tot,
                 noself=noself, level=level)
    return sched, gidx_all, dstloc_all


def _perm_rows(x, cfg):
    """x [N, F] -> 4 arrays [SRCROWS, F]; source s holds global row
    g = r*NPC + s*QROWS + u at position r*QROWS + u."""
    N, NPC, QROWS = cfg.N, cfg.NPC, cfg.QROWS
    g = np.arange(N)
    s = (g % NPC) // QROWS
    pos = (g // NPC) * QROWS + (g % QROWS)
    out = []
    for si in range(4):
        m = s == si
        a = np.empty((cfg.SRCROWS, x.shape[1]), x.dtype)
        a[pos[m]] = x[m]
        out.append(a)
    return out


def _groups(sched):
    """Per (b, q): group list [(tile, c0, c1, lvl, ohpos)] with call-relative
    column ranges; boundary columns appear in two groups."""
    slots_arr = sched["slots"]
    slot_off = sched["slot_off"]
    batches = sched["batches"]
    call_off = sched["call_off"]
    level = sched["level"]
    groups_bq = {}
    maxOHC = 0
    for b, tiles in enumerate(batches):
        for q in range(4):
            lst, ohpos = [], 0
            for t in tiles:
                ns = int(slots_arr[t * 4 + q])
                if ns == 0:
                    continue
                rel = int(slot_off[t * 4 + q] - call_off[b][q])
                c0, c1 = rel // 128, -(-(rel + ns) // 128)
                lst.append((t, c0, c1, int(level[t * 4 + q]), ohpos))
                ohpos += c1 - c0
            groups_bq[(b, q)] = lst
            maxOHC = max(maxOHC, ohpos)
    return groups_bq, maxOHC


def _host_onehots(sched, dstloc, np_dt):
    """Host-precomputed one-hot planes, laid out exactly like the device
    oh_dram reuse buffer: [128, ncalls*maxOHC, 128]."""
    groups_bq, maxOHC = _groups(sched)
    batches = sched["batches"]
    call_off = sched["call_off"]
    ncalls = len(batches) * 4
    out = np.zeros((128, ncalls * maxOHC, 128), np_dt)
    cols = np.arange(128, dtype=np.float32)
    dl32 = dstloc.astype(np.float32)
    for b in range(len(batches)):
        for q in range(4):
            od = b * 4 + q
            base = call_off[b][q] // 128
            for (t, c0, c1, lvl, op_) in groups_bq[(b, q)]:
                for j in range(c1 - c0):
                    v = dl32[:, base + c0 + j]
                    out[:, od * maxOHC + op_ + j, :] = (
                        v[:, None] == (128.0 * lvl + cols)).astype(np_dt)
    return out


def _build_nc(cfg, sched, eps1, eps2):
    from concourse import mybir
    import concourse.bacc as bacc
    import concourse.tile as tile

    F, H, CLS, NT, NPC = cfg.F, cfg.H, cfg.CLS, cfg.NT, cfg.NPC
    slots_arr = sched["slots"]
    slot_off = sched["slot_off"]
    noself = sched["noself"]
    batches = sched["batches"]
    call_slots = sched["call_slots"]
    tot = sched["tot"]
    f32 = mybir.dt.float32
    bf16 = mybir.dt.float16      # data-path dtype (fp16: exact ints to 2048)
    AT = mybir.ActivationFunctionType
    OP = mybir.AluOpType
    level = sched["level"]
    call_off = sched["call_off"]

    groups_bq, maxOHC = _groups(sched)

    assert eps1 == 0.0 and eps2 == 0.0, "nonzero eps not implemented"

    nqueues = int(os.environ.get("GIN_QUEUES", "4"))
    single_packet = bool(int(os.environ.get("GIN_SP", "0")))
    scratch = int(os.environ.get("GIN_SCRATCH", "32768"))
    nc = bacc.Bacc("TRN2", target_bir_lowering=False, debug=False,
                   num_devices=NCORES, num_swdge_queues=nqueues,
                   dynamic_dma_scratch_size=scratch)

    oh_host = bool(int(os.environ.get("GIN_OH_HOST", "0")))
    # pipeline depth for the gather/one-hot pools: 6 buffers = only 1.5
    # batches of lookahead, which stalls the DMA gather stream on buffer
    # recycling behind the DVE one-hot builds. SBUF has headroom for 10.
    depth = int(os.environ.get("GIN_DEPTH", "10"))
    # fp8 one-hots are exact in sim but cost ~2% rel err on HW (the PE
    # appears to quantize the fp16 operand in mixed-dtype mode) -> fp16.
    oh_dt = {"fp8": mybir.dt.float8e4, "fp16": mybir.dt.float16}[
        os.environ.get("GIN_OH_DT", "fp16")]
    qspread = bool(int(os.environ.get("GIN_QSPREAD", "0")))

    elem = int(os.environ.get("GIN_ELEM", str(F)))  # bench knob: 256/512
    xq = [nc.dram_tensor(f"xq{q}", [cfg.SRCROWS * F // elem, elem], bf16,
                         kind="ExternalInput")
          for q in range(4)]
    w1_t = nc.dram_tensor("w1", [F, H], f32, kind="ExternalInput")
    w2_t = nc.dram_tensor("w2", [H, H], f32, kind="ExternalInput")
    w3_t = nc.dram_tensor("w3", [H, H], f32, kind="ExternalInput")
    w4_t = nc.dram_tensor("w4", [H, CLS], f32, kind="ExternalInput")
    b1_t = nc.dram_tensor("b1", [H, 1], f32, kind="ExternalInput")
    b2_t = nc.dram_tensor("b2", [H, 1], f32, kind="ExternalInput")
    b3_t = nc.dram_tensor("b3", [H, 1], f32, kind="ExternalInput")
    b4_t = nc.dram_tensor("b4", [CLS, 1], f32, kind="ExternalInput")
    xown_t = nc.dram_tensor("xown", [NPC, F], bf16, kind="ExternalInput")
    iota_t = nc.dram_tensor("iota", [128, 512], bf16, kind="ExternalInput")
    ident_t = nc.dram_tensor("ident", [128, 128], f32, kind="ExternalInput")
    gidx_t = nc.dram_tensor("gidx", [128, tot // 16], mybir.dt.int16,
                            kind="ExternalInput")
    indirect = bool(int(os.environ.get("GIN_INDIRECT", "0")))
    gidx32_t = nc.dram_tensor("gidx32", [128, tot // 128], mybir.dt.int32,
                              kind="ExternalInput")
    dstloc_t = nc.dram_tensor("dstloc", [128, tot // 128], bf16,
                              kind="ExternalInput")
    ohin_t = None
    if oh_host:
        ncalls_all = len(batches) * 4
        ohin_t = nc.dram_tensor("ohin", [128, ncalls_all * maxOHC, 128],
                                oh_dt, kind="ExternalInput")
    out_t = nc.dram_tensor("out", [NPC, CLS], f32, kind="ExternalOutput")

    maxS = max(max(cs) for cs in call_slots)

    with tile.TileContext(nc) as tc:
        with tc.tile_pool(name="const", bufs=1) as cp, \
             tc.tile_pool(name="gp", bufs=depth) as gp, \
             tc.tile_pool(name="ohp", bufs=depth) as ohp, \
             tc.tile_pool(name="work", bufs=4) as wp, \
             tc.tile_pool(name="small", bufs=4) as sp, \
             tc.tile_pool(name="aggps", bufs=4, space="PSUM") as aggps, \
             tc.tile_pool(name="mmps", bufs=4, space="PSUM") as mmps, \
             tc.tile_pool(name="dram", bufs=1, space="DRAM") as dp:

            w1 = cp.tile([F, H], f32); nc.sync.dma_start(w1[:], w1_t.ap())
            w2 = cp.tile([H, H], f32); nc.sync.dma_start(w2[:], w2_t.ap())
            w3 = cp.tile([H, H], f32); nc.sync.dma_start(w3[:], w3_t.ap())
            w4 = cp.tile([H, CLS], f32); nc.sync.dma_start(w4[:], w4_t.ap())
            b1 = cp.tile([H, 1], f32); nc.sync.dma_start(b1[:], b1_t.ap())
            b2 = cp.tile([H, 1], f32); nc.sync.dma_start(b2[:], b2_t.ap())
            b3 = cp.tile([H, 1], f32); nc.sync.dma_start(b3[:], b3_t.ap())
            b4 = cp.tile([CLS, 1], f32); nc.sync.dma_start(b4[:], b4_t.ap())
            iota = cp.tile([128, 512], bf16); nc.sync.dma_start(iota[:], iota_t.ap())
            ident = cp.tile([128, 128], f32); nc.sync.dma_start(ident[:], ident_t.ap())
            gidx_sb = cp.tile([128, tot // 16], mybir.dt.int16)
            nc.sync.dma_start(gidx_sb[:], gidx_t.ap())
            gidx32_sb = None
            if indirect:
                gidx32_sb = cp.tile([128, tot // 128], mybir.dt.int32)
                nc.sync.dma_start(gidx32_sb[:], gidx32_t.ap())
            dstloc_sb = cp.tile([128, tot // 128], bf16)
            nc.sync.dma_start(dstloc_sb[:], dstloc_t.ap())

            h_own = dp.tile([NPC, H], bf16)
            # rebuilding the one-hot on DVE each layer overlaps with the
            # gather DMA stream; the DRAM store+reload contends with it
            # (measured: 1.78ms rebuild vs 2.00ms reuse).
            oh_reuse = bool(int(os.environ.get("GIN_OH_REUSE", "0")))
            oh_dram = None
            ncalls = len(batches) * 4
            if oh_reuse and not oh_host:
                oh_dram = dp.tile([128, ncalls * maxOHC, 128], bf16)
            # dp.tile (unlike tc.tile) forwards addr_space; Shared is the
            # collectives fast path for HBM-HBM AllGather outputs.
            ag_space = ("Shared" if int(os.environ.get("GIN_AG_SHARED", "0"))
                        else "Local")
            no_ag = bool(os.environ.get("GIN_NO_AG"))
            h_ag = [dp.tile([cfg.SRCROWS, H], bf16, addr_space=ag_space,
                            name=f"h_ag{s}")
                    for s in range(4)]

            skips = set(os.environ.get("GIN_SKIP", "").split(","))
            identb = None
            if noself:
                identb = cp.tile([128, 128], bf16)
                nc.vector.tensor_copy(out=identb[:], in_=ident[:])

            def layer(sources, is_first, batch_limit=None, skip_mlp=False,
                      own_src=None):
                """Emit one GIN layer. sources: list of 4 gather-source APs.
                own_src(t, rows) -> DRAM AP of this tile's own feature rows
                (realizes the +1*h_i self term via an identity matmul)."""
                ag_emitted = [False] * 4
                for b, tiles in enumerate(batches):
                    if batch_limit is not None and b >= batch_limit:
                        break
                    G, OH = [], []
                    for q in range(4):
                        S = call_slots[b][q]
                        if S == 0:
                            G.append(None); OH.append(None)
                            continue
                        o = sched["call_off"][b][q]
                        gi = gidx_sb[:, o // 16:(o + S) // 16]
                        dl = dstloc_sb[:, o // 128:(o + S) // 128]
                        g = gp.tile([128, maxS // 128, elem], bf16, tag="g",
                                    bufs=max(2, depth * F // elem))
                        if "gather" not in skips:
                            if int(os.environ.get("GIN_TMODE", "0")):
                                gt = gp.tile([128, maxS], bf16, tag="gt",
                                             bufs=2)
                                nc.gpsimd.dma_gather(
                                    gt[:, :S], sources[q], gi, S, S, elem,
                                    transpose=True,
                                    single_packet=single_packet,
                                    queue_num=((b * 4 + q) if qspread
                                               else q) % nqueues)
                            elif indirect:
                                from concourse.bass import IndirectOffsetOnAxis
                                nc.gpsimd.indirect_dma_start(
                                    out=g[:, : S // 128, :],
                                    out_offset=None,
                                    in_=sources[q],
                                    in_offset=IndirectOffsetOnAxis(
                                        ap=gidx32_sb[
                                            :, o // 128:(o + S) // 128],
                                        axis=0),
                                )
                            else:
                                nc.gpsimd.dma_gather(
                                    g[:, : S // 128, :], sources[q], gi,
                                    S, S, elem, single_packet=single_packet,
                                    queue_num=((b * 4 + q) if qspread
                                               else q) % nqueues)
                        oh = None
                        if "oh" not in skips:
                            glist = groups_bq[(b, q)]
                            nohc = (glist[-1][4] + glist[-1][2] - glist[-1][1]
                                    if glist else 0)
                            oh = ohp.tile([128, maxOHC, 128],
                                          oh_dt if oh_host else bf16, tag="oh")
                            od = b * 4 + q
                            if oh_host:
                                if nohc:
                                    # host-precomputed one-hot planes; ACT
                                    # (HWDGE) queue keeps sync free for stores
                                    nc.scalar.dma_start(
                                        oh[:, :nohc, :],
                                        ohin_t.ap()[:, od * maxOHC:
                                                    od * maxOHC + nohc, :])
                            elif oh_reuse and not is_first:
                                if nohc:
                                    # ACT-queue DMA keeps the sync queue free
                                    # for stores while the gather stream runs
                                    nc.scalar.dma_start(
                                        oh[:, :nohc, :],
                                        oh_dram[:][:, od * maxOHC:
                                                   od * maxOHC + nohc, :])
                            else:
                                swap = int(os.environ.get("GIN_OH_SWAP", "0"))
                                for (t, c0, c1, lvl, op_) in glist:
                                    n = c1 - c0
                                    ins = [
                                        iota[:, lvl * 128:
                                             (lvl + 1) * 128].unsqueeze(
                                            1).broadcast_to([128, n, 128]),
                                        dl[:, c0:c1].unsqueeze(
                                            2).broadcast_to([128, n, 128]),
                                    ]
                                    nc.vector.tensor_tensor(
                                        out=oh[:, op_:op_ + n, :],
                                        in0=ins[swap], in1=ins[1 - swap],
                                        op=OP.is_equal)
                                if oh_reuse and is_first and nohc:
                                    nc.sync.dma_start(
                                        oh_dram[:][:, od * maxOHC:
                                                   od * maxOHC + nohc, :],
                                        oh[:, :nohc, :])
                        G.append(g); OH.append(oh)

                    if "mm" in skips:
                        continue
                    for t in tiles:
                        # full-column pieces; level-shifted one-hots zero out
                        # the other groups' rows in shared columns
                        pieces = []          # (q, col, ohcol)
                        for q in range(4):
                            for (tt, c0, c1, lvl, op_) in groups_bq[(b, q)]:
                                if tt != t:
                                    continue
                                for j in range(c1 - c0):
                                    pieces.append((q, c0 + j, op_ + j))
                        rows = 128 if t < NT - 1 else cfg.last_rows
                        nmm = len(pieces) + (1 if noself else 0)
                        if nmm == 0:
                            continue
                        agg = aggps.tile([128, 128], f32, tag="agg")
                        k = 0
                        if noself:
                            own = sp.tile([128, 128], bf16, tag="own", bufs=4)
                            nc.scalar.dma_start(own[:rows, :],
                                                own_src(t, rows))
                            nc.tensor.matmul(
                                out=agg[:], lhsT=own[:rows, :],
                                rhs=identb[:rows, :],
                                start=True, stop=(nmm == 1))
                            k += 1
                        for (q, col, ohcol) in pieces:
                            nc.tensor.matmul(
                                out=agg[:],
                                lhsT=G[q][:, col, :],
                                rhs=OH[q][:, ohcol, :],
                                start=(k == 0),
                                stop=(k == nmm - 1))
                            k += 1

                        aggT = wp.tile([128, 128], f32, tag="aggT")
                        nc.scalar.activation(out=aggT[:], in_=agg[:], func=AT.Copy)

                        if skip_mlp:
                            continue
                        if is_first:
                            ps1 = mmps.tile([128, 128], f32, tag="mm")
                            nc.tensor.matmul(out=ps1[:], lhsT=w1[:], rhs=aggT[:],
                                             start=True, stop=True)
                            h1 = wp.tile([128, 128], f32, tag="h1")
                            nc.scalar.activation(out=h1[:], in_=ps1[:],
                                                 func=AT.Relu, bias=b1[:])
                            ps2 = mmps.tile([128, 128], f32, tag="mm")
                            nc.tensor.matmul(out=ps2[:], lhsT=w2[:], rhs=h1[:],
                                             start=True, stop=True)
                            h2 = wp.tile([128, 128], f32, tag="h2")
                            nc.scalar.activation(out=h2[:], in_=ps2[:],
                                                 func=AT.Relu, bias=b2[:])
                            # transpose back to [nodes, feat], cast bf16, store
                            pst = mmps.tile([128, 128], f32, tag="mm")
                            nc.tensor.transpose(out=pst[:], in_=h2[:],
                                                identity=ident[:])
                            hbf = sp.tile([128, 128], bf16, tag="hbf")
                            nc.vector.tensor_copy(out=hbf[:], in_=pst[:])
                            nc.sync.dma_start(
                                h_own[:][t * 128: t * 128 + rows, :],
                                hbf[:rows, :])
                            # fire AllGathers as soon as their rows are done
                            for s in range(4):
                                if not ag_emitted[s] and \
                                        (t + 1) * 128 >= (s + 1) * cfg.QROWS:
                                    ag_emitted[s] = True
                                    if no_ag:
                                        continue
                                    nc.gpsimd.collective_compute(
                                        "AllGather", OP.bypass,
                                        replica_groups=[list(range(NCORES))],
                                        ins=[h_own[:][s * cfg.QROWS:
                                                      (s + 1) * cfg.QROWS, :]],
                                        outs=[h_ag[s][:]])
                        else:
                            ps1 = mmps.tile([128, 128], f32, tag="mm")
                            nc.tensor.matmul(out=ps1[:], lhsT=w3[:], rhs=aggT[:],
                                             start=True, stop=True)
                            h3 = wp.tile([128, 128], f32, tag="h1")
                            nc.scalar.activation(out=h3[:], in_=ps1[:],
                                                 func=AT.Relu, bias=b3[:])
                            ps2 = mmps.tile([128, 128], f32, tag="mm")
                            nc.tensor.matmul(out=ps2[:CLS, :128],
                                             lhsT=w4[:], rhs=h3[:],
                                             start=True, stop=True)
                            c4 = sp.tile([CLS, 128], f32, tag="c4")
                            nc.vector.tensor_tensor(
                                out=c4[:], in0=ps2[:CLS, :128],
                                in1=b4[:].broadcast_to([CLS, 128]),
                                op=OP.add)
                            psf = mmps.tile([128, 128], f32, tag="mm")
                            nc.tensor.transpose(out=psf[:128, :CLS], in_=c4[:],
                                                identity=ident[:CLS, :CLS])
                            mx = sp.tile([128, 1], f32, tag="mx")
                            nc.vector.tensor_reduce(
                                out=mx[:], in_=psf[:128, :CLS],
                                axis=mybir.AxisListType.X, op=OP.max)
                            tsh = sp.tile([128, CLS], f32, tag="tsh")
                            nc.vector.tensor_tensor(
                                out=tsh[:], in0=psf[:128, :CLS],
                                in1=mx[:].broadcast_to([128, CLS]),
                                op=OP.subtract)
                            esum = sp.tile([128, 1], f32, tag="esum")
                            edum = sp.tile([128, CLS], f32, tag="edum")
                            nc.scalar.activation(out=edum[:], in_=tsh[:],
                                                 func=AT.Exp, accum_out=esum[:])
                            lse = sp.tile([128, 1], f32, tag="lse")
                            nc.scalar.activation(out=lse[:], in_=esum[:],
                                                 func=AT.Ln)
                            osb = sp.tile([128, CLS], f32, tag="osb")
                            nc.vector.tensor_tensor(
                                out=osb[:], in0=tsh[:],
                                in1=lse[:].broadcast_to([128, CLS]),
                                op=OP.subtract)
                            nc.sync.dma_start(
                                out_t.ap()[t * 128: t * 128 + rows, :],
                                osb[:rows, :])

            l1b = int(os.environ.get("GIN_L1_BATCHES", "0"))
            repeat = int(os.environ.get("GIN_REPEAT", "1"))

            def own1(t, rows):
                return xown_t.ap()[t * 128: t * 128 + rows, :]

            def own2(t, rows):
                return h_own[:][t * 128: t * 128 + rows, :]

            for _rep in range(repeat):
                if l1b:
                    layer([x.ap() for x in xq], is_first=True,
                          batch_limit=l1b, skip_mlp=True, own_src=own1)
                else:
                    layer([x.ap() for x in xq], is_first=True, own_src=own1)
                    if no_ag:
                        layer([x.ap() for x in xq], is_first=False,
                              own_src=own1)
                    else:
                        layer([h[:] for h in h_ag], is_first=False,
                              own_src=own2)

    nc.compile()
    return nc


#  v2: SBUF-resident source tables + transpose-mode SBUF gathers.
#
#  The gather microbench shows SBUF-source dma_gather at ~1.5 ns/desc vs
#  ~3.7 ns/desc for HBM-source (random 256B reads pay an HBM penalty that
#  SRAM does not). v2 therefore keeps the gather sources in SBUF:
#    - Nodes are padded per-core to 12544 (=98*128) "virtual" rows; the
#      node table is split into 4 quarters of 25088 rows. One quarter
#      (49 KiB/partition fp16, wrapped layout: row i -> partition i%128,
#      rank i//128) is resident per pass; each layer runs 4 passes.
#    - SBUF-source gather is transpose-only: output is [feat, slots]. Each
#      128-slot piece is PE-transposed back to [slots, feat] (fp16, PSUM),
#      copied to SBUF (DVE/ACT alternating), then the usual one-hot matmul
#      accumulates agg[feat, dst] in PSUM; pass results accumulate into an
#      SBUF agg buffer (fp16) since PSUM cannot hold all 98 tiles.
#    - The self term rides along as host-injected self-edges (SPMD-uniform;
#      +6% slots but no per-core table indexing).
#    - One-hot planes are host-precomputed (fp8) and streamed from DRAM.
#    - Layer 1 tables load from host-prepared partition-major inputs (one
#      64KB-descriptor DMA per partition); layer 2 tables come from 4
#      range-AllGathers of the partition-major h buffer.

def _npcv(cfg):
    return cfg.NT * 128          # virtual rows per core (full: 12544)


def _sched_from_counts(counts, NT, pad, B):
    """Shared scheduling: per-(tile,quarter) slot padding, batches, call
    offsets, level assignment. counts: [NCORES, NT*4]."""
    cmax = counts.max(axis=0)
    slots = -(-cmax // pad) * pad
    batches = [list(range(b, min(b + B, NT))) for b in range(0, NT, B)]
    off = 0
    slot_off = np.zeros(NT * 4, np.int64)
    call_slots, call_off = [], []
    level = np.zeros(NT * 4, np.int64)
    for tiles in batches:
        cs, co = [], []
        for q in range(4):
            co.append(off)
            s0 = off
            for i, t in enumerate(tiles):
                level[t * 4 + q] = i % 4
                slot_off[t * 4 + q] = off
                off += slots[t * 4 + q]
            off = -(-off // 128) * 128
            cs.append(off - s0)
        call_slots.append(cs)
        call_off.append(co)
    tot = off
    assert tot % 128 == 0
    return dict(slots=slots, batches=batches, call_slots=call_slots,
                call_off=call_off, slot_off=slot_off, tot=tot, level=level,
                noself=True)


def _prep_v2(edge_index, cfg):
    """v2 host prep: self-edges injected, virtual-row quartering, per-core
    gather indices + dstloc (for host one-hots)."""
    N, NPC, NT = cfg.N, cfg.NPC, cfg.NT
    NPCV = _npcv(cfg)
    QRV = 2 * NPCV
    pad = int(os.environ.get("GIN_PAD", "32"))
    B = cfg.B
    src = np.asarray(edge_index[0], dtype=np.int64)
    dst = np.asarray(edge_index[1], dtype=np.int64)
    # self term handled by an identity matmul on the own-rows slice
    v = (src // NPC) * NPCV + (src % NPC)      # virtual source row
    vq = v // QRV
    iq = (v % QRV).astype(np.int32)

    core = dst // NPC
    per_core = []
    counts = np.zeros((NCORES, NT * 4), np.int64)
    for c in range(NCORES):
        m = core == c
        dl = dst[m] - c * NPC
        t = dl >> 7
        gid = (t * 4 + vq[m]).astype(np.int64)
        counts[c] = np.bincount(gid, minlength=NT * 4)
        per_core.append((gid, iq[m], (dl & 127).astype(np.int32)))

    sched = _sched_from_counts(counts, NT, pad, B)
    slot_off, level, tot = sched["slot_off"], sched["level"], sched["tot"]

    gidx_all, dstloc_all = [], []
    for c in range(NCORES):
        gid, gidxv, dstin = per_core[c]
        order = np.argsort(gid, kind="stable")
        gs = gid[order]
        cnt = counts[c]
        starts = np.zeros(NT * 4, np.int64)
        np.cumsum(cnt[:-1], out=starts[1:])
        rank = np.arange(len(gs)) - starts[gs]
        slot = slot_off[gs] + rank
        gflat = np.zeros(tot, np.int16)
        dflat = np.full(tot, 2000.0, np.float32)
        gflat[slot] = gidxv[order].astype(np.int16)
        dflat[slot] = dstin[order] + 128.0 * level[gs]
        gwr = np.tile(gflat.reshape(tot // 16, 16).T, (8, 1)).copy()
        dloc = dflat.reshape(tot // 128, 128).T.astype(F16).copy()
        gidx_all.append(gwr)
        dstloc_all.append(dloc)
    return sched, gidx_all, dstloc_all


def _own_pm(x, cfg):
    """Per-core own rows, wrapped partition-major [128, NT, F] fp16."""
    NPCV = _npcv(cfg)
    out = []
    for c in range(NCORES):
        xv = np.zeros((NPCV, x.shape[1]), F16)
        xv[:cfg.NPC] = x[c * cfg.NPC:(c + 1) * cfg.NPC]
        out.append(np.ascontiguousarray(
            xv.reshape(cfg.NT, 128, -1).transpose(1, 0, 2)))
    return out


def _perm_v2(x, cfg):
    """x [N,128] -> 4 partition-major quarter tables [128, ranks, 128]."""
    NPCV = _npcv(cfg)
    QRV = 2 * NPCV
    xv = np.zeros((NCORES, NPCV, x.shape[1]), F16)
    xv[:, :x.shape[0] // NCORES] = x.reshape(NCORES, -1, x.shape[1])
    xall = xv.reshape(NCORES * NPCV, x.shape[1])
    out = []
    for q in range(4):
        quarter = xall[q * QRV:(q + 1) * QRV]
        out.append(np.ascontiguousarray(
            quarter.reshape(QRV // 128, 128, -1).transpose(1, 0, 2)))
    return out


def _build_nc_v2(cfg, sched, eps1, eps2):
    from concourse import mybir
    import concourse.bacc as bacc
    import concourse.tile as tile

    F, H, CLS, NT, NPC = cfg.F, cfg.H, cfg.CLS, cfg.NT, cfg.NPC
    batches = sched["batches"]
    call_slots = sched["call_slots"]
    call_off = sched["call_off"]
    tot = sched["tot"]
    f32 = mybir.dt.float32
    f16 = mybir.dt.float16
    f8 = mybir.dt.float8e4
    AT = mybir.ActivationFunctionType
    OP = mybir.AluOpType
    assert eps1 == 0.0 and eps2 == 0.0

    groups_bq, maxOHC = _groups(sched)
    nqueues = int(os.environ.get("GIN_QUEUES", "4"))
    single_packet = bool(int(os.environ.get("GIN_SP", "0")))
    scratch = int(os.environ.get("GIN_SCRATCH", "32768"))
    nc = bacc.Bacc("TRN2", target_bir_lowering=False, debug=False,
                   num_devices=NCORES, num_swdge_queues=nqueues,
                   dynamic_dma_scratch_size=scratch)

    RANKS = 2 * NT
    xpm = [nc.dram_tensor(f"xpm{q}", [128, RANKS, F], f16,
                          kind="ExternalInput") for q in range(4)]
    w1_t = nc.dram_tensor("w1", [F, H], f16, kind="ExternalInput")
    w2_t = nc.dram_tensor("w2", [H, H], f16, kind="ExternalInput")
    w3_t = nc.dram_tensor("w3", [H, H], f16, kind="ExternalInput")
    w4_t = nc.dram_tensor("w4", [H, CLS], f16, kind="ExternalInput")
    b1_t = nc.dram_tensor("b1", [H, 1], f32, kind="ExternalInput")
    b2_t = nc.dram_tensor("b2", [H, 1], f32, kind="ExternalInput")
    b3_t = nc.dram_tensor("b3", [H, 1], f32, kind="ExternalInput")
    b4_t = nc.dram_tensor("b4", [CLS, 1], f32, kind="ExternalInput")
    id16_t = nc.dram_tensor("id16", [128, 128], f16, kind="ExternalInput")
    id32_t = nc.dram_tensor("id32", [128, 128], f32, kind="ExternalInput")
    gidx_t = nc.dram_tensor("gidx", [128, tot // 16], mybir.dt.int16,
                            kind="ExternalInput")
    xown_t = nc.dram_tensor("xown", [128, NT, F], f16, kind="ExternalInput")
    ncalls_all = len(batches) * 4
    oh_dt = {"fp8": f8, "fp16": f16}[os.environ.get("GIN_OH_DT", "fp16")]
    ohin_t = nc.dram_tensor("ohin", [128, ncalls_all * maxOHC, 128], oh_dt,
                            kind="ExternalInput")
    out_t = nc.dram_tensor("out", [NPC, CLS], f32, kind="ExternalOutput")

    maxS = max(max(cs) for cs in call_slots)
    NRL = NT                 # ranks per core chunk
    step = -(-NRL // 4)
    AG_RANGES = [(lo, min(lo + step, NRL)) for lo in range(0, NRL, step)]
    last_rows = cfg.last_rows

    with tile.TileContext(nc) as tc:
        with tc.tile_pool(name="const", bufs=1) as cp, \
             tc.tile_pool(name="tbl", bufs=2) as tp, \
             tc.tile_pool(name="gp", bufs=2) as gp, \
             tc.tile_pool(name="ohp", bufs=3) as ohp, \
             tc.tile_pool(name="gsb", bufs=8) as gsp, \
             tc.tile_pool(name="work", bufs=3) as wp, \
             tc.tile_pool(name="small", bufs=4) as sp, \
             tc.tile_pool(name="tps", bufs=4, space="PSUM") as tps, \
             tc.tile_pool(name="aggps", bufs=2, space="PSUM") as aggps, \
             tc.tile_pool(name="mmps", bufs=2, space="PSUM") as mmps, \
             tc.tile_pool(name="dram", bufs=1, space="DRAM") as dp:

            w1 = cp.tile([F, H], f16); nc.sync.dma_start(w1[:], w1_t.ap())
            w2 = cp.tile([H, H], f16); nc.sync.dma_start(w2[:], w2_t.ap())
            w3 = cp.tile([H, H], f16); nc.sync.dma_start(w3[:], w3_t.ap())
            w4 = cp.tile([H, CLS], f16); nc.sync.dma_start(w4[:], w4_t.ap())
            b1 = cp.tile([H, 1], f32); nc.sync.dma_start(b1[:], b1_t.ap())
            b2 = cp.tile([H, 1], f32); nc.sync.dma_start(b2[:], b2_t.ap())
            b3 = cp.tile([H, 1], f32); nc.sync.dma_start(b3[:], b3_t.ap())
            b4 = cp.tile([CLS, 1], f32); nc.sync.dma_start(b4[:], b4_t.ap())
            id16 = cp.tile([128, 128], f16)
            nc.sync.dma_start(id16[:], id16_t.ap())
            id32 = cp.tile([128, 128], f32)
            nc.sync.dma_start(id32[:], id32_t.ap())
            gidx_sb = cp.tile([128, tot // 16], mybir.dt.int16)
            nc.sync.dma_start(gidx_sb[:], gidx_t.ap())
            zt = cp.tile([128, 128], f16)
            nc.vector.memset(zt[:], 0.0)

            agg_sb = cp.tile([128, NT, 128], f16)
            # one contiguous DRAM buffer per AllGather range (collective
            # inputs must be contiguous; a column slice of one big buffer
            # fails BIR verification)
            h_own = [dp.tile([128, (hi - lo) * 128], f16, name=f"hown{j}")
                     for j, (lo, hi) in enumerate(AG_RANGES)]

            def own_rng(t):
                for j, (lo, hi) in enumerate(AG_RANGES):
                    if lo <= t < hi:
                        return j, lo
                raise AssertionError(t)
            ag_space = ("Shared" if int(os.environ.get("GIN_AG_SHARED", "0"))
                        else "Local")
            h_ag = [dp.tile([NCORES * 128, (hi - lo) * 128], f16,
                            addr_space=ag_space, name=f"hag{j}")
                    for j, (lo, hi) in enumerate(AG_RANGES)]

            def load1(qq, table):
                nc.sync.dma_start(table[:], xpm[qq].ap())

            def load2(qq, table):
                for j, (lo, hi) in enumerate(AG_RANGES):
                    for k in range(2):
                        core = 2 * qq + k
                        nc.sync.dma_start(
                            table[:, k * NRL + lo: k * NRL + hi, :],
                            h_ag[j][:][core * 128:(core + 1) * 128, :])

            def layer(loader, own_of, is_first):
                for qq in range(4):
                    table = tp.tile([128, RANKS, F], f16, tag="tbl")
                    loader(qq, table)
                    for b, tiles in enumerate(batches):
                        S = call_slots[b][qq]
                        o = call_off[b][qq]
                        gt = None
                        oh = None
                        if S:
                            gt = gp.tile([128, 1, maxS], f16, tag="gt")
                            nc.gpsimd.dma_gather(
                                gt[:, :, :S], table[:],
                                gidx_sb[:, o // 16:(o + S) // 16], S, S, F,
                                transpose=True, single_packet=single_packet,
                                queue_num=b % nqueues,
                                sbuf_tokens_per_rank=128,
                                sbuf_free_dim_per_rank=256)
                            glist = groups_bq[(b, qq)]
                            nohc = (glist[-1][4] + glist[-1][2] - glist[-1][1]
                                    if glist else 0)
                            if nohc:
                                oh = ohp.tile([128, maxOHC, 128], oh_dt,
                                              tag="oh")
                                od = b * 4 + qq
                                nc.scalar.dma_start(
                                    oh[:, :nohc, :],
                                    ohin_t.ap()[:, od * maxOHC:
                                                od * maxOHC + nohc, :])
                        for t in tiles:
                            pieces = []
                            for (tt, c0, c1, lvl, op_) in groups_bq[(b, qq)]:
                                if tt != t:
                                    continue
                                for j in range(c1 - c0):
                                    pieces.append((c0 + j, op_ + j))
                            if qq > 0 and not pieces:
                                continue
                            # transpose all pieces FIRST (plain matmuls vs
                            # identity), then run the agg accumulation group
                            # contiguously: interleaving start=True matmuls
                            # into an open PSUM accumulation group corrupts
                            # partials on HW (bank-granular start-zeroing)
                            # even though CoreSim accepts it.
                            g2s = []
                            for k, (col, ohcol) in enumerate(pieces):
                                pst = tps.tile([128, 128], f32, tag="pst")
                                nc.tensor.matmul(
                                    out=pst[:],
                                    lhsT=gt[:, 0, col * 128:(col + 1) * 128],
                                    rhs=id16[:], start=True, stop=True)
                                g2 = gsp.tile([128, 128], f16, tag="g2")
                                if k % 2 == 0:
                                    nc.vector.tensor_copy(out=g2[:],
                                                          in_=pst[:])
                                else:
                                    nc.scalar.activation(out=g2[:],
                                                         in_=pst[:],
                                                         func=AT.Copy)
                                g2s.append((g2, ohcol))
                            agg = aggps.tile([128, 128], f32, tag="agg")
                            kbase = 0
                            if qq == 0:
                                # self term: (1+eps)*h_t via identity matmul
                                own = sp.tile([128, 128], f16, tag="own")
                                nc.scalar.dma_start(own[:], own_of(t))
                                nc.tensor.matmul(
                                    out=agg[:], lhsT=own[:], rhs=id16[:],
                                    start=True, stop=(len(pieces) == 0))
                                kbase = 1
                            for k, (g2, ohcol) in enumerate(g2s):
                                nc.tensor.matmul(
                                    out=agg[:], lhsT=g2[:],
                                    rhs=oh[:, ohcol, :],
                                    start=(kbase == 0 and k == 0),
                                    stop=(k == len(g2s) - 1))
                            if qq == 0:
                                nc.vector.tensor_copy(out=agg_sb[:, t, :],
                                                      in_=agg[:])
                            else:
                                nc.vector.tensor_tensor(
                                    out=agg_sb[:, t, :], in0=agg[:],
                                    in1=agg_sb[:, t, :], op=OP.add)

                # MLP over all tiles from the SBUF agg buffer
                for t in range(NT):
                    rows = 128 if t < NT - 1 else last_rows
                    if is_first:
                        ps1 = mmps.tile([128, 128], f32, tag="mm")
                        nc.tensor.matmul(out=ps1[:], lhsT=w1[:],
                                         rhs=agg_sb[:, t, :],
                                         start=True, stop=True)
                        h1 = wp.tile([128, 128], f16, tag="h1")
                        nc.scalar.activation(out=h1[:], in_=ps1[:],
                                             func=AT.Relu, bias=b1[:])
                        ps2 = mmps.tile([128, 128], f32, tag="mm")
                        nc.tensor.matmul(out=ps2[:], lhsT=w2[:], rhs=h1[:],
                                         start=True, stop=True)
                        h2 = wp.tile([128, 128], f16, tag="h2")
                        nc.scalar.activation(out=h2[:], in_=ps2[:],
                                             func=AT.Relu, bias=b2[:])
                        pst = tps.tile([128, 128], f32, tag="pst")
                        nc.tensor.matmul(out=pst[:], lhsT=h2[:], rhs=id16[:],
                                         start=True, stop=True)
                        hst = sp.tile([128, 128], f16, tag="hst")
                        nc.vector.tensor_copy(out=hst[:], in_=pst[:])
                        jr, lo_r = own_rng(t)
                        nc.sync.dma_start(
                            h_own[jr][:][:, (t - lo_r) * 128:
                                         (t - lo_r + 1) * 128], hst[:])
                        if t == NT - 1 and last_rows < 128:
                            # zero the pad rows (virtual >= NPC)
                            nc.sync.dma_start(
                                h_own[jr][:][last_rows:128,
                                             (t - lo_r) * 128:
                                             (t - lo_r + 1) * 128],
                                zt[:128 - last_rows, :])
                        for j, (lo, hi) in enumerate(AG_RANGES):
                            if t == hi - 1:
                                nc.gpsimd.collective_compute(
                                    "AllGather", OP.bypass,
                                    replica_groups=[list(range(NCORES))],
                                    ins=[h_own[j][:]],
                                    outs=[h_ag[j][:]])
                    else:
                        ps1 = mmps.tile([128, 128], f32, tag="mm")
                        nc.tensor.matmul(out=ps1[:], lhsT=w3[:],
                                         rhs=agg_sb[:, t, :],
                                         start=True, stop=True)
                        h3 = wp.tile([128, 128], f16, tag="h1")
                        nc.scalar.activation(out=h3[:], in_=ps1[:],
                                             func=AT.Relu, bias=b3[:])
                        ps2 = mmps.tile([128, 128], f32, tag="mm")
                        nc.tensor.matmul(out=ps2[:CLS, :128],
                                         lhsT=w4[:], rhs=h3[:],
                                         start=True, stop=True)
                        c4 = sp.tile([CLS, 128], f32, tag="c4")
                        nc.vector.tensor_tensor(
                            out=c4[:], in0=ps2[:CLS, :128],
                            in1=b4[:].broadcast_to([CLS, 128]), op=OP.add)
                        psf = mmps.tile([128, 128], f32, tag="mm")
                        nc.tensor.transpose(out=psf[:128, :CLS], in_=c4[:],
                                            identity=id32[:CLS, :CLS])
                        mx = sp.tile([128, 1], f32, tag="mx")
                        nc.vector.tensor_reduce(
                            out=mx[:], in_=psf[:128, :CLS],
                            axis=mybir.AxisListType.X, op=OP.max)
                        tsh = sp.tile([128, CLS], f32, tag="tsh")
                        nc.vector.tensor_tensor(
                            out=tsh[:], in0=psf[:128, :CLS],
                            in1=mx[:].broadcast_to([128, CLS]),
                            op=OP.subtract)
                        esum = sp.tile([128, 1], f32, tag="esum")
                        edum = sp.tile([128, CLS], f32, tag="edum")
                        nc.scalar.activation(out=edum[:], in_=tsh[:],
                                             func=AT.Exp, accum_out=esum[:])
                        lse = sp.tile([128, 1], f32, tag="lse")
                        nc.scalar.activation(out=lse[:], in_=esum[:],
                                             func=AT.Ln)
                        osb = sp.tile([128, CLS], f32, tag="osb")
                        nc.vector.tensor_tensor(
                            out=osb[:], in0=tsh[:],
                            in1=lse[:].broadcast_to([128, CLS]),
                            op=OP.subtract)
                        nc.sync.dma_start(
                            out_t.ap()[t * 128: t * 128 + rows, :],
                            osb[:rows, :])

            repeat = int(os.environ.get("GIN_REPEAT", "1"))
            def own2(t):
                j, lo = own_rng(t)
                return h_own[j][:][:, (t - lo) * 128:(t - lo + 1) * 128]

            for _rep in range(repeat):
                layer(load1, lambda t: xown_t.ap()[:, t, :], is_first=True)
                layer(load2, own2, is_first=False)

    nc.compile()
    return nc


def _run_v2(inputs, cfg):
    from concourse.bass_utils import run_bass_kernel_spmd

    # v2 needs smaller tile batches than v1 to fit its SBUF tables
    cfg = Cfg(cfg.N, cfg.F, cfg.H, cfg.CLS,
              tiles_per_batch=int(os.environ.get("GIN_B2", "4")))

    x = np.asarray(inputs["x"], np.float32)
    edge_index = np.asarray(inputs["edge_index"])
    eps1 = float(np.asarray(inputs["eps1"]))
    eps2 = float(np.asarray(inputs["eps2"]))

    sched, gidx_all, dstloc_all = _prep_v2(edge_index, cfg)
    nc = _build_nc_v2(cfg, sched, eps1, eps2)

    xq = _perm_v2(x.astype(F16), cfg)
    np_ohdt = {"fp8": ml_dtypes.float8_e4m3,
               "fp16": np.float16}[os.environ.get("GIN_OH_DT", "fp16")]
    base = {
        "w1": np.asarray(inputs["w1"], np.float16),
        "w2": np.asarray(inputs["w2"], np.float16),
        "w3": np.asarray(inputs["w3"], np.float16),
        "w4": np.asarray(inputs["w4"], np.float16),
        "b1": np.asarray(inputs["b1"], np.float32).reshape(-1, 1),
        "b2": np.asarray(inputs["b2"], np.float32).reshape(-1, 1),
        "b3": np.asarray(inputs["b3"], np.float32).reshape(-1, 1),
        "b4": np.asarray(inputs["b4"], np.float32).reshape(-1, 1),
        "id16": np.eye(128, dtype=np.float16),
        "id32": np.eye(128, dtype=np.float32),
    }
    for q in range(4):
        base[f"xpm{q}"] = xq[q]
    xown = _own_pm(x.astype(F16), cfg)

    in_maps = []
    for c in range(NCORES):
        m = dict(base)
        m["gidx"] = gidx_all[c]
        m["xown"] = xown[c]
        m["ohin"] = _host_onehots(sched, dstloc_all[c], np_ohdt)
        in_maps.append(m)

    bench = int(os.environ.get("GIN_BENCH", "0"))
    trace = int(os.environ.get("GIN_TRACE", "0"))
    if bench:
        results = _exec_spmd_timed(nc, in_maps, bench)
    elif trace:
        res = run_bass_kernel_spmd(nc, in_maps, core_ids=list(range(NCORES)),
                                   trace=True,
                                   tmpdir=os.environ.get("GIN_TRACE_DIR"))
        print("TRACE exec_time_ns:", res.exec_time_ns)
        results = res.results
    else:
        res = run_bass_kernel_spmd(nc, in_maps, core_ids=list(range(NCORES)))
        results = res.results
    out = np.concatenate([r["out"] for r in results], axis=0)
    return out.astype(np.float32)


LAST_BENCH_NS = None
DEFERRED = []


def _exec_spmd_timed(nc, in_maps, iters):
    """jit-once SPMD exec (mirrors bass2jax.run_bass_via_pjrt) + steady-state
    timing of repeated NEFF executions. Returns per-core result dicts."""
    import time

    import jax
    import numpy as _np
    from jax.experimental.shard_map import shard_map
    from jax.sharding import Mesh, NamedSharding, PartitionSpec

    from concourse import bass2jax, mybir
    from concourse.bass2jax import _bass_exec_p, install_neuronx_cc_hook, \
        partition_id_tensor

    install_neuronx_cc_hook()
    n_cores = len(in_maps)
    partition_name = (nc.partition_id_tensor.name
                      if nc.partition_id_tensor else None)
    in_names, out_names, out_avals, zero_outs = [], [], [], []
    for alloc in nc.m.functions[0].allocations:
        if not isinstance(alloc, mybir.MemoryLocationSet):
            continue
        name = alloc.memorylocations[0].name
        if alloc.kind == "ExternalInput":
            if name != partition_name:
                in_names.append(name)
        elif alloc.kind == "ExternalOutput":
            out_names.append(name)
            shape = tuple(alloc.tensor_shape)
            dtype = mybir.dt.np(alloc.dtype)
            out_avals.append(jax.core.ShapedArray(shape, dtype))
            zero_outs.append(_np.zeros(shape, dtype))
    n_params = len(in_names)
    all_in_names = in_names + out_names
    if partition_name is not None:
        all_in_names = all_in_names + [partition_name]

    chain = int(os.environ.get("GIN_CHAIN", "1"))

    def _body(*args):
        ins = list(args[:n_params])
        outs = list(args[n_params:])
        for _ in range(chain):
            operands = ins + outs
            if partition_name is not None:
                operands.append(partition_id_tensor())
            outs = list(_bass_exec_p.bind(
                *operands,
                out_avals=tuple(out_avals),
                in_names=tuple(all_in_names),
                out_names=tuple(out_names),
                lowering_input_output_aliases=(),
                sim_require_finite=True,
                sim_require_nnan=True,
                nc=nc,
            ))
        return tuple(outs)

    devices = jax.devices()[:n_cores]
    mesh = Mesh(_np.asarray(devices), ("core",))
    spec = PartitionSpec("core")
    n_outs = len(out_names)
    fn = jax.jit(
        shard_map(_body, mesh=mesh, in_specs=(spec,) * (n_params + n_outs),
                  out_specs=(spec,) * n_outs, check_rep=False),
        keep_unused=True,
    )
    sh = NamedSharding(mesh, spec)
    concat_in = [
        jax.device_put(
            _np.concatenate([_np.asarray(in_maps[c][nm]) for c in
                             range(n_cores)], axis=0), sh)
        for nm in in_names
    ]
    concat_zeros = [
        jax.device_put(_np.zeros((n_cores * z.shape[0], *z.shape[1:]),
                                 z.dtype), sh)
        for z in zero_outs
    ]
    out = jax.block_until_ready(fn(*concat_in, *concat_zeros))
    if os.environ.get("GIN_DEFER"):
        DEFERRED.append((fn, concat_in, concat_zeros))
        times = [0.0]
    else:
        times = []
        for _ in range(max(iters, 1)):
            t0 = time.perf_counter()
            r = jax.block_until_ready(fn(*concat_in, *concat_zeros))
            times.append((time.perf_counter() - t0) * 1e9)
    global LAST_BENCH_NS
    LAST_BENCH_NS = times
    res = []
    for c in range(n_cores):
        res.append({
            nm: _np.asarray(out[i]).reshape(n_cores, *out_avals[i].shape)[c]
            for i, nm in enumerate(out_names)
        })
    return res


def _run(inputs, cfg):
    from concourse.bass_utils import run_bass_kernel_spmd

    x = np.asarray(inputs["x"], np.float32)
    edge_index = np.asarray(inputs["edge_index"])
    eps1 = float(np.asarray(inputs["eps1"]))
    eps2 = float(np.asarray(inputs["eps2"]))

    sched, gidx_all, dstloc_all = _prep_graph(edge_index, cfg)
    xqs = _perm_rows(x.astype(F16), cfg)

    nc = _build_nc(cfg, sched, eps1, eps2)

    iota_np = np.tile(np.arange(512, dtype=np.float32),
                      (128, 1)).astype(F16)
    ident_np = np.eye(128, dtype=np.float32)
    base = {
        "w1": np.asarray(inputs["w1"], np.float32),
        "w2": np.asarray(inputs["w2"], np.float32),
        "w3": np.asarray(inputs["w3"], np.float32),
        "w4": np.asarray(inputs["w4"], np.float32),
        "b1": np.asarray(inputs["b1"], np.float32).reshape(-1, 1),
        "b2": np.asarray(inputs["b2"], np.float32).reshape(-1, 1),
        "b3": np.asarray(inputs["b3"], np.float32).reshape(-1, 1),
        "b4": np.asarray(inputs["b4"], np.float32).reshape(-1, 1),
        "iota": iota_np,
        "ident": ident_np,
    }
    elem = int(os.environ.get("GIN_ELEM", str(cfg.F)))
    shift = (elem // cfg.F).bit_length() - 1
    for q in range(4):
        base[f"xq{q}"] = np.ascontiguousarray(
            xqs[q].reshape(-1, elem))

    oh_host = bool(int(os.environ.get("GIN_OH_HOST", "0")))
    np_ohdt = {"fp8": ml_dtypes.float8_e4m3,
               "fp16": np.float16}[os.environ.get("GIN_OH_DT", "fp16")]

    xbf = x.astype(F16)
    in_maps = []
    for c in range(NCORES):
        m = dict(base)
        gwr, g32 = gidx_all[c]
        m["gidx"] = gwr >> shift if shift else gwr
        m["gidx32"] = g32 >> shift if shift else g32
        m["dstloc"] = dstloc_all[c]
        m["xown"] = xbf[c * cfg.NPC:(c + 1) * cfg.NPC]
        if oh_host:
            m["ohin"] = _host_onehots(sched, dstloc_all[c], np_ohdt)
        in_maps.append(m)

    bench = int(os.environ.get("GIN_BENCH", "0"))
    trace = int(os.environ.get("GIN_TRACE", "0"))
    if bench:
        results = _exec_spmd_timed(nc, in_maps, bench)
    elif trace:
        tdir = os.environ.get("GIN_TRACE_DIR") or None
        tcores = [int(c) for c in
                  os.environ.get("GIN_TRACE_CORES", "0").split(",")]
        res = run_bass_kernel_spmd(nc, in_maps, core_ids=list(range(NCORES)),
                                   trace=True, tmpdir=tdir,
                                   trace_cores=tcores)
        print("TRACE exec_time_ns:", res.exec_time_ns,
              "mean:", res.mean_exec_time_ns)
        results = res.results
    else:
        res = run_bass_kernel_spmd(nc, in_maps, core_ids=list(range(NCORES)))
        results = res.results
    out = np.concatenate([r["out"] for r in results], axis=0)
    return out.astype(np.float32)


def kernel(**inputs):
    if os.environ.get("GIN_V2", "0") == "1":
        return _run_v2(inputs, FULL)
    return _run(inputs, FULL)



# revision 31
# speedup vs baseline: 2.1223x; 1.0333x over previous
"""GIN 2-layer message-passing network on 8 Trainium2 NeuronCores.

Strategy (matches the dst-partitioned sharding hint):
  - Nodes are split into 8 contiguous chunks of N/8; core c owns chunk c and
    all edges whose destination lands in it. The `+ 1*h_i` self term is NOT
    materialized as self-edges: each destination tile adds its own dense
    feature rows via one identity matmul (saves ~6% of gather slots).
  - segment_sum becomes, per core: dma_gather of source-node feature rows
    (fp16) in edge order (4 SWDGE queues — the gather is descriptor-count
    bound at ~5.3 ns/row, so slot count is the kernel's critical resource),
    then a segment-sum on the tensor engine using one-hot matrices built on
    the vector engine, accumulating in PSUM per 128-destination-node tile:
        aggT[feat, dst] += G[edges, feat].T-contract-. onehot[edges, dst]
  - Edge groups (tile, quarter) are padded to 32 slots, not 128. Groups
    sharing a 128-slot column are disambiguated by a *level*: one-hot
    columns are shifted by 128*level (fp16 iota 0..511), so a full-column
    matmul for one group sees exact zeros on the other groups' rows. This
    avoids PE partition-offset matmuls, which hang real HW.
  - The one-hot depends only on the edge structure, so layer 1 stores it to
    DRAM and layer 2 reloads it with dense DMA instead of re-running the
    (slow, ~76 G elem/s) DVE broadcast compare.
  - The MLP runs in "transposed land" ([feat, nodes] layout) so activations
    never need transposing between matmuls; per tile:
        h = relu(w.T @ aggT + b)  via PE matmul + ACT relu-with-bias.
  - Between the two GIN layers the per-core h chunks are exchanged with 4
    AllGathers (one per quarter of each core's rows) so layer-2 gathers can
    index any node with int16 indices (< 32768 rows per gather source).
  - log_softmax of the final [40, nodes] tile is done after a PE transpose
    back to [nodes, 40]: row-max, subtract, exp-with-accumulated-sum (ACT),
    ln, subtract.

All per-core variability lives in the *data* (index / one-hot-column arrays,
padded to a per-group max across cores) so a single SPMD NEFF serves all 8
cores.
"""

import os
import sys

sys.path.insert(0, "/opt/trn_rl_repo")
sys.path.insert(0, "/opt/trn_rl_repo/concourse")
os.environ.setdefault("TRN_TYPE", "TRN2")

import numpy as np
import ml_dtypes

BF16 = ml_dtypes.bfloat16
F16 = np.float16

NCORES = 8


class Cfg:
    def __init__(self, n, feat, hid, cls, tiles_per_batch=5):
        assert n % (NCORES * 4) == 0
        self.N = n
        self.F = feat          # input feature dim (must be 128 here)
        self.H = hid           # hidden dim (128)
        self.CLS = cls         # classes
        self.NPC = n // NCORES          # nodes per core
        self.QROWS = self.NPC // 4      # rows per quarter per core
        self.SRCROWS = self.QROWS * NCORES  # rows per gather source tensor
        self.NT = -(-self.NPC // 128)   # dst tiles per core
        self.last_rows = self.NPC - (self.NT - 1) * 128
        self.B = tiles_per_batch


FULL = Cfg(100000, 128, 128, 40,
           tiles_per_batch=int(os.environ.get("GIN_B", "4")))


def _prep_graph(edge_index, cfg):
    """Host-side sharding: returns (schedule, per-core index arrays).

    schedule: dict with
      slots[t*4+q]   padded slot count per (tile, quarter) group (max/cores)
      batches        list of lists of tile ids
      call_slots[b][q], call_off[b][q], slot_off maps for emission
    per-core: gidx_wr [128, TOT//16] int16, dstloc [128, TOT//128] bf16
    """
    N, NPC, QROWS, NT = cfg.N, cfg.NPC, cfg.QROWS, cfg.NT
    noself = bool(int(os.environ.get("GIN_NOSELF", "1")))
    pad = int(os.environ.get("GIN_PAD", "32"))
    src = np.asarray(edge_index[0], dtype=np.int64)
    dst = np.asarray(edge_index[1], dtype=np.int64)
    if not noself:
        # self-edges give the +h_i term of the GIN aggregate
        allid = np.arange(N, dtype=np.int64)
        src = np.concatenate([src, allid])
        dst = np.concatenate([dst, allid])

    core = dst // NPC
    per_core = []
    counts = np.zeros((NCORES, NT * 4), np.int64)
    for c in range(NCORES):
        m = core == c
        s = src[m]
        dloc = (dst[m] - c * NPC).astype(np.int64)
        t = dloc >> 7
        q = (s % NPC) // QROWS
        gidxv = (s // NPC) * QROWS + (s % QROWS)
        dstin = dloc & 127
        gid = (t * 4 + q).astype(np.int64)
        counts[c] = np.bincount(gid, minlength=NT * 4)
        per_core.append((gid, gidxv.astype(np.int32), dstin.astype(np.int32)))

    cmax = counts.max(axis=0)                       # [NT*4]
    slots = -(-cmax // pad) * pad                   # slots per (t,q)
    # batches of tiles
    B = cfg.B
    batches = [list(range(b, min(b + B, NT))) for b in range(0, NT, B)]
    # slot offsets in (b, q, t) order; call boundaries stay 128-aligned
    off = 0
    slot_off = np.zeros(NT * 4, np.int64)
    call_slots = []           # [b][q] -> num slots in that gather call
    call_off = []             # [b][q] -> slot offset of call start
    # level disambiguates groups sharing a 128-slot column: one-hot columns
    # are shifted by 128*level, so a full-column matmul for one group sees
    # zeros on the other groups' rows.
    level = np.zeros(NT * 4, np.int64)
    for tiles in batches:
        cs, co = [], []
        for q in range(4):
            co.append(off)
            s0 = off
            for i, t in enumerate(tiles):
                level[t * 4 + q] = i % 4
                slot_off[t * 4 + q] = off
                off += slots[t * 4 + q]
            off = -(-off // 128) * 128      # align next call to 128
            cs.append(off - s0)
        call_slots.append(cs)
        call_off.append(co)
    tot = off
    assert tot % 128 == 0

    gidx_all, dstloc_all = [], []
    for c in range(NCORES):
        gid, gidxv, dstin = per_core[c]
        order = np.argsort(gid, kind="stable")
        gs = gid[order]
        cnt = counts[c]
        starts = np.zeros(NT * 4, np.int64)
        np.cumsum(cnt[:-1], out=starts[1:])
        rank = np.arange(len(gs)) - starts[gs]
        slot = slot_off[gs] + rank
        gflat = np.zeros(tot, np.int16)
        dflat = np.full(tot, 2000.0, np.float32)
        gflat[slot] = gidxv[order].astype(np.int16)
        # one-hot column = dst-in-tile + 128*level of the slot's group
        dflat[slot] = dstin[order] + 128.0 * level[gs]
        # wrap for dma_gather: [p, col] = gflat[col*16 + p%16], replicated x8
        gwr = np.tile(gflat.reshape(tot // 16, 16).T, (8, 1)).copy()
        dloc = dflat.reshape(tot // 128, 128).T.astype(F16).copy()
        gidx_all.append((gwr, gflat.astype(np.int32).reshape(128, tot // 128)))
        dstloc_all.append(dloc)

    sched = dict(slots=slots, batches=batches, call_slots=call_slots,
                 call_off=call_off, slot_off=slot_off, tot=tot,
                 noself=noself, level=level)
    return sched, gidx_all, dstloc_all


def _perm_rows(x, cfg):
    """x [N, F] -> 4 arrays [SRCROWS, F]; source s holds global row
    g = r*NPC + s*QROWS + u at position r*QROWS + u."""
    N, NPC, QROWS = cfg.N, cfg.NPC, cfg.QROWS
    g = np.arange(N)
    s = (g % NPC) // QROWS
    pos = (g // NPC) * QROWS + (g % QROWS)
    out = []
    for si in range(4):
        m = s == si
        a = np.empty((cfg.SRCROWS, x.shape[1]), x.dtype)
        a[pos[m]] = x[m]
        out.append(a)
    return out


def _groups(sched):
    """Per (b, q): group list [(tile, c0, c1, lvl, ohpos)] with call-relative
    column ranges; boundary columns appear in two groups."""
    slots_arr = sched["slots"]
    slot_off = sched["slot_off"]
    batches = sched["batches"]
    call_off = sched["call_off"]
    level = sched["level"]
    groups_bq = {}
    maxOHC = 0
    for b, tiles in enumerate(batches):
        for q in range(4):
            lst, ohpos = [], 0
            for t in tiles:
                ns = int(slots_arr[t * 4 + q])
                if ns == 0:
                    continue
                rel = int(slot_off[t * 4 + q] - call_off[b][q])
                c0, c1 = rel // 128, -(-(rel + ns) // 128)
                lst.append((t, c0, c1, int(level[t * 4 + q]), ohpos))
                ohpos += c1 - c0
            groups_bq[(b, q)] = lst
            maxOHC = max(maxOHC, ohpos)
    return groups_bq, maxOHC


def _host_onehots(sched, dstloc, np_dt):
    """Host-precomputed one-hot planes, laid out exactly like the device
    oh_dram reuse buffer: [128, ncalls*maxOHC, 128]."""
    groups_bq, maxOHC = _groups(sched)
    batches = sched["batches"]
    call_off = sched["call_off"]
    ncalls = len(batches) * 4
    out = np.zeros((128, ncalls * maxOHC, 128), np_dt)
    cols = np.arange(128, dtype=np.float32)
    dl32 = dstloc.astype(np.float32)
    for b in range(len(batches)):
        for q in range(4):
            od = b * 4 + q
            base = call_off[b][q] // 128
            for (t, c0, c1, lvl, op_) in groups_bq[(b, q)]:
                for j in range(c1 - c0):
                    v = dl32[:, base + c0 + j]
                    out[:, od * maxOHC + op_ + j, :] = (
                        v[:, None] == (128.0 * lvl + cols)).astype(np_dt)
    return out


def _build_nc(cfg, sched, eps1, eps2):
    from concourse import mybir
    import concourse.bacc as bacc
    import concourse.tile as tile

    F, H, CLS, NT, NPC = cfg.F, cfg.H, cfg.CLS, cfg.NT, cfg.NPC
    slots_arr = sched["slots"]
    slot_off = sched["slot_off"]
    noself = sched["noself"]
    batches = sched["batches"]
    call_slots = sched["call_slots"]
    tot = sched["tot"]
    f32 = mybir.dt.float32
    bf16 = mybir.dt.float16      # data-path dtype (fp16: exact ints to 2048)
    AT = mybir.ActivationFunctionType
    OP = mybir.AluOpType
    level = sched["level"]
    call_off = sched["call_off"]

    groups_bq, maxOHC = _groups(sched)

    assert eps1 == 0.0 and eps2 == 0.0, "nonzero eps not implemented"

    nqueues = int(os.environ.get("GIN_QUEUES", "4"))
    single_packet = bool(int(os.environ.get("GIN_SP", "0")))
    scratch = int(os.environ.get("GIN_SCRATCH", "32768"))
    nc = bacc.Bacc("TRN2", target_bir_lowering=False, debug=False,
                   num_devices=NCORES, num_swdge_queues=nqueues,
                   dynamic_dma_scratch_size=scratch)

    oh_host = bool(int(os.environ.get("GIN_OH_HOST", "0")))
    # pipeline depth for the gather/one-hot pools: 6 buffers = only 1.5
    # batches of lookahead, which stalls the DMA gather stream on buffer
    # recycling behind the DVE one-hot builds. SBUF has headroom for 10.
    depth = int(os.environ.get("GIN_DEPTH", "12"))
    # fp8 one-hots are exact in sim but cost ~2% rel err on HW (the PE
    # appears to quantize the fp16 operand in mixed-dtype mode) -> fp16.
    oh_dt = {"fp8": mybir.dt.float8e4, "fp16": mybir.dt.float16}[
        os.environ.get("GIN_OH_DT", "fp16")]
    qspread = bool(int(os.environ.get("GIN_QSPREAD", "0")))

    elem = int(os.environ.get("GIN_ELEM", str(F)))  # bench knob: 256/512
    xq = [nc.dram_tensor(f"xq{q}", [cfg.SRCROWS * F // elem, elem], bf16,
                         kind="ExternalInput")
          for q in range(4)]
    w1_t = nc.dram_tensor("w1", [F, H], f32, kind="ExternalInput")
    w2_t = nc.dram_tensor("w2", [H, H], f32, kind="ExternalInput")
    w3_t = nc.dram_tensor("w3", [H, H], f32, kind="ExternalInput")
    w4_t = nc.dram_tensor("w4", [H, CLS], f32, kind="ExternalInput")
    b1_t = nc.dram_tensor("b1", [H, 1], f32, kind="ExternalInput")
    b2_t = nc.dram_tensor("b2", [H, 1], f32, kind="ExternalInput")
    b3_t = nc.dram_tensor("b3", [H, 1], f32, kind="ExternalInput")
    b4_t = nc.dram_tensor("b4", [CLS, 1], f32, kind="ExternalInput")
    xown_t = nc.dram_tensor("xown", [NPC, F], bf16, kind="ExternalInput")
    iota_t = nc.dram_tensor("iota", [128, 512], bf16, kind="ExternalInput")
    ident_t = nc.dram_tensor("ident", [128, 128], f32, kind="ExternalInput")
    gidx_t = nc.dram_tensor("gidx", [128, tot // 16], mybir.dt.int16,
                            kind="ExternalInput")
    indirect = bool(int(os.environ.get("GIN_INDIRECT", "0")))
    gidx32_t = nc.dram_tensor("gidx32", [128, tot // 128], mybir.dt.int32,
                              kind="ExternalInput")
    dstloc_t = nc.dram_tensor("dstloc", [128, tot // 128], bf16,
                              kind="ExternalInput")
    ohin_t = None
    if oh_host:
        ncalls_all = len(batches) * 4
        ohin_t = nc.dram_tensor("ohin", [128, ncalls_all * maxOHC, 128],
                                oh_dt, kind="ExternalInput")
    out_t = nc.dram_tensor("out", [NPC, CLS], f32, kind="ExternalOutput")

    maxS = max(max(cs) for cs in call_slots)

    with tile.TileContext(nc) as tc:
        with tc.tile_pool(name="const", bufs=1) as cp, \
             tc.tile_pool(name="gp", bufs=depth) as gp, \
             tc.tile_pool(name="ohp", bufs=depth) as ohp, \
             tc.tile_pool(name="work", bufs=4) as wp, \
             tc.tile_pool(name="small", bufs=4) as sp, \
             tc.tile_pool(name="aggps", bufs=4, space="PSUM") as aggps, \
             tc.tile_pool(name="mmps", bufs=4, space="PSUM") as mmps, \
             tc.tile_pool(name="dram", bufs=1, space="DRAM") as dp:

            w1 = cp.tile([F, H], f32); nc.sync.dma_start(w1[:], w1_t.ap())
            w2 = cp.tile([H, H], f32); nc.sync.dma_start(w2[:], w2_t.ap())
            w3 = cp.tile([H, H], f32); nc.sync.dma_start(w3[:], w3_t.ap())
            w4 = cp.tile([H, CLS], f32); nc.sync.dma_start(w4[:], w4_t.ap())
            b1 = cp.tile([H, 1], f32); nc.sync.dma_start(b1[:], b1_t.ap())
            b2 = cp.tile([H, 1], f32); nc.sync.dma_start(b2[:], b2_t.ap())
            b3 = cp.tile([H, 1], f32); nc.sync.dma_start(b3[:], b3_t.ap())
            b4 = cp.tile([CLS, 1], f32); nc.sync.dma_start(b4[:], b4_t.ap())
            iota = cp.tile([128, 512], bf16); nc.sync.dma_start(iota[:], iota_t.ap())
            ident = cp.tile([128, 128], f32); nc.sync.dma_start(ident[:], ident_t.ap())
            gidx_sb = cp.tile([128, tot // 16], mybir.dt.int16)
            nc.sync.dma_start(gidx_sb[:], gidx_t.ap())
            gidx32_sb = None
            if indirect:
                gidx32_sb = cp.tile([128, tot // 128], mybir.dt.int32)
                nc.sync.dma_start(gidx32_sb[:], gidx32_t.ap())
            dstloc_sb = cp.tile([128, tot // 128], bf16)
            nc.sync.dma_start(dstloc_sb[:], dstloc_t.ap())

            h_own = dp.tile([NPC, H], bf16)
            # rebuilding the one-hot on DVE each layer overlaps with the
            # gather DMA stream; the DRAM store+reload contends with it
            # (measured: 1.78ms rebuild vs 2.00ms reuse).
            oh_reuse = bool(int(os.environ.get("GIN_OH_REUSE", "0")))
            oh_dram = None
            ncalls = len(batches) * 4
            if oh_reuse and not oh_host:
                oh_dram = dp.tile([128, ncalls * maxOHC, 128], bf16)
            # dp.tile (unlike tc.tile) forwards addr_space; Shared is the
            # collectives fast path for HBM-HBM AllGather outputs.
            ag_space = ("Shared" if int(os.environ.get("GIN_AG_SHARED", "0"))
                        else "Local")
            no_ag = bool(os.environ.get("GIN_NO_AG"))
            h_ag = [dp.tile([cfg.SRCROWS, H], bf16, addr_space=ag_space,
                            name=f"h_ag{s}")
                    for s in range(4)]

            skips = set(os.environ.get("GIN_SKIP", "").split(","))
            identb = None
            if noself:
                identb = cp.tile([128, 128], bf16)
                nc.vector.tensor_copy(out=identb[:], in_=ident[:])

            def layer(sources, is_first, batch_limit=None, skip_mlp=False,
                      own_src=None):
                """Emit one GIN layer. sources: list of 4 gather-source APs.
                own_src(t, rows) -> DRAM AP of this tile's own feature rows
                (realizes the +1*h_i self term via an identity matmul)."""
                ag_emitted = [False] * 4
                for b, tiles in enumerate(batches):
                    if batch_limit is not None and b >= batch_limit:
                        break
                    G, OH = [], []
                    for q in range(4):
                        S = call_slots[b][q]
                        if S == 0:
                            G.append(None); OH.append(None)
                            continue
                        o = sched["call_off"][b][q]
                        gi = gidx_sb[:, o // 16:(o + S) // 16]
                        dl = dstloc_sb[:, o // 128:(o + S) // 128]
                        g = gp.tile([128, maxS // 128, elem], bf16, tag="g",
                                    bufs=max(2, depth * F // elem))
                        if "gather" not in skips:
                            if int(os.environ.get("GIN_TMODE", "0")):
                                gt = gp.tile([128, maxS], bf16, tag="gt",
                                             bufs=2)
                                nc.gpsimd.dma_gather(
                                    gt[:, :S], sources[q], gi, S, S, elem,
                                    transpose=True,
                                    single_packet=single_packet,
                                    queue_num=((b * 4 + q) if qspread
                                               else q) % nqueues)
                            elif indirect:
                                from concourse.bass import IndirectOffsetOnAxis
                                nc.gpsimd.indirect_dma_start(
                                    out=g[:, : S // 128, :],
                                    out_offset=None,
                                    in_=sources[q],
                                    in_offset=IndirectOffsetOnAxis(
                                        ap=gidx32_sb[
                                            :, o // 128:(o + S) // 128],
                                        axis=0),
                                )
                            else:
                                nc.gpsimd.dma_gather(
                                    g[:, : S // 128, :], sources[q], gi,
                                    S, S, elem, single_packet=single_packet,
                                    queue_num=((b * 4 + q) if qspread
                                               else q) % nqueues)
                        oh = None
                        if "oh" not in skips:
                            glist = groups_bq[(b, q)]
                            nohc = (glist[-1][4] + glist[-1][2] - glist[-1][1]
                                    if glist else 0)
                            oh = ohp.tile([128, maxOHC, 128],
                                          oh_dt if oh_host else bf16, tag="oh")
                            od = b * 4 + q
                            if oh_host:
                                if nohc:
                                    # host-precomputed one-hot planes; ACT
                                    # (HWDGE) queue keeps sync free for stores
                                    nc.scalar.dma_start(
                                        oh[:, :nohc, :],
                                        ohin_t.ap()[:, od * maxOHC:
                                                    od * maxOHC + nohc, :])
                            elif oh_reuse and not is_first:
                                if nohc:
                                    # ACT-queue DMA keeps the sync queue free
                                    # for stores while the gather stream runs
                                    nc.scalar.dma_start(
                                        oh[:, :nohc, :],
                                        oh_dram[:][:, od * maxOHC:
                                                   od * maxOHC + nohc, :])
                            else:
                                swap = int(os.environ.get("GIN_OH_SWAP", "0"))
                                for (t, c0, c1, lvl, op_) in glist:
                                    n = c1 - c0
                                    ins = [
                                        iota[:, lvl * 128:
                                             (lvl + 1) * 128].unsqueeze(
                                            1).broadcast_to([128, n, 128]),
                                        dl[:, c0:c1].unsqueeze(
                                            2).broadcast_to([128, n, 128]),
                                    ]
                                    nc.vector.tensor_tensor(
                                        out=oh[:, op_:op_ + n, :],
                                        in0=ins[swap], in1=ins[1 - swap],
                                        op=OP.is_equal)
                                if oh_reuse and is_first and nohc:
                                    nc.sync.dma_start(
                                        oh_dram[:][:, od * maxOHC:
                                                   od * maxOHC + nohc, :],
                                        oh[:, :nohc, :])
                        G.append(g); OH.append(oh)

                    if "mm" in skips:
                        continue
                    for t in tiles:
                        # full-column pieces; level-shifted one-hots zero out
                        # the other groups' rows in shared columns
                        pieces = []          # (q, col, ohcol)
                        for q in range(4):
                            for (tt, c0, c1, lvl, op_) in groups_bq[(b, q)]:
                                if tt != t:
                                    continue
                                for j in range(c1 - c0):
                                    pieces.append((q, c0 + j, op_ + j))
                        rows = 128 if t < NT - 1 else cfg.last_rows
                        nmm = len(pieces) + (1 if noself else 0)
                        if nmm == 0:
                            continue
                        agg = aggps.tile([128, 128], f32, tag="agg")
                        k = 0
                        if noself:
                            own = sp.tile([128, 128], bf16, tag="own", bufs=4)
                            nc.scalar.dma_start(own[:rows, :],
                                                own_src(t, rows))
                            nc.tensor.matmul(
                                out=agg[:], lhsT=own[:rows, :],
                                rhs=identb[:rows, :],
                                start=True, stop=(nmm == 1))
                            k += 1
                        for (q, col, ohcol) in pieces:
                            nc.tensor.matmul(
                                out=agg[:],
                                lhsT=G[q][:, col, :],
                                rhs=OH[q][:, ohcol, :],
                                start=(k == 0),
                                stop=(k == nmm - 1))
                            k += 1

                        aggT = wp.tile([128, 128], f32, tag="aggT")
                        nc.scalar.activation(out=aggT[:], in_=agg[:], func=AT.Copy)

                        if skip_mlp:
                            continue
                        if is_first:
                            ps1 = mmps.tile([128, 128], f32, tag="mm")
                            nc.tensor.matmul(out=ps1[:], lhsT=w1[:], rhs=aggT[:],
                                             start=True, stop=True)
                            h1 = wp.tile([128, 128], f32, tag="h1")
                            nc.scalar.activation(out=h1[:], in_=ps1[:],
                                                 func=AT.Relu, bias=b1[:])
                            ps2 = mmps.tile([128, 128], f32, tag="mm")
                            nc.tensor.matmul(out=ps2[:], lhsT=w2[:], rhs=h1[:],
                                             start=True, stop=True)
                            h2 = wp.tile([128, 128], f32, tag="h2")
                            nc.scalar.activation(out=h2[:], in_=ps2[:],
                                                 func=AT.Relu, bias=b2[:])
                            # transpose back to [nodes, feat], cast bf16, store
                            pst = mmps.tile([128, 128], f32, tag="mm")
                            nc.tensor.transpose(out=pst[:], in_=h2[:],
                                                identity=ident[:])
                            hbf = sp.tile([128, 128], bf16, tag="hbf")
                            nc.vector.tensor_copy(out=hbf[:], in_=pst[:])
                            nc.sync.dma_start(
                                h_own[:][t * 128: t * 128 + rows, :],
                                hbf[:rows, :])
                            # fire AllGathers as soon as their rows are done
                            for s in range(4):
                                if not ag_emitted[s] and \
                                        (t + 1) * 128 >= (s + 1) * cfg.QROWS:
                                    ag_emitted[s] = True
                                    if no_ag:
                                        continue
                                    nc.gpsimd.collective_compute(
                                        "AllGather", OP.bypass,
                                        replica_groups=[list(range(NCORES))],
                                        ins=[h_own[:][s * cfg.QROWS:
                                                      (s + 1) * cfg.QROWS, :]],
                                        outs=[h_ag[s][:]])
                        else:
                            ps1 = mmps.tile([128, 128], f32, tag="mm")
                            nc.tensor.matmul(out=ps1[:], lhsT=w3[:], rhs=aggT[:],
                                             start=True, stop=True)
                            h3 = wp.tile([128, 128], f32, tag="h1")
                            nc.scalar.activation(out=h3[:], in_=ps1[:],
                                                 func=AT.Relu, bias=b3[:])
                            ps2 = mmps.tile([128, 128], f32, tag="mm")
                            nc.tensor.matmul(out=ps2[:CLS, :128],
                                             lhsT=w4[:], rhs=h3[:],
                                             start=True, stop=True)
                            c4 = sp.tile([CLS, 128], f32, tag="c4")
                            nc.vector.tensor_tensor(
                                out=c4[:], in0=ps2[:CLS, :128],
                                in1=b4[:].broadcast_to([CLS, 128]),
                                op=OP.add)
                            psf = mmps.tile([128, 128], f32, tag="mm")
                            nc.tensor.transpose(out=psf[:128, :CLS], in_=c4[:],
                                                identity=ident[:CLS, :CLS])
                            mx = sp.tile([128, 1], f32, tag="mx")
                            nc.vector.tensor_reduce(
                                out=mx[:], in_=psf[:128, :CLS],
                                axis=mybir.AxisListType.X, op=OP.max)
                            tsh = sp.tile([128, CLS], f32, tag="tsh")
                            nc.vector.tensor_tensor(
                                out=tsh[:], in0=psf[:128, :CLS],
                                in1=mx[:].broadcast_to([128, CLS]),
                                op=OP.subtract)
                            esum = sp.tile([128, 1], f32, tag="esum")
                            edum = sp.tile([128, CLS], f32, tag="edum")
                            nc.scalar.activation(out=edum[:], in_=tsh[:],
                                                 func=AT.Exp, accum_out=esum[:])
                            lse = sp.tile([128, 1], f32, tag="lse")
                            nc.scalar.activation(out=lse[:], in_=esum[:],
                                                 func=AT.Ln)
                            osb = sp.tile([128, CLS], f32, tag="osb")
                            nc.vector.tensor_tensor(
                                out=osb[:], in0=tsh[:],
                                in1=lse[:].broadcast_to([128, CLS]),
                                op=OP.subtract)
                            nc.sync.dma_start(
                                out_t.ap()[t * 128: t * 128 + rows, :],
                                osb[:rows, :])

            l1b = int(os.environ.get("GIN_L1_BATCHES", "0"))
            repeat = int(os.environ.get("GIN_REPEAT", "1"))

            def own1(t, rows):
                return xown_t.ap()[t * 128: t * 128 + rows, :]

            def own2(t, rows):
                return h_own[:][t * 128: t * 128 + rows, :]

            for _rep in range(repeat):
                if l1b:
                    layer([x.ap() for x in xq], is_first=True,
                          batch_limit=l1b, skip_mlp=True, own_src=own1)
                else:
                    layer([x.ap() for x in xq], is_first=True, own_src=own1)
                    if no_ag:
                        layer([x.ap() for x in xq], is_first=False,
                              own_src=own1)
                    else:
                        layer([h[:] for h in h_ag], is_first=False,
                              own_src=own2)

    nc.compile()
    return nc


#  v2: SBUF-resident source tables + transpose-mode SBUF gathers.
#
#  The gather microbench shows SBUF-source dma_gather at ~1.5 ns/desc vs
#  ~3.7 ns/desc for HBM-source (random 256B reads pay an HBM penalty that
#  SRAM does not). v2 therefore keeps the gather sources in SBUF:
#    - Nodes are padded per-core to 12544 (=98*128) "virtual" rows; the
#      node table is split into 4 quarters of 25088 rows. One quarter
#      (49 KiB/partition fp16, wrapped layout: row i -> partition i%128,
#      rank i//128) is resident per pass; each layer runs 4 passes.
#    - SBUF-source gather is transpose-only: output is [feat, slots]. Each
#      128-slot piece is PE-transposed back to [slots, feat] (fp16, PSUM),
#      copied to SBUF (DVE/ACT alternating), then the usual one-hot matmul
#      accumulates agg[feat, dst] in PSUM; pass results accumulate into an
#      SBUF agg buffer (fp16) since PSUM cannot hold all 98 tiles.
#    - The self term rides along as host-injected self-edges (SPMD-uniform;
#      +6% slots but no per-core table indexing).
#    - One-hot planes are host-precomputed (fp8) and streamed from DRAM.
#    - Layer 1 tables load from host-prepared partition-major inputs (one
#      64KB-descriptor DMA per partition); layer 2 tables come from 4
#      range-AllGathers of the partition-major h buffer.

def _npcv(cfg):
    return cfg.NT * 128          # virtual rows per core (full: 12544)


def _sched_from_counts(counts, NT, pad, B):
    """Shared scheduling: per-(tile,quarter) slot padding, batches, call
    offsets, level assignment. counts: [NCORES, NT*4]."""
    cmax = counts.max(axis=0)
    slots = -(-cmax // pad) * pad
    batches = [list(range(b, min(b + B, NT))) for b in range(0, NT, B)]
    off = 0
    slot_off = np.zeros(NT * 4, np.int64)
    call_slots, call_off = [], []
    level = np.zeros(NT * 4, np.int64)
    for tiles in batches:
        cs, co = [], []
        for q in range(4):
            co.append(off)
            s0 = off
            for i, t in enumerate(tiles):
                level[t * 4 + q] = i % 4
                slot_off[t * 4 + q] = off
                off += slots[t * 4 + q]
            off = -(-off // 128) * 128
            cs.append(off - s0)
        call_slots.append(cs)
        call_off.append(co)
    tot = off
    assert tot % 128 == 0
    return dict(slots=slots, batches=batches, call_slots=call_slots,
                call_off=call_off, slot_off=slot_off, tot=tot, level=level,
                noself=True)


def _prep_v2(edge_index, cfg):
    """v2 host prep: self-edges injected, virtual-row quartering, per-core
    gather indices + dstloc (for host one-hots)."""
    N, NPC, NT = cfg.N, cfg.NPC, cfg.NT
    NPCV = _npcv(cfg)
    QRV = 2 * NPCV
    pad = int(os.environ.get("GIN_PAD", "32"))
    B = cfg.B
    src = np.asarray(edge_index[0], dtype=np.int64)
    dst = np.asarray(edge_index[1], dtype=np.int64)
    # self term handled by an identity matmul on the own-rows slice
    v = (src // NPC) * NPCV + (src % NPC)      # virtual source row
    vq = v // QRV
    iq = (v % QRV).astype(np.int32)

    core = dst // NPC
    per_core = []
    counts = np.zeros((NCORES, NT * 4), np.int64)
    for c in range(NCORES):
        m = core == c
        dl = dst[m] - c * NPC
        t = dl >> 7
        gid = (t * 4 + vq[m]).astype(np.int64)
        counts[c] = np.bincount(gid, minlength=NT * 4)
        per_core.append((gid, iq[m], (dl & 127).astype(np.int32)))

    sched = _sched_from_counts(counts, NT, pad, B)
    slot_off, level, tot = sched["slot_off"], sched["level"], sched["tot"]

    gidx_all, dstloc_all = [], []
    for c in range(NCORES):
        gid, gidxv, dstin = per_core[c]
        order = np.argsort(gid, kind="stable")
        gs = gid[order]
        cnt = counts[c]
        starts = np.zeros(NT * 4, np.int64)
        np.cumsum(cnt[:-1], out=starts[1:])
        rank = np.arange(len(gs)) - starts[gs]
        slot = slot_off[gs] + rank
        gflat = np.zeros(tot, np.int16)
        dflat = np.full(tot, 2000.0, np.float32)
        gflat[slot] = gidxv[order].astype(np.int16)
        dflat[slot] = dstin[order] + 128.0 * level[gs]
        gwr = np.tile(gflat.reshape(tot // 16, 16).T, (8, 1)).copy()
        dloc = dflat.reshape(tot // 128, 128).T.astype(F16).copy()
        gidx_all.append(gwr)
        dstloc_all.append(dloc)
    return sched, gidx_all, dstloc_all


def _own_pm(x, cfg):
    """Per-core own rows, wrapped partition-major [128, NT, F] fp16."""
    NPCV = _npcv(cfg)
    out = []
    for c in range(NCORES):
        xv = np.zeros((NPCV, x.shape[1]), F16)
        xv[:cfg.NPC] = x[c * cfg.NPC:(c + 1) * cfg.NPC]
        out.append(np.ascontiguousarray(
            xv.reshape(cfg.NT, 128, -1).transpose(1, 0, 2)))
    return out


def _perm_v2(x, cfg):
    """x [N,128] -> 4 partition-major quarter tables [128, ranks, 128]."""
    NPCV = _npcv(cfg)
    QRV = 2 * NPCV
    xv = np.zeros((NCORES, NPCV, x.shape[1]), F16)
    xv[:, :x.shape[0] // NCORES] = x.reshape(NCORES, -1, x.shape[1])
    xall = xv.reshape(NCORES * NPCV, x.shape[1])
    out = []
    for q in range(4):
        quarter = xall[q * QRV:(q + 1) * QRV]
        out.append(np.ascontiguousarray(
            quarter.reshape(QRV // 128, 128, -1).transpose(1, 0, 2)))
    return out


def _build_nc_v2(cfg, sched, eps1, eps2):
    from concourse import mybir
    import concourse.bacc as bacc
    import concourse.tile as tile

    F, H, CLS, NT, NPC = cfg.F, cfg.H, cfg.CLS, cfg.NT, cfg.NPC
    batches = sched["batches"]
    call_slots = sched["call_slots"]
    call_off = sched["call_off"]
    tot = sched["tot"]
    f32 = mybir.dt.float32
    f16 = mybir.dt.float16
    f8 = mybir.dt.float8e4
    AT = mybir.ActivationFunctionType
    OP = mybir.AluOpType
    assert eps1 == 0.0 and eps2 == 0.0

    groups_bq, maxOHC = _groups(sched)
    nqueues = int(os.environ.get("GIN_QUEUES", "4"))
    single_packet = bool(int(os.environ.get("GIN_SP", "0")))
    scratch = int(os.environ.get("GIN_SCRATCH", "32768"))
    nc = bacc.Bacc("TRN2", target_bir_lowering=False, debug=False,
                   num_devices=NCORES, num_swdge_queues=nqueues,
                   dynamic_dma_scratch_size=scratch)

    RANKS = 2 * NT
    xpm = [nc.dram_tensor(f"xpm{q}", [128, RANKS, F], f16,
                          kind="ExternalInput") for q in range(4)]
    w1_t = nc.dram_tensor("w1", [F, H], f16, kind="ExternalInput")
    w2_t = nc.dram_tensor("w2", [H, H], f16, kind="ExternalInput")
    w3_t = nc.dram_tensor("w3", [H, H], f16, kind="ExternalInput")
    w4_t = nc.dram_tensor("w4", [H, CLS], f16, kind="ExternalInput")
    b1_t = nc.dram_tensor("b1", [H, 1], f32, kind="ExternalInput")
    b2_t = nc.dram_tensor("b2", [H, 1], f32, kind="ExternalInput")
    b3_t = nc.dram_tensor("b3", [H, 1], f32, kind="ExternalInput")
    b4_t = nc.dram_tensor("b4", [CLS, 1], f32, kind="ExternalInput")
    id16_t = nc.dram_tensor("id16", [128, 128], f16, kind="ExternalInput")
    id32_t = nc.dram_tensor("id32", [128, 128], f32, kind="ExternalInput")
    gidx_t = nc.dram_tensor("gidx", [128, tot // 16], mybir.dt.int16,
                            kind="ExternalInput")
    xown_t = nc.dram_tensor("xown", [128, NT, F], f16, kind="ExternalInput")
    ncalls_all = len(batches) * 4
    oh_dt = {"fp8": f8, "fp16": f16}[os.environ.get("GIN_OH_DT", "fp16")]
    ohin_t = nc.dram_tensor("ohin", [128, ncalls_all * maxOHC, 128], oh_dt,
                            kind="ExternalInput")
    out_t = nc.dram_tensor("out", [NPC, CLS], f32, kind="ExternalOutput")

    maxS = max(max(cs) for cs in call_slots)
    NRL = NT                 # ranks per core chunk
    step = -(-NRL // 4)
    AG_RANGES = [(lo, min(lo + step, NRL)) for lo in range(0, NRL, step)]
    last_rows = cfg.last_rows

    with tile.TileContext(nc) as tc:
        with tc.tile_pool(name="const", bufs=1) as cp, \
             tc.tile_pool(name="tbl", bufs=2) as tp, \
             tc.tile_pool(name="gp", bufs=2) as gp, \
             tc.tile_pool(name="ohp", bufs=3) as ohp, \
             tc.tile_pool(name="gsb", bufs=8) as gsp, \
             tc.tile_pool(name="work", bufs=3) as wp, \
             tc.tile_pool(name="small", bufs=4) as sp, \
             tc.tile_pool(name="tps", bufs=4, space="PSUM") as tps, \
             tc.tile_pool(name="aggps", bufs=2, space="PSUM") as aggps, \
             tc.tile_pool(name="mmps", bufs=2, space="PSUM") as mmps, \
             tc.tile_pool(name="dram", bufs=1, space="DRAM") as dp:

            w1 = cp.tile([F, H], f16); nc.sync.dma_start(w1[:], w1_t.ap())
            w2 = cp.tile([H, H], f16); nc.sync.dma_start(w2[:], w2_t.ap())
            w3 = cp.tile([H, H], f16); nc.sync.dma_start(w3[:], w3_t.ap())
            w4 = cp.tile([H, CLS], f16); nc.sync.dma_start(w4[:], w4_t.ap())
            b1 = cp.tile([H, 1], f32); nc.sync.dma_start(b1[:], b1_t.ap())
            b2 = cp.tile([H, 1], f32); nc.sync.dma_start(b2[:], b2_t.ap())
            b3 = cp.tile([H, 1], f32); nc.sync.dma_start(b3[:], b3_t.ap())
            b4 = cp.tile([CLS, 1], f32); nc.sync.dma_start(b4[:], b4_t.ap())
            id16 = cp.tile([128, 128], f16)
            nc.sync.dma_start(id16[:], id16_t.ap())
            id32 = cp.tile([128, 128], f32)
            nc.sync.dma_start(id32[:], id32_t.ap())
            gidx_sb = cp.tile([128, tot // 16], mybir.dt.int16)
            nc.sync.dma_start(gidx_sb[:], gidx_t.ap())
            zt = cp.tile([128, 128], f16)
            nc.vector.memset(zt[:], 0.0)

            agg_sb = cp.tile([128, NT, 128], f16)
            # one contiguous DRAM buffer per AllGather range (collective
            # inputs must be contiguous; a column slice of one big buffer
            # fails BIR verification)
            h_own = [dp.tile([128, (hi - lo) * 128], f16, name=f"hown{j}")
                     for j, (lo, hi) in enumerate(AG_RANGES)]

            def own_rng(t):
                for j, (lo, hi) in enumerate(AG_RANGES):
                    if lo <= t < hi:
                        return j, lo
                raise AssertionError(t)
            ag_space = ("Shared" if int(os.environ.get("GIN_AG_SHARED", "0"))
                        else "Local")
            h_ag = [dp.tile([NCORES * 128, (hi - lo) * 128], f16,
                            addr_space=ag_space, name=f"hag{j}")
                    for j, (lo, hi) in enumerate(AG_RANGES)]

            def load1(qq, table):
                nc.sync.dma_start(table[:], xpm[qq].ap())

            def load2(qq, table):
                for j, (lo, hi) in enumerate(AG_RANGES):
                    for k in range(2):
                        core = 2 * qq + k
                        nc.sync.dma_start(
                            table[:, k * NRL + lo: k * NRL + hi, :],
                            h_ag[j][:][core * 128:(core + 1) * 128, :])

            def layer(loader, own_of, is_first):
                for qq in range(4):
                    table = tp.tile([128, RANKS, F], f16, tag="tbl")
                    loader(qq, table)
                    for b, tiles in enumerate(batches):
                        S = call_slots[b][qq]
                        o = call_off[b][qq]
                        gt = None
                        oh = None
                        if S:
                            gt = gp.tile([128, 1, maxS], f16, tag="gt")
                            nc.gpsimd.dma_gather(
                                gt[:, :, :S], table[:],
                                gidx_sb[:, o // 16:(o + S) // 16], S, S, F,
                                transpose=True, single_packet=single_packet,
                                queue_num=b % nqueues,
                                sbuf_tokens_per_rank=128,
                                sbuf_free_dim_per_rank=256)
                            glist = groups_bq[(b, qq)]
                            nohc = (glist[-1][4] + glist[-1][2] - glist[-1][1]
                                    if glist else 0)
                            if nohc:
                                oh = ohp.tile([128, maxOHC, 128], oh_dt,
                                              tag="oh")
                                od = b * 4 + qq
                                nc.scalar.dma_start(
                                    oh[:, :nohc, :],
                                    ohin_t.ap()[:, od * maxOHC:
                                                od * maxOHC + nohc, :])
                        for t in tiles:
                            pieces = []
                            for (tt, c0, c1, lvl, op_) in groups_bq[(b, qq)]:
                                if tt != t:
                                    continue
                                for j in range(c1 - c0):
                                    pieces.append((c0 + j, op_ + j))
                            if qq > 0 and not pieces:
                                continue
                            # transpose all pieces FIRST (plain matmuls vs
                            # identity), then run the agg accumulation group
                            # contiguously: interleaving start=True matmuls
                            # into an open PSUM accumulation group corrupts
                            # partials on HW (bank-granular start-zeroing)
                            # even though CoreSim accepts it.
                            g2s = []
                            for k, (col, ohcol) in enumerate(pieces):
                                pst = tps.tile([128, 128], f32, tag="pst")
                                nc.tensor.matmul(
                                    out=pst[:],
                                    lhsT=gt[:, 0, col * 128:(col + 1) * 128],
                                    rhs=id16[:], start=True, stop=True)
                                g2 = gsp.tile([128, 128], f16, tag="g2")
                                if k % 2 == 0:
                                    nc.vector.tensor_copy(out=g2[:],
                                                          in_=pst[:])
                                else:
                                    nc.scalar.activation(out=g2[:],
                                                         in_=pst[:],
                                                         func=AT.Copy)
                                g2s.append((g2, ohcol))
                            agg = aggps.tile([128, 128], f32, tag="agg")
                            kbase = 0
                            if qq == 0:
                                # self term: (1+eps)*h_t via identity matmul
                                own = sp.tile([128, 128], f16, tag="own")
                                nc.scalar.dma_start(own[:], own_of(t))
                                nc.tensor.matmul(
                                    out=agg[:], lhsT=own[:], rhs=id16[:],
                                    start=True, stop=(len(pieces) == 0))
                                kbase = 1
                            for k, (g2, ohcol) in enumerate(g2s):
                                nc.tensor.matmul(
                                    out=agg[:], lhsT=g2[:],
                                    rhs=oh[:, ohcol, :],
                                    start=(kbase == 0 and k == 0),
                                    stop=(k == len(g2s) - 1))
                            if qq == 0:
                                nc.vector.tensor_copy(out=agg_sb[:, t, :],
                                                      in_=agg[:])
                            else:
                                nc.vector.tensor_tensor(
                                    out=agg_sb[:, t, :], in0=agg[:],
                                    in1=agg_sb[:, t, :], op=OP.add)

                # MLP over all tiles from the SBUF agg buffer
                for t in range(NT):
                    rows = 128 if t < NT - 1 else last_rows
                    if is_first:
                        ps1 = mmps.tile([128, 128], f32, tag="mm")
                        nc.tensor.matmul(out=ps1[:], lhsT=w1[:],
                                         rhs=agg_sb[:, t, :],
                                         start=True, stop=True)
                        h1 = wp.tile([128, 128], f16, tag="h1")
                        nc.scalar.activation(out=h1[:], in_=ps1[:],
                                             func=AT.Relu, bias=b1[:])
                        ps2 = mmps.tile([128, 128], f32, tag="mm")
                        nc.tensor.matmul(out=ps2[:], lhsT=w2[:], rhs=h1[:],
                                         start=True, stop=True)
                        h2 = wp.tile([128, 128], f16, tag="h2")
                        nc.scalar.activation(out=h2[:], in_=ps2[:],
                                             func=AT.Relu, bias=b2[:])
                        pst = tps.tile([128, 128], f32, tag="pst")
                        nc.tensor.matmul(out=pst[:], lhsT=h2[:], rhs=id16[:],
                                         start=True, stop=True)
                        hst = sp.tile([128, 128], f16, tag="hst")
                        nc.vector.tensor_copy(out=hst[:], in_=pst[:])
                        jr, lo_r = own_rng(t)
                        nc.sync.dma_start(
                            h_own[jr][:][:, (t - lo_r) * 128:
                                         (t - lo_r + 1) * 128], hst[:])
                        if t == NT - 1 and last_rows < 128:
                            # zero the pad rows (virtual >= NPC)
                            nc.sync.dma_start(
                                h_own[jr][:][last_rows:128,
                                             (t - lo_r) * 128:
                                             (t - lo_r + 1) * 128],
                                zt[:128 - last_rows, :])
                        for j, (lo, hi) in enumerate(AG_RANGES):
                            if t == hi - 1:
                                nc.gpsimd.collective_compute(
                                    "AllGather", OP.bypass,
                                    replica_groups=[list(range(NCORES))],
                                    ins=[h_own[j][:]],
                                    outs=[h_ag[j][:]])
                    else:
                        ps1 = mmps.tile([128, 128], f32, tag="mm")
                        nc.tensor.matmul(out=ps1[:], lhsT=w3[:],
                                         rhs=agg_sb[:, t, :],
                                         start=True, stop=True)
                        h3 = wp.tile([128, 128], f16, tag="h1")
                        nc.scalar.activation(out=h3[:], in_=ps1[:],
                                             func=AT.Relu, bias=b3[:])
                        ps2 = mmps.tile([128, 128], f32, tag="mm")
                        nc.tensor.matmul(out=ps2[:CLS, :128],
                                         lhsT=w4[:], rhs=h3[:],
                                         start=True, stop=True)
                        c4 = sp.tile([CLS, 128], f32, tag="c4")
                        nc.vector.tensor_tensor(
                            out=c4[:], in0=ps2[:CLS, :128],
                            in1=b4[:].broadcast_to([CLS, 128]), op=OP.add)
                        psf = mmps.tile([128, 128], f32, tag="mm")
                        nc.tensor.transpose(out=psf[:128, :CLS], in_=c4[:],
                                            identity=id32[:CLS, :CLS])
                        mx = sp.tile([128, 1], f32, tag="mx")
                        nc.vector.tensor_reduce(
                            out=mx[:], in_=psf[:128, :CLS],
                            axis=mybir.AxisListType.X, op=OP.max)
                        tsh = sp.tile([128, CLS], f32, tag="tsh")
                        nc.vector.tensor_tensor(
                            out=tsh[:], in0=psf[:128, :CLS],
                            in1=mx[:].broadcast_to([128, CLS]),
                            op=OP.subtract)
                        esum = sp.tile([128, 1], f32, tag="esum")
                        edum = sp.tile([128, CLS], f32, tag="edum")
                        nc.scalar.activation(out=edum[:], in_=tsh[:],
                                             func=AT.Exp, accum_out=esum[:])
                        lse = sp.tile([128, 1], f32, tag="lse")
                        nc.scalar.activation(out=lse[:], in_=esum[:],
                                             func=AT.Ln)
                        osb = sp.tile([128, CLS], f32, tag="osb")
                        nc.vector.tensor_tensor(
                            out=osb[:], in0=tsh[:],
                            in1=lse[:].broadcast_to([128, CLS]),
                            op=OP.subtract)
                        nc.sync.dma_start(
                            out_t.ap()[t * 128: t * 128 + rows, :],
                            osb[:rows, :])

            repeat = int(os.environ.get("GIN_REPEAT", "1"))
            def own2(t):
                j, lo = own_rng(t)
                return h_own[j][:][:, (t - lo) * 128:(t - lo + 1) * 128]

            for _rep in range(repeat):
                layer(load1, lambda t: xown_t.ap()[:, t, :], is_first=True)
                layer(load2, own2, is_first=False)

    nc.compile()
    return nc


def _run_v2(inputs, cfg):
    from concourse.bass_utils import run_bass_kernel_spmd

    # v2 needs smaller tile batches than v1 to fit its SBUF tables
    cfg = Cfg(cfg.N, cfg.F, cfg.H, cfg.CLS,
              tiles_per_batch=int(os.environ.get("GIN_B2", "4")))

    x = np.asarray(inputs["x"], np.float32)
    edge_index = np.asarray(inputs["edge_index"])
    eps1 = float(np.asarray(inputs["eps1"]))
    eps2 = float(np.asarray(inputs["eps2"]))

    sched, gidx_all, dstloc_all = _prep_v2(edge_index, cfg)
    nc = _build_nc_v2(cfg, sched, eps1, eps2)

    xq = _perm_v2(x.astype(F16), cfg)
    np_ohdt = {"fp8": ml_dtypes.float8_e4m3,
               "fp16": np.float16}[os.environ.get("GIN_OH_DT", "fp16")]
    base = {
        "w1": np.asarray(inputs["w1"], np.float16),
        "w2": np.asarray(inputs["w2"], np.float16),
        "w3": np.asarray(inputs["w3"], np.float16),
        "w4": np.asarray(inputs["w4"], np.float16),
        "b1": np.asarray(inputs["b1"], np.float32).reshape(-1, 1),
        "b2": np.asarray(inputs["b2"], np.float32).reshape(-1, 1),
        "b3": np.asarray(inputs["b3"], np.float32).reshape(-1, 1),
        "b4": np.asarray(inputs["b4"], np.float32).reshape(-1, 1),
        "id16": np.eye(128, dtype=np.float16),
        "id32": np.eye(128, dtype=np.float32),
    }
    for q in range(4):
        base[f"xpm{q}"] = xq[q]
    xown = _own_pm(x.astype(F16), cfg)

    in_maps = []
    for c in range(NCORES):
        m = dict(base)
        m["gidx"] = gidx_all[c]
        m["xown"] = xown[c]
        m["ohin"] = _host_onehots(sched, dstloc_all[c], np_ohdt)
        in_maps.append(m)

    bench = int(os.environ.get("GIN_BENCH", "0"))
    trace = int(os.environ.get("GIN_TRACE", "0"))
    if bench:
        results = _exec_spmd_timed(nc, in_maps, bench)
    elif trace:
        res = run_bass_kernel_spmd(nc, in_maps, core_ids=list(range(NCORES)),
                                   trace=True,
                                   tmpdir=os.environ.get("GIN_TRACE_DIR"))
        print("TRACE exec_time_ns:", res.exec_time_ns)
        results = res.results
    else:
        res = run_bass_kernel_spmd(nc, in_maps, core_ids=list(range(NCORES)))
        results = res.results
    out = np.concatenate([r["out"] for r in results], axis=0)
    return out.astype(np.float32)


LAST_BENCH_NS = None
DEFERRED = []


def _exec_spmd_timed(nc, in_maps, iters):
    """jit-once SPMD exec (mirrors bass2jax.run_bass_via_pjrt) + steady-state
    timing of repeated NEFF executions. Returns per-core result dicts."""
    import time

    import jax
    import numpy as _np
    from jax.experimental.shard_map import shard_map
    from jax.sharding import Mesh, NamedSharding, PartitionSpec

    from concourse import bass2jax, mybir
    from concourse.bass2jax import _bass_exec_p, install_neuronx_cc_hook, \
        partition_id_tensor

    install_neuronx_cc_hook()
    n_cores = len(in_maps)
    partition_name = (nc.partition_id_tensor.name
                      if nc.partition_id_tensor else None)
    in_names, out_names, out_avals, zero_outs = [], [], [], []
    for alloc in nc.m.functions[0].allocations:
        if not isinstance(alloc, mybir.MemoryLocationSet):
            continue
        name = alloc.memorylocations[0].name
        if alloc.kind == "ExternalInput":
            if name != partition_name:
                in_names.append(name)
        elif alloc.kind == "ExternalOutput":
            out_names.append(name)
            shape = tuple(alloc.tensor_shape)
            dtype = mybir.dt.np(alloc.dtype)
            out_avals.append(jax.core.ShapedArray(shape, dtype))
            zero_outs.append(_np.zeros(shape, dtype))
    n_params = len(in_names)
    all_in_names = in_names + out_names
    if partition_name is not None:
        all_in_names = all_in_names + [partition_name]

    chain = int(os.environ.get("GIN_CHAIN", "1"))

    def _body(*args):
        ins = list(args[:n_params])
        outs = list(args[n_params:])
        for _ in range(chain):
            operands = ins + outs
            if partition_name is not None:
                operands.append(partition_id_tensor())
            outs = list(_bass_exec_p.bind(
                *operands,
                out_avals=tuple(out_avals),
                in_names=tuple(all_in_names),
                out_names=tuple(out_names),
                lowering_input_output_aliases=(),
                sim_require_finite=True,
                sim_require_nnan=True,
                nc=nc,
            ))
        return tuple(outs)

    devices = jax.devices()[:n_cores]
    mesh = Mesh(_np.asarray(devices), ("core",))
    spec = PartitionSpec("core")
    n_outs = len(out_names)
    fn = jax.jit(
        shard_map(_body, mesh=mesh, in_specs=(spec,) * (n_params + n_outs),
                  out_specs=(spec,) * n_outs, check_rep=False),
        keep_unused=True,
    )
    sh = NamedSharding(mesh, spec)
    concat_in = [
        jax.device_put(
            _np.concatenate([_np.asarray(in_maps[c][nm]) for c in
                             range(n_cores)], axis=0), sh)
        for nm in in_names
    ]
    concat_zeros = [
        jax.device_put(_np.zeros((n_cores * z.shape[0], *z.shape[1:]),
                                 z.dtype), sh)
        for z in zero_outs
    ]
    out = jax.block_until_ready(fn(*concat_in, *concat_zeros))
    if os.environ.get("GIN_DEFER"):
        DEFERRED.append((fn, concat_in, concat_zeros))
        times = [0.0]
    else:
        times = []
        for _ in range(max(iters, 1)):
            t0 = time.perf_counter()
            r = jax.block_until_ready(fn(*concat_in, *concat_zeros))
            times.append((time.perf_counter() - t0) * 1e9)
    global LAST_BENCH_NS
    LAST_BENCH_NS = times
    res = []
    for c in range(n_cores):
        res.append({
            nm: _np.asarray(out[i]).reshape(n_cores, *out_avals[i].shape)[c]
            for i, nm in enumerate(out_names)
        })
    return res


def _run(inputs, cfg):
    from concourse.bass_utils import run_bass_kernel_spmd

    x = np.asarray(inputs["x"], np.float32)
    edge_index = np.asarray(inputs["edge_index"])
    eps1 = float(np.asarray(inputs["eps1"]))
    eps2 = float(np.asarray(inputs["eps2"]))

    sched, gidx_all, dstloc_all = _prep_graph(edge_index, cfg)
    xqs = _perm_rows(x.astype(F16), cfg)

    nc = _build_nc(cfg, sched, eps1, eps2)

    iota_np = np.tile(np.arange(512, dtype=np.float32),
                      (128, 1)).astype(F16)
    ident_np = np.eye(128, dtype=np.float32)
    base = {
        "w1": np.asarray(inputs["w1"], np.float32),
        "w2": np.asarray(inputs["w2"], np.float32),
        "w3": np.asarray(inputs["w3"], np.float32),
        "w4": np.asarray(inputs["w4"], np.float32),
        "b1": np.asarray(inputs["b1"], np.float32).reshape(-1, 1),
        "b2": np.asarray(inputs["b2"], np.float32).reshape(-1, 1),
        "b3": np.asarray(inputs["b3"], np.float32).reshape(-1, 1),
        "b4": np.asarray(inputs["b4"], np.float32).reshape(-1, 1),
        "iota": iota_np,
        "ident": ident_np,
    }
    elem = int(os.environ.get("GIN_ELEM", str(cfg.F)))
    shift = (elem // cfg.F).bit_length() - 1
    for q in range(4):
        base[f"xq{q}"] = np.ascontiguousarray(
            xqs[q].reshape(-1, elem))

    oh_host = bool(int(os.environ.get("GIN_OH_HOST", "0")))
    np_ohdt = {"fp8": ml_dtypes.float8_e4m3,
               "fp16": np.float16}[os.environ.get("GIN_OH_DT", "fp16")]

    xbf = x.astype(F16)
    in_maps = []
    for c in range(NCORES):
        m = dict(base)
        gwr, g32 = gidx_all[c]
        m["gidx"] = gwr >> shift if shift else gwr
        m["gidx32"] = g32 >> shift if shift else g32
        m["dstloc"] = dstloc_all[c]
        m["xown"] = xbf[c * cfg.NPC:(c + 1) * cfg.NPC]
        if oh_host:
            m["ohin"] = _host_onehots(sched, dstloc_all[c], np_ohdt)
        in_maps.append(m)

    bench = int(os.environ.get("GIN_BENCH", "0"))
    trace = int(os.environ.get("GIN_TRACE", "0"))
    if bench:
        results = _exec_spmd_timed(nc, in_maps, bench)
    elif trace:
        tdir = os.environ.get("GIN_TRACE_DIR") or None
        tcores = [int(c) for c in
                  os.environ.get("GIN_TRACE_CORES", "0").split(",")]
        res = run_bass_kernel_spmd(nc, in_maps, core_ids=list(range(NCORES)),
                                   trace=True, tmpdir=tdir,
                                   trace_cores=tcores)
        print("TRACE exec_time_ns:", res.exec_time_ns,
              "mean:", res.mean_exec_time_ns)
        results = res.results
    else:
        res = run_bass_kernel_spmd(nc, in_maps, core_ids=list(range(NCORES)))
        results = res.results
    out = np.concatenate([r["out"] for r in results], axis=0)
    return out.astype(np.float32)


def kernel(**inputs):
    if os.environ.get("GIN_V2", "0") == "1":
        return _run_v2(inputs, FULL)
    return _run(inputs, FULL)

